# revision 1
# baseline (speedup 1.0000x reference)
"""Trainium2 Bass kernel for the DTI R-GCN (bdd) model, 8 NeuronCores.

Strategy (SPMD, one program, per-core data):
  - dst-shard the graph: core c owns nodes [c*2500, (c+1)*2500); host routes
    each edge to its dst owner and buckets it into (dst-tile, relation) cells,
    tiles of 128 dst nodes.
  - per 128-edge chunk: indirect-DMA gather of x[src] rows (1KB each) onto
    partitions, build a scatter matrix S[e, d] = norm_e * (iota == dstloc_e)
    with one tensor_scalar op, and matmul xg^T @ S to accumulate the
    transposed per-(tile, rel) aggregate aggT[feat, dst] in PSUM.  Pad slots
    use an out-of-bounds src index (descriptor skipped -> no DMA traffic) and
    norm 0.
  - per cell: two [128,128]x[128,128] matmuls with the block-diagonal
    relation weight apply W_r and accumulate msg[d, of] in PSUM across all
    16 relations; the self-loop term x_own @ loop_w joins the same PSUM
    accumulation (x_own^T built with PE transposes).
  - layer output (own 2500 rows) is AllGather'd so every core has the full
    [20000, 256] activations for the next layer's gathers.
  - MLP head is data-parallel over pairs (512 per core) after a final
    AllGather of h2.

The program is built fresh per invocation from the actual inputs (host does
all index preprocessing; trip counts are data-dependent but identical across
cores by padding cells to the max chunk count over cores).
"""
import sys

sys.path.insert(0, "/opt/trn_rl_repo")
import numpy as np

P = 128
NCORES = 8
OOB = np.int32(2**28)


def _preprocess(inputs, ncores=NCORES):
    node_ids = np.asarray(inputs["node_ids"])
    src = np.asarray(inputs["src"])
    dst = np.asarray(inputs["dst"])
    etype = np.asarray(inputs["etype"])
    norm = np.asarray(inputs["norm"]).reshape(-1)
    emb = np.asarray(inputs["emb"], dtype=np.float32)
    drugs = np.asarray(inputs["drugs_index"])
    targets = np.asarray(inputs["targets_index"])

    N = node_ids.shape[0]
    H = emb.shape[1]
    R = int(inputs["w1"].shape[0])
    PAIRS = drugs.shape[0]
    assert N % ncores == 0 and PAIRS % ncores == 0
    NOWN = N // ncores
    TILES = -(-NOWN // P)
    NCELL = (-(-NOWN // (2 * P))) * R
    PPC = PAIRS // ncores
    assert PPC % P == 0
    Q = PPC // P

    TILE2 = 2 * P  # dst nodes per aggregation cell (segments on matmul free dim)
    T2 = -(-NOWN // TILE2)
    owner = dst // NOWN
    d_local = dst - owner * NOWN
    t_of_e = d_local // TILE2
    dstloc_of_e = (d_local % TILE2).astype(np.float32)
    cell_of_e = t_of_e * R + etype

    counts = np.zeros((ncores, NCELL), np.int64)
    for c in range(ncores):
        counts[c] = np.bincount(cell_of_e[owner == c], minlength=NCELL)
    nch = -(-counts.max(axis=0) // P)  # chunks per cell (0 if empty everywhere)
    chunk_start = np.zeros(NCELL, np.int64)
    chunk_start[1:] = np.cumsum(nch)[:-1]
    TC = int(nch.sum())

    srcT = np.full((ncores, P, TC), OOB, np.int32)
    dstlocT = np.zeros((ncores, P, TC), np.float32)
    normT = np.zeros((ncores, P, TC), np.float32)
    for c in range(ncores):
        m = owner == c
        eidx = np.where(m)[0]
        cell = cell_of_e[eidx]
        order = np.argsort(cell, kind="stable")
        eidx = eidx[order]
        cell = cell[order]
        cstart = np.zeros(NCELL, np.int64)
        cstart[1:] = np.cumsum(counts[c])[:-1]
        rank = np.arange(len(eidx)) - cstart[cell]
        col = chunk_start[cell] + rank // P
        part = rank % P
        srcT[c, part, col] = src[eidx]
        dstlocT[c, part, col] = dstloc_of_e[eidx]
        normT[c, part, col] = norm[eidx]

    # host-side embedding lookup: pure data movement, shrinks per-core upload
    # from the full table to the active [N, H] slab
    h0 = emb[node_ids]  # [N, H]
    x0own = np.zeros((ncores, TILES * P, H), np.float32)
    for c in range(ncores):
        x0own[c, :NOWN] = h0[c * NOWN : (c + 1) * NOWN]

    drugsT = drugs.reshape(ncores, Q, P).transpose(0, 2, 1).astype(np.int32).copy()
    targetsT = targets.reshape(ncores, Q, P).transpose(0, 2, 1).astype(np.int32).copy()

    # relation block weights as lhsT [if_local, of_local] per (layer, rel, half)
    B = int(inputs["w1"].shape[1])
    si = H // B
    hb = (P // si)  # blocks per half
    wblk = np.zeros((2, R, 2, P, P), np.float32)
    for l, W in enumerate([inputs["w1"], inputs["w2"]]):
        W = np.asarray(W, np.float32)
        for r in range(R):
            for h in range(2):
                for bb in range(hb):
                    b = hb * h + bb
                    wblk[l, r, h, bb * si : (bb + 1) * si, bb * si : (bb + 1) * si] = W[r, b]
    wblk_in = wblk.transpose(3, 0, 1, 2, 4).reshape(P, 2 * R * 2 * P).copy()

    loopw = np.stack(
        [np.asarray(inputs["loop_w1"], np.float32), np.asarray(inputs["loop_w2"], np.float32)]
    )  # [2, H, H]
    loopw_in = loopw.reshape(2, 2, P, H).transpose(2, 0, 1, 3).reshape(P, 2 * 2 * H).copy()

    bias_in = np.concatenate(
        [
            np.tile(np.asarray(inputs["b1"], np.float32)[None, :], (P, 1)),
            np.tile(np.asarray(inputs["b2"], np.float32)[None, :], (P, 1)),
        ],
        axis=1,
    )  # [P, 2H]

    d2 = 2 * H
    KC = d2 // P  # fc1 contraction chunks
    MC = d2 // P  # fc1 output chunks
    fc1_in = (
        np.asarray(inputs["fc1_W"], np.float32)
        .reshape(KC, P, MC, P)
        .transpose(1, 0, 2, 3)
        .reshape(P, KC * MC * P)
        .copy()
    )
    fc1b_in = np.asarray(inputs["fc1_b"], np.float32).reshape(MC, P).T.copy()
    fc2_in = np.asarray(inputs["fc2_W"], np.float32).reshape(MC, P).T.copy()
    fc2b = float(np.asarray(inputs["fc2_b"]).reshape(-1)[0])

    iota = np.tile(np.arange(2 * P, dtype=np.float32), (P, 1))

    meta = dict(
        N=N, H=H, R=R, NOWN=NOWN, TILES=TILES, T2=T2, NCELL=NCELL, TC=TC, Q=Q,
        KC=KC, MC=MC, nch=nch, chunk_start=chunk_start, fc2b=fc2b,
    )
    shared = dict(
        h0=h0, iota=iota, wblk=wblk_in, loopw=loopw_in, biasbc=bias_in,
        fc1=fc1_in, fc1b=fc1b_in, fc2=fc2_in,
    )
    in_maps = []
    for c in range(ncores):
        m = dict(shared)
        m.update(
            srcT=srcT[c], dstlocT=dstlocT[c], normT=normT[c],
            x0own=x0own[c], drugsT=drugsT[c], targetsT=targetsT[c],
        )
        in_maps.append(m)
    return meta, in_maps


def _build(meta, ncores=NCORES, single=False):
    from concourse import bass, mybir, tile, bacc
    from concourse.masks import make_identity

    N, H, R = meta["N"], meta["H"], meta["R"]
    NOWN, TILES, TC, Q = meta["NOWN"], meta["TILES"], meta["TC"], meta["Q"]
    T2 = meta["T2"]
    KC, MC = meta["KC"], meta["MC"]
    nch, chunk_start = meta["nch"], meta["chunk_start"]
    f32 = mybir.dt.float32
    i32 = mybir.dt.int32

    nc = bacc.Bacc(
        "TRN2", target_bir_lowering=False, debug=False,
        num_devices=(1 if single else ncores),
    )

    h0_t = nc.dram_tensor("h0", [N, H], f32, kind="ExternalInput")
    srcT_t = nc.dram_tensor("srcT", [P, TC], i32, kind="ExternalInput")
    dstlocT_t = nc.dram_tensor("dstlocT", [P, TC], f32, kind="ExternalInput")
    normT_t = nc.dram_tensor("normT", [P, TC], f32, kind="ExternalInput")
    x0own_t = nc.dram_tensor("x0own", [TILES * P, H], f32, kind="ExternalInput")
    drugsT_t = nc.dram_tensor("drugsT", [P, Q], i32, kind="ExternalInput")
    targetsT_t = nc.dram_tensor("targetsT", [P, Q], i32, kind="ExternalInput")
    iota_t = nc.dram_tensor("iota", [P, 2 * P], f32, kind="ExternalInput")
    wblk_t = nc.dram_tensor("wblk", [P, 2 * R * 2 * P], f32, kind="ExternalInput")
    loopw_t = nc.dram_tensor("loopw", [P, 2 * 2 * H], f32, kind="ExternalInput")
    biasbc_t = nc.dram_tensor("biasbc", [P, 2 * H], f32, kind="ExternalInput")
    fc1_t = nc.dram_tensor("fc1", [P, KC * MC * P], f32, kind="ExternalInput")
    fc1b_t = nc.dram_tensor("fc1b", [P, MC], f32, kind="ExternalInput")
    fc2_t = nc.dram_tensor("fc2", [P, MC], f32, kind="ExternalInput")
    out_t = nc.dram_tensor("out", [Q * P, 1], f32, kind="ExternalOutput")

    with tile.TileContext(nc) as tc:
        with (
            tc.tile_pool(name="const", bufs=1) as cp,
            tc.tile_pool(name="work", bufs=10) as wp,
            tc.tile_pool(name="ps", bufs=1, space="PSUM") as pp,
        ):
            # ---- DRAM internals; AllGather is split into piece collectives so
            # each piece can fire as soon as its tiles are stored (overlaps the
            # rest of the layer), then one strided DMA folds it into the
            # node-indexed full table.
            tpp = max(1, -(-TILES // 4))  # tiles per AG piece
            pieces = []  # (row0, nrows)
            for p0 in range(0, TILES, tpp):
                row0 = p0 * P
                nrows = min(NOWN, (p0 + tpp) * P) - row0
                if nrows > 0:
                    pieces.append((row0, nrows))
            h1_own = nc.dram_tensor("h1_own", [TILES * P, H], f32, kind="Internal").ap()
            h1_full = nc.dram_tensor("h1_full", [N, H], f32, kind="Internal").ap()
            h2_full = nc.dram_tensor("h2_full", [N, H], f32, kind="Internal").ap()
            agin = {}
            agout = {}
            for li in (1, 2):
                for pi, (row0, nrows) in enumerate(pieces):
                    agin[(li, pi)] = nc.dram_tensor(
                        f"h{li}_agin{pi}", [nrows, H], f32, kind="Internal"
                    ).ap()
                    agout[(li, pi)] = nc.dram_tensor(
                        f"h{li}_agout{pi}", [ncores * nrows, H], f32,
                        kind="Internal", addr_space="Shared",
                    ).ap()

            # ---- resident constants ----
            srcT = cp.tile([P, TC], i32, name="srcT")
            nc.sync.dma_start(srcT[:], srcT_t.ap()[:])
            dstlocT = cp.tile([P, TC], f32, name="dstlocT")
            nc.sync.dma_start(dstlocT[:], dstlocT_t.ap()[:])
            normT = cp.tile([P, TC], f32, name="normT")
            nc.sync.dma_start(normT[:], normT_t.ap()[:])
            drugsT = cp.tile([P, Q], i32, name="drugsT")
            nc.sync.dma_start(drugsT[:], drugsT_t.ap()[:])
            targetsT = cp.tile([P, Q], i32, name="targetsT")
            nc.sync.dma_start(targetsT[:], targetsT_t.ap()[:])
            iota_sb = cp.tile([P, 2 * P], f32, name="iota_sb")
            nc.sync.dma_start(iota_sb[:], iota_t.ap()[:])
            wblk_sb = cp.tile([P, 2 * R * 2 * P], f32, name="wblk_sb")
            nc.sync.dma_start(wblk_sb[:], wblk_t.ap()[:])
            loopw_sb = cp.tile([P, 2 * 2 * H], f32, name="loopw_sb")
            nc.sync.dma_start(loopw_sb[:], loopw_t.ap()[:])
            biasbc_sb = cp.tile([P, 2 * H], f32, name="biasbc_sb")
            nc.sync.dma_start(biasbc_sb[:], biasbc_t.ap()[:])
            fc1_sb = cp.tile([P, KC * MC * P], f32, name="fc1_sb")
            nc.sync.dma_start(fc1_sb[:], fc1_t.ap()[:])
            fc1b_sb = cp.tile([P, MC], f32, name="fc1b_sb")
            nc.sync.dma_start(fc1b_sb[:], fc1b_t.ap()[:])
            fc2_sb = cp.tile([P, MC], f32, name="fc2_sb")
            nc.sync.dma_start(fc2_sb[:], fc2_t.ap()[:])
            ident = cp.tile([P, P], f32, name="ident")
            make_identity(nc, ident[:])

            def wblk_ap(l, r, h):
                o = ((l * R + r) * 2 + h) * P
                return wblk_sb[:, o : o + P]

            def loopw_ap(l, h):
                o = (l * 2 + h) * H
                return loopw_sb[:, o : o + H]

            # warm the xg pool slots with finite data (OOB-skipped pad rows
            # keep whatever the slot held; must never be NaN/Inf)
            for w in range(10):
                xg = wp.tile([P, H], f32, name="xgwarm", tag="xg")
                nc.sync.dma_start(xg[:], h0_t.ap()[0:P, :])

            def emit_ag_piece(li, pi, h_full):
                row0, nrows = pieces[pi]
                if single:
                    nc.sync.dma_start(
                        h_full[row0 : row0 + nrows, :], agin[(li, pi)][:]
                    )
                    return
                nc.gpsimd.collective_compute(
                    "AllGather", mybir.AluOpType.bypass,
                    replica_groups=[list(range(ncores))],
                    ins=[agin[(li, pi)]], outs=[agout[(li, pi)]],
                )
                src_ap = agout[(li, pi)].rearrange("(c n) h -> c n h", c=ncores)
                dst_ap = h_full.rearrange("(c n) h -> c n h", c=ncores)[
                    :, row0 : row0 + nrows, :
                ]
                nc.sync.dma_start(dst_ap, src_ap)

            def layer(l, src_sb, xsrc_ap, xsrc_rows, xown_ap, out_pad_ap, li, h_full_out, gdt=f32):
                for t2 in range(T2):
                    rels = [r for r in range(R) if nch[t2 * R + r] > 0]
                    subs = [st for st in (2 * t2, 2 * t2 + 1) if st < TILES]
                    msg_ps = {}
                    for si_, st in enumerate(subs):
                        xown_sb = wp.tile([P, H], f32, name="xown", tag="xg")
                        nc.sync.dma_start(xown_sb[:], xown_ap[st * P : (st + 1) * P, :])
                        for h in range(2):
                            tp_ps = pp.tile([P, P], f32, name="tp", tag="agg0", bufs=2)
                            nc.tensor.transpose(
                                tp_ps[:], xown_sb[:, h * P : (h + 1) * P], ident[:]
                            )
                            xT_sb = wp.tile([P, P], f32, name="xT", tag="xT")
                            nc.vector.tensor_copy(xT_sb[:], tp_ps[:])
                            for ho in range(2):
                                if h == 0:
                                    msg_ps[(si_, ho)] = pp.tile(
                                        [P, P], f32, name=f"msg{si_}{ho}",
                                        tag=f"msg{si_}{ho}", bufs=1,
                                    )
                                nc.tensor.matmul(
                                    msg_ps[(si_, ho)][:], lhsT=xT_sb[:],
                                    rhs=loopw_ap(l, h)[:, ho * P : (ho + 1) * P],
                                    start=(h == 0), stop=(h == 1 and not rels),
                                )
                    for ri, r in enumerate(rels):
                        cell = t2 * R + r
                        cs = int(chunk_start[cell])
                        n = int(nch[cell])
                        aggT_ps = [
                            pp.tile([P, 2 * P], f32, name=f"agg{h}", tag=f"agg{h}", bufs=2)
                            for h in range(2)
                        ]
                        for ci in range(n):
                            col = cs + ci
                            xg = wp.tile([P, H], f32, name="xg", tag="xg")
                            nc.gpsimd.indirect_dma_start(
                                out=xg[:], out_offset=None, in_=xsrc_ap,
                                in_offset=bass.IndirectOffsetOnAxis(
                                    ap=src_sb[:, col : col + 1], axis=0
                                ),
                                bounds_check=xsrc_rows - 1, oob_is_err=False,
                            )
                            S = wp.tile([P, 2 * P], f32, name="S", tag="S")
                            nc.vector.tensor_scalar(
                                out=S[:], in0=iota_sb[:],
                                scalar1=dstlocT[:, col : col + 1],
                                scalar2=normT[:, col : col + 1],
                                op0=mybir.AluOpType.is_equal, op1=mybir.AluOpType.mult,
                            )
                            for h in range(2):
                                nc.tensor.matmul(
                                    aggT_ps[h][:], lhsT=xg[:, h * P : (h + 1) * P],
                                    rhs=S[:], start=(ci == 0), stop=(ci == n - 1),
                                )
                        last_rel = ri == len(rels) - 1
                        for h in range(2):
                            aggT_sb = wp.tile(
                                [P, 2 * P], f32, name=f"aggsb{h}", tag=f"aggsb{h}"
                            )
                            if h == 0:
                                nc.vector.tensor_copy(aggT_sb[:], aggT_ps[h][:])
                            else:
                                nc.scalar.copy(aggT_sb[:], aggT_ps[h][:])
                            for si_, st in enumerate(subs):
                                nc.tensor.matmul(
                                    msg_ps[(si_, h)][:],
                                    lhsT=aggT_sb[:, si_ * P : (si_ + 1) * P],
                                    rhs=wblk_ap(l, r, h), start=False, stop=last_rel,
                                )
                    for si_, st in enumerate(subs):
                        out_sb = wp.tile([P, H], f32, name="outsb", tag="xg")
                        for ho in range(2):
                            nc.vector.tensor_tensor(
                                out=out_sb[:, ho * P : (ho + 1) * P],
                                in0=msg_ps[(si_, ho)][:],
                                in1=biasbc_sb[:, l * H + ho * P : l * H + (ho + 1) * P],
                                op=mybir.AluOpType.add,
                            )
                        if out_pad_ap is not None:
                            nc.sync.dma_start(
                                out_pad_ap[st * P : (st + 1) * P, :], out_sb[:]
                            )
                        rows = min(P, NOWN - st * P)
                        pi = st // tpp
                        off = (st - pi * tpp) * P
                        nc.sync.dma_start(
                            agin[(li, pi)][off : off + rows, :], out_sb[:rows, :]
                        )
                        # fire this piece's AllGather as soon as its tiles are
                        # stored so the collective overlaps the rest of the layer
                        if st == TILES - 1 or (st + 1) % tpp == 0:
                            emit_ag_piece(li, pi, h_full_out)

            layer(0, srcT, h0_t.ap()[:], N, x0own_t.ap(), h1_own, 1, h1_full)
            layer(1, srcT, h1_full[:], N, h1_own, None, 2, h2_full)

            # ---- MLP head over this core's Q*P pairs ----
            for q in range(Q):
                xcat = wp.tile([P, 2 * H], f32, name="xcat", tag="xcat")
                nc.gpsimd.indirect_dma_start(
                    out=xcat[:, 0:H], out_offset=None, in_=h2_full[:],
                    in_offset=bass.IndirectOffsetOnAxis(ap=drugsT[:, q : q + 1], axis=0),
                )
                nc.gpsimd.indirect_dma_start(
                    out=xcat[:, H : 2 * H], out_offset=None, in_=h2_full[:],
                    in_offset=bass.IndirectOffsetOnAxis(ap=targetsT[:, q : q + 1], axis=0),
                )
                xcatT = []
                for k in range(KC):
                    tp_ps = pp.tile([P, P], f32, name="tpm", tag="agg0", bufs=2)
                    nc.tensor.transpose(tp_ps[:], xcat[:, k * P : (k + 1) * P], ident[:])
                    xcT = wp.tile([P, P], f32, name=f"xcT{k}", tag=f"xcT{k}")
                    nc.vector.tensor_copy(xcT[:], tp_ps[:])
                    xcatT.append(xcT)
                z_ps = pp.tile([1, P], f32, name="z", tag="agg1", bufs=2)
                for m in range(MC):
                    yT_ps = pp.tile([P, P], f32, name="yT", tag="msg00", bufs=1)
                    for k in range(KC):
                        nc.tensor.matmul(
                            yT_ps[:], lhsT=fc1_sb[:, (k * MC + m) * P : (k * MC + m + 1) * P],
                            rhs=xcatT[k][:], start=(k == 0), stop=(k == KC - 1),
                        )
                    yTr = wp.tile([P, P], f32, name="yTr", tag="S")
                    nc.scalar.activation(
                        yTr[:], yT_ps[:], mybir.ActivationFunctionType.Relu,
                        bias=fc1b_sb[:, m : m + 1], scale=1.0,
                    )
                    nc.tensor.matmul(
                        z_ps[:], lhsT=fc2_sb[:, m : m + 1], rhs=yTr[:],
                        start=(m == 0), stop=(m == MC - 1),
                    )
                zs = wp.tile([1, P], f32, name="zs", tag="zs")
                nc.scalar.activation(
                    zs[:], z_ps[:], mybir.ActivationFunctionType.Sigmoid,
                    bias=meta["fc2b"], scale=1.0,
                )
                nc.sync.dma_start(out_t.ap()[q * P : (q + 1) * P, :], zs[:])
    return nc


_NC_CACHE = []


def kernel(**inputs):
    from concourse import bass_utils

    meta, in_maps = _preprocess(inputs)
    key = (meta["N"], meta["H"], meta["R"], meta["TC"], meta["Q"],
           tuple(int(x) for x in meta["nch"]))
    if _NC_CACHE and _NC_CACHE[0][0] == key:
        nc = _NC_CACHE[0][1]
    else:
        nc = _build(meta)
        nc.compile()
        _NC_CACHE[:] = [(key, nc)]
    res = bass_utils.run_bass_kernel_spmd(nc, in_maps, core_ids=list(range(NCORES)))
    out = np.concatenate([res.results[c]["out"] for c in range(NCORES)], axis=0)
    return out.astype(np.float32)



# revision 27
# speedup vs baseline: 3.0916x; 3.0916x over previous
"""Trainium2 Bass kernel for the DTI R-GCN (bdd) model, 8 NeuronCores.

Strategy (SPMD, one program, per-core data):
  - dst-shard the graph: core c owns nodes [c*2500, (c+1)*2500); host routes
    each edge to its dst owner and buckets it into (dst-tile, relation) cells,
    tiles of 256 dst nodes.
  - node features are bf16 in a piece-contiguous shared table (row =
    (piece*8 + core)*640 + pos) so each AllGather piece's output is a
    contiguous slice written by the collective directly, and both layers'
    gathers (plus the head's) read it with one host-remapped index space.
  - gathers use InstDMAGatherAnt (gpsimd.dma_gather, mlp Q7 library): one
    instruction fetches up to 48 chunks x 128 rows via an int16 index list,
    amortizing the ~1us SWDGE fixed cost; multi-packet mode for >1024 rows.
  - per 128-edge chunk: a scatter matrix S[e, d] = norm_e * (iota == dstloc_e)
    (one bf16 tensor_scalar) and two bf16 matmuls xg^T @ S accumulate the
    transposed per-(tile, rel) aggregate aggT[feat half, dst] in fp32 PSUM.
    Pad slots use src row 0 with norm 0 (contribution exactly zero).
  - everything downstream of the edge aggregation runs in float32r (tf32-ish,
    1 cycle/row at 256-wide moving dim -- same PE cost as bf16, ~16x less
    rounding error): per (cell, rel, half) one matmul msgT[of, 256 nodes] +=
    wblk^T aggT accumulates transposed messages; the self-loop joins the same
    PSUM accumulation with lhsT=loop_w slices and rhs=x^T (host-pretransposed
    f32 for layer 1, the resident SBUF h1T written by layer 1 for layer 2).
    The W-apply of relation r is emitted after the scatter of r+1 so PE never
    waits on the aggT PSUM->SBUF copy.
  - bias lands via per-partition tensor_scalar into h1T (resident, f32r) or a
    transient h2T tile; PE transposes produce the row-layout table tiles
    (bf16 for h1, f32 for h2 feeding the head) which are DMA'd to agin and
    AllGather'd in 4 pieces fired as soon as their tiles are stored.
  - MLP head (f32r): ONE dma_gather fetches drug+target rows for all pairs
    from the f32 h2 table; fc1/fc2 run 256 pairs per matmul sweep.

The program is built fresh per invocation from the actual inputs (host does
all index preprocessing; trip counts are data-dependent but identical across
cores by padding cells to the max chunk count over cores).
"""
import sys

sys.path.insert(0, "/opt/trn_rl_repo")
import numpy as np
import ml_dtypes

BF16 = ml_dtypes.bfloat16
P = 128
NCORES = 8
NPIECE = 4  # AllGather pieces per layer; full table is piece-contiguous


def _groups(TC):
    """dma_gather group boundaries over chunk columns: small leading groups
    so PE starts quickly, then big groups to amortize instruction overhead."""
    out = []
    g0 = 0
    for gw in (16, 16, 32):
        if g0 >= TC:
            return out
        gw = min(gw, TC - g0)
        out.append((g0, gw))
        g0 += gw
    while g0 < TC:
        gw = min(48, TC - g0)
        out.append((g0, gw))
        g0 += gw
    return out


def _wrap_idx(cols):
    """[P, K] int chunk-column indices -> [128, K*8] int16 dma_gather layout:
    linear i = c*128 + p; wrapped[p', s] = linear[s*16 + p' % 16]."""
    K = cols.shape[1]
    lin = cols.T.reshape(-1)  # [K*128]
    w16 = lin.reshape(K * 8, 16).T  # [16, K*8]
    return np.tile(w16, (8, 1)).astype(np.int16)


def _preprocess(inputs, ncores=NCORES):
    node_ids = np.asarray(inputs["node_ids"])
    src = np.asarray(inputs["src"])
    dst = np.asarray(inputs["dst"])
    etype = np.asarray(inputs["etype"])
    norm = np.asarray(inputs["norm"]).reshape(-1)
    emb = np.asarray(inputs["emb"], dtype=np.float32)
    drugs = np.asarray(inputs["drugs_index"])
    targets = np.asarray(inputs["targets_index"])

    N = node_ids.shape[0]
    H = emb.shape[1]
    R = int(inputs["w1"].shape[0])
    PAIRS = drugs.shape[0]
    assert N % ncores == 0 and PAIRS % ncores == 0
    NOWN = N // ncores
    TILES = -(-NOWN // P)
    PADN = TILES * P
    PPC = PAIRS // ncores
    assert PPC % P == 0
    Q = PPC // P

    TILE2 = 2 * P  # dst nodes per aggregation cell (segments on matmul free dim)
    T2 = -(-NOWN // TILE2)
    NCELL = T2 * R
    owner = dst // NOWN
    d_local = dst - owner * NOWN
    t_of_e = d_local // TILE2
    dstloc_of_e = (d_local % TILE2).astype(np.float32)
    cell_of_e = t_of_e * R + etype

    # global row in the piece-contiguous shared table: each AllGather piece's
    # output [(p*ncores + c)*PROWS + pos] is a contiguous slice, so the
    # collective writes it directly (no re-fold) and gathers read it directly.
    PROWS = PADN // NPIECE

    def grow(i):
        c = i // NOWN
        n = i - c * NOWN
        p = n // PROWS
        pos = n - p * PROWS
        return ((p * ncores + c) * PROWS + pos).astype(np.int32)

    gsrc = grow(src)

    counts = np.zeros((ncores, NCELL), np.int64)
    for c in range(ncores):
        counts[c] = np.bincount(cell_of_e[owner == c], minlength=NCELL)
    nch = -(-counts.max(axis=0) // P)  # chunks per cell (0 if empty everywhere)
    chunk_start = np.zeros(NCELL, np.int64)
    chunk_start[1:] = np.cumsum(nch)[:-1]
    TC = int(nch.sum())

    # pad slots: src row 0 (valid gather), norm 0 -> zero contribution
    srcT = np.zeros((ncores, P, TC), np.int32)
    dstlocT = np.zeros((ncores, P, TC), np.float32)
    normT = np.zeros((ncores, P, TC), np.float32)
    for c in range(ncores):
        m = owner == c
        eidx = np.where(m)[0]
        cell = cell_of_e[eidx]
        order = np.argsort(cell, kind="stable")
        eidx = eidx[order]
        cell = cell[order]
        cstart = np.zeros(NCELL, np.int64)
        cstart[1:] = np.cumsum(counts[c])[:-1]
        rank = np.arange(len(eidx)) - cstart[cell]
        col = chunk_start[cell] + rank // P
        part = rank % P
        srcT[c, part, col] = gsrc[eidx]
        dstlocT[c, part, col] = dstloc_of_e[eidx]
        normT[c, part, col] = norm[eidx]

    # host-side embedding lookup into the piece-contiguous bf16 table
    h0f = emb[node_ids].astype(np.float32)  # [N, H]
    h0 = h0f.astype(BF16)
    NFULL = ncores * PADN
    h0full = np.zeros((NFULL, H), BF16)
    h0full[grow(np.arange(N))] = h0

    # per-core pretransposed own features (local node order, f32) for the
    # layer-1 self-loop: x0ownT[p, h*PADN + n] = x0own[n, h*P + p]
    x0ownT = np.zeros((ncores, P, 2 * PADN), np.float32)
    for c in range(ncores):
        xo = np.zeros((PADN, H), np.float32)
        xo[:NOWN] = h0f[c * NOWN : (c + 1) * NOWN]
        t = np.ascontiguousarray(xo.T).reshape(2, P, PADN)
        x0ownT[c] = t.transpose(1, 0, 2).reshape(P, 2 * PADN)

    # gather index tables in dma_gather wrapped-int16 layout, per group
    srcW = np.zeros((ncores, P, TC * 8), np.int16)
    for c in range(ncores):
        for g0, gw in _groups(TC):
            srcW[c, :, g0 * 8 : (g0 + gw) * 8] = _wrap_idx(srcT[c][:, g0 : g0 + gw])

    # head pair indices, remapped to the shared-table layout, drug/target
    # interleaved: chunk col 2q = drugs of pair-chunk q, col 2q+1 = targets
    dtW = np.zeros((ncores, P, 2 * Q * 8), np.int16)
    for c in range(ncores):
        dtT = np.zeros((P, 2 * Q), np.int32)
        d = grow(drugs[c * PPC : (c + 1) * PPC]).reshape(Q, P)
        t = grow(targets[c * PPC : (c + 1) * PPC]).reshape(Q, P)
        dtT[:, 0::2] = d.T
        dtT[:, 1::2] = t.T
        dtW[c] = _wrap_idx(dtT)

    # relation block weights as lhsT [if_local, of_local] per (layer, rel, half)
    B = int(inputs["w1"].shape[1])
    si = H // B
    hb = P // si  # blocks per half
    wblk = np.zeros((2, R, 2, P, P), np.float32)
    for l, W in enumerate([inputs["w1"], inputs["w2"]]):
        W = np.asarray(W, np.float32)
        for r in range(R):
            for h in range(2):
                for bb in range(hb):
                    b = hb * h + bb
                    wblk[l, r, h, bb * si : (bb + 1) * si, bb * si : (bb + 1) * si] = W[r, b]
    wblk_in = wblk.transpose(3, 0, 1, 2, 4).reshape(P, 2 * R * 2 * P).copy()

    loopw = np.stack(
        [np.asarray(inputs["loop_w1"], np.float32), np.asarray(inputs["loop_w2"], np.float32)]
    )  # [2, H, H]
    loopw_in = loopw.reshape(2, 2, P, H).transpose(2, 0, 1, 3).reshape(P, 2 * 2 * H).copy()

    # bias as per-partition columns for the transposed pipeline:
    # biasT[p, l*2 + h] = b_l[h*128 + p]
    biasT_in = np.zeros((P, 4), np.float32)
    for l, b in enumerate([inputs["b1"], inputs["b2"]]):
        b = np.asarray(b, np.float32)
        for h in range(2):
            biasT_in[:, l * 2 + h] = b[h * P : (h + 1) * P]

    d2 = 2 * H
    KC = d2 // P  # fc1 contraction chunks
    MC = d2 // P  # fc1 output chunks
    fc1_in = (
        np.asarray(inputs["fc1_W"], np.float32)
        .reshape(KC, P, MC, P)
        .transpose(1, 0, 2, 3)
        .reshape(P, KC * MC * P)
        .copy()
    )
    fc1b_in = np.asarray(inputs["fc1_b"], np.float32).reshape(MC, P).T.copy()
    fc2_in = np.asarray(inputs["fc2_W"], np.float32).reshape(MC, P).T.copy()
    fc2b = float(np.asarray(inputs["fc2_b"]).reshape(-1)[0])

    iota = np.tile(np.arange(2 * P, dtype=np.float32), (P, 1)).astype(BF16)

    meta = dict(
        N=N, H=H, R=R, NOWN=NOWN, TILES=TILES, PADN=PADN, T2=T2, NCELL=NCELL,
        TC=TC, Q=Q, KC=KC, MC=MC, nch=nch, chunk_start=chunk_start, fc2b=fc2b,
    )
    shared = dict(
        h0full=h0full, iota=iota, wblk=wblk_in, loopw=loopw_in, biasT=biasT_in,
        fc1=fc1_in, fc1b=fc1b_in, fc2=fc2_in,
    )
    in_maps = []
    for c in range(ncores):
        m = dict(shared)
        m.update(
            srcW=srcW[c], dstlocT=dstlocT[c], normT=normT[c],
            x0ownT=x0ownT[c], dtW=dtW[c],
        )
        in_maps.append(m)
    return meta, in_maps


def _build(meta, ncores=NCORES, single=False):
    from concourse import bass, mybir, tile, bacc, library_config
    from concourse.masks import make_identity

    N, H, R = meta["N"], meta["H"], meta["R"]
    TILES, PADN, TC, Q = meta["TILES"], meta["PADN"], meta["TC"], meta["Q"]
    T2 = meta["T2"]
    KC, MC = meta["KC"], meta["MC"]
    nch, chunk_start = meta["nch"], meta["chunk_start"]
    NFULL = NCORES * PADN  # same table shape in both builds
    f32 = mybir.dt.float32
    f32r = mybir.dt.float32r
    bf = mybir.dt.bfloat16
    i16 = mybir.dt.int16

    assert PADN % NPIECE == 0 and TILES % NPIECE == 0
    PROWS = PADN // NPIECE
    TPP = TILES // NPIECE

    nc = bacc.Bacc(
        "TRN2", target_bir_lowering=False, debug=False,
        num_devices=(1 if single else ncores),
    )

    h0full_t = nc.dram_tensor("h0full", [NFULL, H], bf, kind="ExternalInput")
    srcW_t = nc.dram_tensor("srcW", [P, TC * 8], i16, kind="ExternalInput")
    dstlocT_t = nc.dram_tensor("dstlocT", [P, TC], f32, kind="ExternalInput")
    normT_t = nc.dram_tensor("normT", [P, TC], f32, kind="ExternalInput")
    x0ownT_t = nc.dram_tensor("x0ownT", [P, 2 * PADN], f32r, kind="ExternalInput")
    dtW_t = nc.dram_tensor("dtW", [P, 2 * Q * 8], i16, kind="ExternalInput")
    iota_t = nc.dram_tensor("iota", [P, 2 * P], bf, kind="ExternalInput")
    wblk_t = nc.dram_tensor("wblk", [P, 2 * R * 2 * P], f32r, kind="ExternalInput")
    loopw_t = nc.dram_tensor("loopw", [P, 2 * 2 * H], f32r, kind="ExternalInput")
    biasT_t = nc.dram_tensor("biasT", [P, 4], f32, kind="ExternalInput")
    fc1_t = nc.dram_tensor("fc1", [P, KC * MC * P], f32r, kind="ExternalInput")
    fc1b_t = nc.dram_tensor("fc1b", [P, MC], f32, kind="ExternalInput")
    fc2_t = nc.dram_tensor("fc2", [P, MC], f32r, kind="ExternalInput")
    out_t = nc.dram_tensor("out", [Q * P, 1], f32, kind="ExternalOutput")

    with tile.TileContext(nc) as tc:
        with (
            tc.tile_pool(name="const", bufs=1) as cp,
            tc.tile_pool(name="work", bufs=1) as wp,
            tc.tile_pool(name="ps", bufs=1, space="PSUM") as pp,
        ):
            # layer tables: h1 bf16 (gathered by layer-2 edges), h2 f32
            # (gathered only by the head; precision matters there)
            tdt = {1: bf, 2: f32r}
            agin = {}
            agout = {}
            for li in (1, 2):
                agin[li] = nc.dram_tensor(
                    f"h{li}_agin", [PADN, H], tdt[li], kind="Internal"
                ).ap()
                agout[li] = nc.dram_tensor(
                    f"h{li}_agout", [NFULL, H], tdt[li], kind="Internal",
                    addr_space=("Local" if single else "Shared"),
                ).ap()

            # ---- resident constants ----
            srcW_sb = cp.tile([P, TC * 8], i16, name="srcW_sb")
            nc.sync.dma_start(srcW_sb[:], srcW_t.ap()[:])
            dstlocT = cp.tile([P, TC], f32, name="dstlocT")
            nc.sync.dma_start(dstlocT[:], dstlocT_t.ap()[:])
            normT = cp.tile([P, TC], f32, name="normT")
            nc.sync.dma_start(normT[:], normT_t.ap()[:])
            dtW_sb = cp.tile([P, 2 * Q * 8], i16, name="dtW_sb")
            nc.sync.dma_start(dtW_sb[:], dtW_t.ap()[:])
            iota_sb = cp.tile([P, 2 * P], bf, name="iota_sb")
            nc.sync.dma_start(iota_sb[:], iota_t.ap()[:])
            wblk_sb = cp.tile([P, 2 * R * 2 * P], f32r, name="wblk_sb")
            nc.sync.dma_start(wblk_sb[:], wblk_t.ap()[:])
            loopw_sb = cp.tile([P, 2 * 2 * H], f32r, name="loopw_sb")
            nc.sync.dma_start(loopw_sb[:], loopw_t.ap()[:])
            biasT_sb = cp.tile([P, 4], f32, name="biasT_sb")
            nc.sync.dma_start(biasT_sb[:], biasT_t.ap()[:])
            fc1_sb = cp.tile([P, KC * MC * P], f32r, name="fc1_sb")
            nc.sync.dma_start(fc1_sb[:], fc1_t.ap()[:])
            fc1b_sb = cp.tile([P, MC], f32, name="fc1b_sb")
            nc.sync.dma_start(fc1b_sb[:], fc1b_t.ap()[:])
            fc2_sb = cp.tile([P, MC], f32r, name="fc2_sb")
            nc.sync.dma_start(fc2_sb[:], fc2_t.ap()[:])
            ident_f = cp.tile([P, P], f32, name="ident_f")
            make_identity(nc, ident_f[:])
            ident = cp.tile([P, P], f32r, name="ident")
            nc.vector.tensor_copy(ident[:], ident_f[:])
            nc.gpsimd.load_library(library_config.mlp)  # Q7 dma_gather ucode
            # resident transposed layer-1 activations (written by layer 1,
            # self-loop rhs for layer 2): h1T[p, h*PADN + n] = h1[n, h*P + p]
            h1T_sb = cp.tile([P, 2 * PADN], f32r, name="h1T_sb")

            def wblk_ap(l, r, h):
                o = ((l * R + r) * 2 + h) * P
                return wblk_sb[:, o : o + P]

            def loopw_ap(l, h):
                o = (l * 2 + h) * H
                return loopw_sb[:, o : o + H]

            def emit_ag_piece(li, pi):
                row0 = pi * PROWS
                orow0 = pi * ncores * PROWS
                if single:
                    nc.sync.dma_start(
                        agout[li][orow0 : orow0 + PROWS, :],
                        agin[li][row0 : row0 + PROWS, :],
                    )
                    return
                nc.gpsimd.collective_compute(
                    "AllGather", mybir.AluOpType.bypass,
                    replica_groups=[list(range(ncores))],
                    ins=[agin[li][row0 : row0 + PROWS, :]],
                    outs=[agout[li][orow0 : orow0 + ncores * PROWS, :]],
                )

            groups = _groups(TC)
            GBMAX = max(gw for _, gw in groups)
            col2group = {}
            for gi, (g0, gw) in enumerate(groups):
                for c in range(g0, g0 + gw):
                    col2group[c] = gi

            def emit_w(l, sbs, r, last_rel, msgT):
                for h in range(2):
                    nc.tensor.matmul(
                        msgT[h][:], lhsT=wblk_ap(l, r, h), rhs=sbs[h][:],
                        start=False, stop=last_rel,
                    )

            def layer(l, xsrc_ap, xT_fn, li, hT_dst):
                group_tiles = {}

                def get_xg(col):
                    gi = col2group[col]
                    if gi not in group_tiles:
                        g0, gw = groups[gi]
                        xg = wp.tile([P, GBMAX * H], bf, name="xg", tag="xg", bufs=3)
                        dst3 = xg[:, : gw * H].rearrange("p (c w) -> p c w", w=H)
                        nc.gpsimd.dma_gather(
                            dst3, xsrc_ap, srcW_sb[:, g0 * 8 : (g0 + gw) * 8],
                            gw * P, gw * P, H, single_packet=False,
                        )
                        group_tiles[gi] = xg
                    return group_tiles[gi], col - groups[col2group[col]][0]

                pend_out = None  # deferred transpose/store closure of prev t2
                for t2 in range(T2):
                    rels = [r for r in range(R) if nch[t2 * R + r] > 0]
                    msgT = {}
                    for h in range(2):
                        msgT[h] = pp.tile(
                            [P, 2 * P], f32, name=f"msgT{h}", tag=f"msgT{h}", bufs=2
                        )
                    # self-loop starts the msgT accumulation:
                    # msgT[ho][of, n] += sum_f loopw[f, of] * xT[f, n]
                    for h in range(2):
                        xT = xT_fn(t2, h)
                        for ho in range(2):
                            nc.tensor.matmul(
                                msgT[ho][:],
                                lhsT=loopw_ap(l, h)[:, ho * P : (ho + 1) * P],
                                rhs=xT, start=(h == 0), stop=(h == 1 and not rels),
                            )
                    if pend_out is not None:
                        pend_out()
                        pend_out = None
                    pend = None  # deferred W-apply (aggT sb tiles, rel, last?)
                    for ri, r in enumerate(rels):
                        cell = t2 * R + r
                        cs = int(chunk_start[cell])
                        n = int(nch[cell])
                        aggT_ps = [
                            pp.tile([P, 2 * P], f32, name=f"agg{h}", tag=f"agg{h}", bufs=2)
                            for h in range(2)
                        ]
                        for ci in range(n):
                            col = cs + ci
                            xg, off = get_xg(col)
                            S = wp.tile([P, 2 * P], bf, name="S", tag="S", bufs=4)
                            nc.vector.tensor_scalar(
                                out=S[:], in0=iota_sb[:],
                                scalar1=dstlocT[:, col : col + 1],
                                scalar2=normT[:, col : col + 1],
                                op0=mybir.AluOpType.is_equal, op1=mybir.AluOpType.mult,
                            )
                            for h in range(2):
                                nc.tensor.matmul(
                                    aggT_ps[h][:],
                                    lhsT=xg[:, off * H + h * P : off * H + (h + 1) * P],
                                    rhs=S[:], start=(ci == 0), stop=(ci == n - 1),
                                )
                        sbs = []
                        for h in range(2):
                            aggT_sb = wp.tile(
                                [P, 2 * P], f32r, name=f"aggsb{h}", tag=f"aggsb{h}", bufs=2
                            )
                            if h == 0:
                                nc.vector.tensor_copy(aggT_sb[:], aggT_ps[h][:])
                            else:
                                nc.scalar.copy(aggT_sb[:], aggT_ps[h][:])
                            sbs.append(aggT_sb)
                        if pend is not None:
                            emit_w(l, *pend)
                        pend = (sbs, r, ri == len(rels) - 1, msgT)
                    if pend is not None:
                        emit_w(l, *pend)

                    # bias -> hT tiles (f32r); transposes/stores are deferred
                    # past the next t2's self-loop so PE doesn't wait on DVE
                    hTs = {}
                    for h in range(2):
                        hT = hT_dst(t2, h)
                        bcol = biasT_sb[:, l * 2 + h : l * 2 + h + 1]
                        if h == 0:
                            nc.vector.tensor_scalar(
                                out=hT, in0=msgT[h][:], scalar1=bcol,
                                scalar2=None, op0=mybir.AluOpType.add,
                            )
                        else:
                            nc.scalar.add(hT, msgT[h][:], bcol)
                        hTs[h] = hT

                    def make_out(t2=t2, hTs=hTs):
                        def go():
                            for si_ in range(2):
                                st = 2 * t2 + si_
                                out_sb = wp.tile(
                                    [P, H], tdt[li], name="outsb", tag="outsb", bufs=3
                                )
                                for h in range(2):
                                    tp = pp.tile(
                                        [P, P], f32r, name="tp", tag="agg0", bufs=2
                                    )
                                    nc.tensor.transpose(
                                        tp[:], hTs[h][:, si_ * P : (si_ + 1) * P],
                                        ident[:],
                                    )
                                    eng = nc.vector if h == 0 else nc.scalar
                                    if h == 0:
                                        nc.vector.tensor_copy(
                                            out_sb[:, h * P : (h + 1) * P], tp[:]
                                        )
                                    else:
                                        nc.scalar.copy(
                                            out_sb[:, h * P : (h + 1) * P], tp[:]
                                        )
                                nc.sync.dma_start(
                                    agin[li][st * P : (st + 1) * P, :], out_sb[:]
                                )
                                if (st + 1) % TPP == 0:
                                    emit_ag_piece(li, (st + 1) // TPP - 1)

                        return go

                    pend_out = make_out()
                pend_out()

            # xT providers return [128 (feat half h), 256 nodes] f32r slices
            x0ownT_cache = {}

            def xT_l1(t2, h):
                if (t2, h) not in x0ownT_cache:
                    xsl = wp.tile([P, 2 * P], f32r, name="xsl", tag="xsl", bufs=4)
                    nc.sync.dma_start(
                        xsl[:],
                        x0ownT_t.ap()[:, h * PADN + t2 * 2 * P : h * PADN + (t2 + 1) * 2 * P],
                    )
                    x0ownT_cache[(t2, h)] = xsl
                return x0ownT_cache[(t2, h)][:]

            def xT_l2(t2, h):
                return h1T_sb[:, h * PADN + t2 * 2 * P : h * PADN + (t2 + 1) * 2 * P]

            def hT_dst_l1(t2, h):
                return h1T_sb[:, h * PADN + t2 * 2 * P : h * PADN + (t2 + 1) * 2 * P]

            h2T_tiles = {}

            def hT_dst_l2(t2, h):
                hv = wp.tile([P, 2 * P], f32r, name="h2T", tag="h2T", bufs=4)
                h2T_tiles[(t2, h)] = hv
                return hv[:]

            layer(0, h0full_t.ap()[:], xT_l1, 1, hT_dst_l1)
            layer(1, agout[1][:], xT_l2, 2, hT_dst_l2)

            # ---- MLP head over this core's Q*P pairs (f32r, 256-pair sweeps)
            xcat_big = wp.tile([P, 2 * Q * H], f32r, name="xcat_big", tag="xcat", bufs=1)
            nc.gpsimd.dma_gather(
                xcat_big[:].rearrange("p (c w) -> p c w", w=H),
                agout[2][:], dtW_sb[:], 2 * Q * P, 2 * Q * P, H,
                single_packet=False,
            )
            qblocks = []
            q0 = 0
            while q0 < Q:
                nq = min(2, Q - q0)
                qblocks.append((q0, nq))
                q0 += nq
            for q0, nq in qblocks:
                PW = nq * P  # pairs in this sweep
                # pair j: q = q0 + j//128, p = j%128
                # xcatT[k][f, j]: f-chunk k of [drug(0:KC/2) | target] halves
                xcatT = []
                for k in range(KC):
                    xcT = wp.tile([P, 2 * P], f32r, name=f"xcT{k}", tag=f"xcT{k}", bufs=1)
                    for jh in range(nq):
                        q = q0 + jh
                        c = 2 * q + (1 if k >= KC // 2 else 0)
                        off = c * H + (k % (KC // 2)) * P
                        tp = pp.tile([P, P], f32r, name="tpm", tag="agg0", bufs=2)
                        nc.tensor.transpose(
                            tp[:], xcat_big[:, off : off + P], ident[:]
                        )
                        if jh == 0:
                            nc.vector.tensor_copy(xcT[:, jh * P : (jh + 1) * P], tp[:])
                        else:
                            nc.scalar.copy(xcT[:, jh * P : (jh + 1) * P], tp[:])
                    xcatT.append(xcT)
                z_ps = pp.tile([1, PW], f32, name="z", tag="msgT1", bufs=2)
                ypend = None  # deferred fc2 matmul, same PE-stall dodge as emit_w
                for m in range(MC):
                    yT_ps = pp.tile(
                        [P, PW], f32, name="yT",
                        tag=("msgT0" if m % 2 == 0 else "agg1"), bufs=2,
                    )
                    for k in range(KC):
                        nc.tensor.matmul(
                            yT_ps[:], lhsT=fc1_sb[:, (k * MC + m) * P : (k * MC + m + 1) * P],
                            rhs=xcatT[k][:, :PW], start=(k == 0), stop=(k == KC - 1),
                        )
                    yTr = wp.tile([P, 2 * P], f32r, name="yTr", tag="yTr", bufs=2)
                    nc.scalar.activation(
                        yTr[:, :PW], yT_ps[:], mybir.ActivationFunctionType.Relu,
                        bias=fc1b_sb[:, m : m + 1], scale=1.0,
                    )
                    if ypend is not None:
                        nc.tensor.matmul(
                            z_ps[:], lhsT=fc2_sb[:, ypend[1] : ypend[1] + 1],
                            rhs=ypend[0][:, :PW], start=(ypend[1] == 0), stop=False,
                        )
                    ypend = (yTr, m)
                nc.tensor.matmul(
                    z_ps[:], lhsT=fc2_sb[:, ypend[1] : ypend[1] + 1],
                    rhs=ypend[0][:, :PW], start=False, stop=True,
                )
                zs = wp.tile([1, 2 * P], f32, name="zs", tag="zs", bufs=2)
                nc.scalar.activation(
                    zs[:, :PW], z_ps[:], mybir.ActivationFunctionType.Sigmoid,
                    bias=meta["fc2b"], scale=1.0,
                )
                nc.sync.dma_start(
                    out_t.ap()[q0 * P : q0 * P + PW, :], zs[:, :PW]
                )
    return nc


_NC_CACHE = []


def kernel(**inputs):
    from concourse import bass_utils

    meta, in_maps = _preprocess(inputs)
    key = (meta["N"], meta["H"], meta["R"], meta["TC"], meta["Q"],
           tuple(int(x) for x in meta["nch"]))
    if _NC_CACHE and _NC_CACHE[0][0] == key:
        nc = _NC_CACHE[0][1]
    else:
        nc = _build(meta)
        nc.compile()
        _NC_CACHE[:] = [(key, nc)]
    res = bass_utils.run_bass_kernel_spmd(nc, in_maps, core_ids=list(range(NCORES)))
    out = np.concatenate([res.results[c]["out"] for c in range(NCORES)], axis=0)
    return out.astype(np.float32)


# revision 29
# speedup vs baseline: 3.1009x; 1.0030x over previous
"""Trainium2 Bass kernel for the DTI R-GCN (bdd) model, 8 NeuronCores.

Strategy (SPMD, one program, per-core data):
  - dst-shard the graph: core c owns nodes [c*2500, (c+1)*2500); host routes
    each edge to its dst owner and buckets it into (dst-tile, relation) cells,
    tiles of 256 dst nodes.
  - node features are bf16 in a piece-contiguous shared table (row =
    (piece*8 + core)*640 + pos) so each AllGather piece's output is a
    contiguous slice written by the collective directly, and both layers'
    gathers (plus the head's) read it with one host-remapped index space.
  - gathers use InstDMAGatherAnt (gpsimd.dma_gather, mlp Q7 library): one
    instruction fetches up to 48 chunks x 128 rows via an int16 index list,
    amortizing the ~1us SWDGE fixed cost; multi-packet mode for >1024 rows.
  - per 128-edge chunk: a scatter matrix S[e, d] = norm_e * (iota == dstloc_e)
    (one bf16 tensor_scalar) and two bf16 matmuls xg^T @ S accumulate the
    transposed per-(tile, rel) aggregate aggT[feat half, dst] in fp32 PSUM.
    Pad slots use src row 0 with norm 0 (contribution exactly zero).
  - everything downstream of the edge aggregation runs in float32r (tf32-ish,
    1 cycle/row at 256-wide moving dim -- same PE cost as bf16, ~16x less
    rounding error): per (cell, rel, half) one matmul msgT[of, 256 nodes] +=
    wblk^T aggT accumulates transposed messages; the self-loop joins the same
    PSUM accumulation with lhsT=loop_w slices and rhs=x^T (host-pretransposed
    f32 for layer 1, the resident SBUF h1T written by layer 1 for layer 2).
    The W-apply of relation r is emitted after the scatter of r+1 so PE never
    waits on the aggT PSUM->SBUF copy.
  - bias lands via per-partition tensor_scalar into h1T (resident, f32r) or a
    transient h2T tile; PE transposes produce the row-layout table tiles
    (bf16 for h1, f32 for h2 feeding the head) which are DMA'd to agin and
    AllGather'd in 4 pieces fired as soon as their tiles are stored.
  - MLP head (f32r): ONE dma_gather fetches drug+target rows for all pairs
    from the f32 h2 table; fc1/fc2 run 256 pairs per matmul sweep.

The program is built fresh per invocation from the actual inputs (host does
all index preprocessing; trip counts are data-dependent but identical across
cores by padding cells to the max chunk count over cores).
"""
import sys

sys.path.insert(0, "/opt/trn_rl_repo")
import numpy as np
import ml_dtypes

BF16 = ml_dtypes.bfloat16
P = 128
NCORES = 8
NPIECE = 4  # AllGather pieces per layer; full table is piece-contiguous


def _groups(TC):
    """dma_gather group boundaries over chunk columns: small leading groups
    so PE starts quickly, then big groups to amortize instruction overhead."""
    out = []
    g0 = 0
    for gw in (16, 16, 32):
        if g0 >= TC:
            return out
        gw = min(gw, TC - g0)
        out.append((g0, gw))
        g0 += gw
    while g0 < TC:
        gw = min(48, TC - g0)
        out.append((g0, gw))
        g0 += gw
    return out


def _wrap_idx(cols):
    """[P, K] int chunk-column indices -> [128, K*8] int16 dma_gather layout:
    linear i = c*128 + p; wrapped[p', s] = linear[s*16 + p' % 16]."""
    K = cols.shape[1]
    lin = cols.T.reshape(-1)  # [K*128]
    w16 = lin.reshape(K * 8, 16).T  # [16, K*8]
    return np.tile(w16, (8, 1)).astype(np.int16)


def _preprocess(inputs, ncores=NCORES):
    node_ids = np.asarray(inputs["node_ids"])
    src = np.asarray(inputs["src"])
    dst = np.asarray(inputs["dst"])
    etype = np.asarray(inputs["etype"])
    norm = np.asarray(inputs["norm"]).reshape(-1)
    emb = np.asarray(inputs["emb"], dtype=np.float32)
    drugs = np.asarray(inputs["drugs_index"])
    targets = np.asarray(inputs["targets_index"])

    N = node_ids.shape[0]
    H = emb.shape[1]
    R = int(inputs["w1"].shape[0])
    PAIRS = drugs.shape[0]
    assert N % ncores == 0 and PAIRS % ncores == 0
    NOWN = N // ncores
    TILES = -(-NOWN // P)
    PADN = TILES * P
    PPC = PAIRS // ncores
    assert PPC % P == 0
    Q = PPC // P

    TILE2 = 2 * P  # dst nodes per aggregation cell (segments on matmul free dim)
    T2 = -(-NOWN // TILE2)
    NCELL = T2 * R
    owner = dst // NOWN
    d_local = dst - owner * NOWN
    t_of_e = d_local // TILE2
    dstloc_of_e = (d_local % TILE2).astype(np.float32)
    cell_of_e = t_of_e * R + etype

    # global row in the piece-contiguous shared table: each AllGather piece's
    # output [(p*ncores + c)*PROWS + pos] is a contiguous slice, so the
    # collective writes it directly (no re-fold) and gathers read it directly.
    PROWS = PADN // NPIECE

    def grow(i):
        c = i // NOWN
        n = i - c * NOWN
        p = n // PROWS
        pos = n - p * PROWS
        return ((p * ncores + c) * PROWS + pos).astype(np.int32)

    gsrc = grow(src)

    counts = np.zeros((ncores, NCELL), np.int64)
    for c in range(ncores):
        counts[c] = np.bincount(cell_of_e[owner == c], minlength=NCELL)
    nch = -(-counts.max(axis=0) // P)  # chunks per cell (0 if empty everywhere)
    chunk_start = np.zeros(NCELL, np.int64)
    chunk_start[1:] = np.cumsum(nch)[:-1]
    TC = int(nch.sum())

    # pad slots: src row 0 (valid gather), norm 0 -> zero contribution
    srcT = np.zeros((ncores, P, TC), np.int32)
    dstlocT = np.zeros((ncores, P, TC), np.float32)
    normT = np.zeros((ncores, P, TC), np.float32)
    for c in range(ncores):
        m = owner == c
        eidx = np.where(m)[0]
        cell = cell_of_e[eidx]
        order = np.argsort(cell, kind="stable")
        eidx = eidx[order]
        cell = cell[order]
        cstart = np.zeros(NCELL, np.int64)
        cstart[1:] = np.cumsum(counts[c])[:-1]
        rank = np.arange(len(eidx)) - cstart[cell]
        col = chunk_start[cell] + rank // P
        part = rank % P
        srcT[c, part, col] = gsrc[eidx]
        dstlocT[c, part, col] = dstloc_of_e[eidx]
        normT[c, part, col] = norm[eidx]

    # host-side embedding lookup into the piece-contiguous bf16 table
    h0f = emb[node_ids].astype(np.float32)  # [N, H]
    h0 = h0f.astype(BF16)
    NFULL = ncores * PADN
    h0full = np.zeros((NFULL, H), BF16)
    h0full[grow(np.arange(N))] = h0

    # per-core pretransposed own features (local node order, f32) for the
    # layer-1 self-loop: x0ownT[p, h*PADN + n] = x0own[n, h*P + p]
    x0ownT = np.zeros((ncores, P, 2 * PADN), np.float32)
    for c in range(ncores):
        xo = np.zeros((PADN, H), np.float32)
        xo[:NOWN] = h0f[c * NOWN : (c + 1) * NOWN]
        t = np.ascontiguousarray(xo.T).reshape(2, P, PADN)
        x0ownT[c] = t.transpose(1, 0, 2).reshape(P, 2 * PADN)

    # gather index tables in dma_gather wrapped-int16 layout, per group
    srcW = np.zeros((ncores, P, TC * 8), np.int16)
    for c in range(ncores):
        for g0, gw in _groups(TC):
            srcW[c, :, g0 * 8 : (g0 + gw) * 8] = _wrap_idx(srcT[c][:, g0 : g0 + gw])

    # head pair indices, remapped to the shared-table layout, drug/target
    # interleaved: chunk col 2q = drugs of pair-chunk q, col 2q+1 = targets
    dtW = np.zeros((ncores, P, 2 * Q * 8), np.int16)
    for c in range(ncores):
        dtT = np.zeros((P, 2 * Q), np.int32)
        d = grow(drugs[c * PPC : (c + 1) * PPC]).reshape(Q, P)
        t = grow(targets[c * PPC : (c + 1) * PPC]).reshape(Q, P)
        dtT[:, 0::2] = d.T
        dtT[:, 1::2] = t.T
        dtW[c] = _wrap_idx(dtT)

    # relation block weights as lhsT [if_local, of_local] per (layer, rel, half)
    B = int(inputs["w1"].shape[1])
    si = H // B
    hb = P // si  # blocks per half
    wblk = np.zeros((2, R, 2, P, P), np.float32)
    for l, W in enumerate([inputs["w1"], inputs["w2"]]):
        W = np.asarray(W, np.float32)
        for r in range(R):
            for h in range(2):
                for bb in range(hb):
                    b = hb * h + bb
                    wblk[l, r, h, bb * si : (bb + 1) * si, bb * si : (bb + 1) * si] = W[r, b]
    wblk_in = wblk.transpose(3, 0, 1, 2, 4).reshape(P, 2 * R * 2 * P).copy()

    loopw = np.stack(
        [np.asarray(inputs["loop_w1"], np.float32), np.asarray(inputs["loop_w2"], np.float32)]
    )  # [2, H, H]
    loopw_in = loopw.reshape(2, 2, P, H).transpose(2, 0, 1, 3).reshape(P, 2 * 2 * H).copy()

    # bias as per-partition columns for the transposed pipeline:
    # biasT[p, l*2 + h] = b_l[h*128 + p]
    biasT_in = np.zeros((P, 4), np.float32)
    for l, b in enumerate([inputs["b1"], inputs["b2"]]):
        b = np.asarray(b, np.float32)
        for h in range(2):
            biasT_in[:, l * 2 + h] = b[h * P : (h + 1) * P]

    d2 = 2 * H
    KC = d2 // P  # fc1 contraction chunks
    MC = d2 // P  # fc1 output chunks
    fc1_in = (
        np.asarray(inputs["fc1_W"], np.float32)
        .reshape(KC, P, MC, P)
        .transpose(1, 0, 2, 3)
        .reshape(P, KC * MC * P)
        .copy()
    )
    fc1b_in = np.asarray(inputs["fc1_b"], np.float32).reshape(MC, P).T.copy()
    fc2_in = np.asarray(inputs["fc2_W"], np.float32).reshape(MC, P).T.copy()
    fc2b = float(np.asarray(inputs["fc2_b"]).reshape(-1)[0])

    iota = np.tile(np.arange(2 * P, dtype=np.float32), (P, 1)).astype(BF16)

    meta = dict(
        N=N, H=H, R=R, NOWN=NOWN, TILES=TILES, PADN=PADN, T2=T2, NCELL=NCELL,
        TC=TC, Q=Q, KC=KC, MC=MC, nch=nch, chunk_start=chunk_start, fc2b=fc2b,
    )
    shared = dict(
        h0full=h0full, iota=iota, wblk=wblk_in, loopw=loopw_in, biasT=biasT_in,
        fc1=fc1_in, fc1b=fc1b_in, fc2=fc2_in,
    )
    in_maps = []
    for c in range(ncores):
        m = dict(shared)
        m.update(
            srcW=srcW[c], dstlocT=dstlocT[c], normT=normT[c],
            x0ownT=x0ownT[c], dtW=dtW[c],
        )
        in_maps.append(m)
    return meta, in_maps


def _build(meta, ncores=NCORES, single=False):
    from concourse import bass, mybir, tile, bacc, library_config
    from concourse.masks import make_identity

    N, H, R = meta["N"], meta["H"], meta["R"]
    TILES, PADN, TC, Q = meta["TILES"], meta["PADN"], meta["TC"], meta["Q"]
    T2 = meta["T2"]
    KC, MC = meta["KC"], meta["MC"]
    nch, chunk_start = meta["nch"], meta["chunk_start"]
    NFULL = NCORES * PADN  # same table shape in both builds
    f32 = mybir.dt.float32
    f32r = mybir.dt.float32r
    bf = mybir.dt.bfloat16
    i16 = mybir.dt.int16

    assert PADN % NPIECE == 0 and TILES % NPIECE == 0
    PROWS = PADN // NPIECE
    TPP = TILES // NPIECE

    nc = bacc.Bacc(
        "TRN2", target_bir_lowering=False, debug=False,
        num_devices=(1 if single else ncores),
    )

    h0full_t = nc.dram_tensor("h0full", [NFULL, H], bf, kind="ExternalInput")
    srcW_t = nc.dram_tensor("srcW", [P, TC * 8], i16, kind="ExternalInput")
    dstlocT_t = nc.dram_tensor("dstlocT", [P, TC], f32, kind="ExternalInput")
    normT_t = nc.dram_tensor("normT", [P, TC], f32, kind="ExternalInput")
    x0ownT_t = nc.dram_tensor("x0ownT", [P, 2 * PADN], f32r, kind="ExternalInput")
    dtW_t = nc.dram_tensor("dtW", [P, 2 * Q * 8], i16, kind="ExternalInput")
    iota_t = nc.dram_tensor("iota", [P, 2 * P], bf, kind="ExternalInput")
    wblk_t = nc.dram_tensor("wblk", [P, 2 * R * 2 * P], f32r, kind="ExternalInput")
    loopw_t = nc.dram_tensor("loopw", [P, 2 * 2 * H], f32r, kind="ExternalInput")
    biasT_t = nc.dram_tensor("biasT", [P, 4], f32, kind="ExternalInput")
    fc1_t = nc.dram_tensor("fc1", [P, KC * MC * P], f32r, kind="ExternalInput")
    fc1b_t = nc.dram_tensor("fc1b", [P, MC], f32, kind="ExternalInput")
    fc2_t = nc.dram_tensor("fc2", [P, MC], f32r, kind="ExternalInput")
    out_t = nc.dram_tensor("out", [Q * P, 1], f32, kind="ExternalOutput")

    with tile.TileContext(nc) as tc:
        with (
            tc.tile_pool(name="const", bufs=1) as cp,
            tc.tile_pool(name="work", bufs=1) as wp,
            tc.tile_pool(name="ps", bufs=1, space="PSUM") as pp,
        ):
            # layer tables: h1 bf16 (gathered by layer-2 edges), h2 f32
            # (gathered only by the head; precision matters there)
            tdt = {1: bf, 2: f32r}
            agin = {}
            agout = {}
            for li in (1, 2):
                agin[li] = nc.dram_tensor(
                    f"h{li}_agin", [PADN, H], tdt[li], kind="Internal"
                ).ap()
                agout[li] = nc.dram_tensor(
                    f"h{li}_agout", [NFULL, H], tdt[li], kind="Internal",
                    addr_space=("Local" if single else "Shared"),
                ).ap()

            # ---- resident constants ----
            srcW_sb = cp.tile([P, TC * 8], i16, name="srcW_sb")
            nc.sync.dma_start(srcW_sb[:], srcW_t.ap()[:])
            dstlocT = cp.tile([P, TC], f32, name="dstlocT")
            nc.sync.dma_start(dstlocT[:], dstlocT_t.ap()[:])
            normT = cp.tile([P, TC], f32, name="normT")
            nc.sync.dma_start(normT[:], normT_t.ap()[:])
            iota_sb = cp.tile([P, 2 * P], bf, name="iota_sb")
            nc.sync.dma_start(iota_sb[:], iota_t.ap()[:])
            loopw_sb = cp.tile([P, 2 * 2 * H], f32r, name="loopw_sb")
            nc.sync.dma_start(loopw_sb[:], loopw_t.ap()[:])
            biasT_sb = cp.tile([P, 4], f32, name="biasT_sb")
            nc.sync.dma_start(biasT_sb[:], biasT_t.ap()[:])
            # wblk split by layer so layer-0's half lands before its first
            # W-apply; layer-1's half streams in behind it
            wblk_sb = cp.tile([P, 2 * R * 2 * P], f32r, name="wblk_sb")
            WHALF = R * 2 * P
            nc.sync.dma_start(wblk_sb[:, :WHALF], wblk_t.ap()[:, :WHALF])
            nc.sync.dma_start(wblk_sb[:, WHALF:], wblk_t.ap()[:, WHALF:])
            # head constants: emitted late (only needed after both layers)
            dtW_sb = cp.tile([P, 2 * Q * 8], i16, name="dtW_sb")
            fc1_sb = cp.tile([P, KC * MC * P], f32r, name="fc1_sb")
            fc1b_sb = cp.tile([P, MC], f32, name="fc1b_sb")
            fc2_sb = cp.tile([P, MC], f32r, name="fc2_sb")
            ident_f = cp.tile([P, P], f32, name="ident_f")
            make_identity(nc, ident_f[:])
            ident = cp.tile([P, P], f32r, name="ident")
            nc.vector.tensor_copy(ident[:], ident_f[:])
            nc.gpsimd.load_library(library_config.mlp)  # Q7 dma_gather ucode
            # resident transposed layer-1 activations (written by layer 1,
            # self-loop rhs for layer 2): h1T[p, h*PADN + n] = h1[n, h*P + p]
            h1T_sb = cp.tile([P, 2 * PADN], f32r, name="h1T_sb")

            def wblk_ap(l, r, h):
                o = ((l * R + r) * 2 + h) * P
                return wblk_sb[:, o : o + P]

            def loopw_ap(l, h):
                o = (l * 2 + h) * H
                return loopw_sb[:, o : o + H]

            def emit_ag_piece(li, pi):
                row0 = pi * PROWS
                orow0 = pi * ncores * PROWS
                if single:
                    nc.sync.dma_start(
                        agout[li][orow0 : orow0 + PROWS, :],
                        agin[li][row0 : row0 + PROWS, :],
                    )
                    return
                nc.gpsimd.collective_compute(
                    "AllGather", mybir.AluOpType.bypass,
                    replica_groups=[list(range(ncores))],
                    ins=[agin[li][row0 : row0 + PROWS, :]],
                    outs=[agout[li][orow0 : orow0 + ncores * PROWS, :]],
                )

            groups = _groups(TC)
            GBMAX = max(gw for _, gw in groups)
            col2group = {}
            for gi, (g0, gw) in enumerate(groups):
                for c in range(g0, g0 + gw):
                    col2group[c] = gi

            def emit_w(l, sbs, r, last_rel, msgT):
                for h in range(2):
                    nc.tensor.matmul(
                        msgT[h][:], lhsT=wblk_ap(l, r, h), rhs=sbs[h][:],
                        start=False, stop=last_rel,
                    )

            def layer(l, xsrc_ap, xT_fn, li, hT_dst):
                group_tiles = {}

                def get_xg(col):
                    gi = col2group[col]
                    if gi not in group_tiles:
                        g0, gw = groups[gi]
                        xg = wp.tile([P, GBMAX * H], bf, name="xg", tag="xg", bufs=3)
                        dst3 = xg[:, : gw * H].rearrange("p (c w) -> p c w", w=H)
                        nc.gpsimd.dma_gather(
                            dst3, xsrc_ap, srcW_sb[:, g0 * 8 : (g0 + gw) * 8],
                            gw * P, gw * P, H, single_packet=False,
                        )
                        group_tiles[gi] = xg
                    return group_tiles[gi], col - groups[col2group[col]][0]

                pend_out = None  # deferred transpose/store closure of prev t2
                for t2 in range(T2):
                    rels = [r for r in range(R) if nch[t2 * R + r] > 0]
                    msgT = {}
                    for h in range(2):
                        msgT[h] = pp.tile(
                            [P, 2 * P], f32, name=f"msgT{h}", tag=f"msgT{h}", bufs=2
                        )
                    # self-loop starts the msgT accumulation:
                    # msgT[ho][of, n] += sum_f loopw[f, of] * xT[f, n]
                    for h in range(2):
                        xT = xT_fn(t2, h)
                        for ho in range(2):
                            nc.tensor.matmul(
                                msgT[ho][:],
                                lhsT=loopw_ap(l, h)[:, ho * P : (ho + 1) * P],
                                rhs=xT, start=(h == 0), stop=(h == 1 and not rels),
                            )
                    if pend_out is not None:
                        pend_out()
                        pend_out = None
                    pend = None  # deferred W-apply (aggT sb tiles, rel, last?)
                    for ri, r in enumerate(rels):
                        cell = t2 * R + r
                        cs = int(chunk_start[cell])
                        n = int(nch[cell])
                        aggT_ps = [
                            pp.tile([P, 2 * P], f32, name=f"agg{h}", tag=f"agg{h}", bufs=2)
                            for h in range(2)
                        ]
                        for ci in range(n):
                            col = cs + ci
                            xg, off = get_xg(col)
                            S = wp.tile([P, 2 * P], bf, name="S", tag="S", bufs=4)
                            nc.vector.tensor_scalar(
                                out=S[:], in0=iota_sb[:],
                                scalar1=dstlocT[:, col : col + 1],
                                scalar2=normT[:, col : col + 1],
                                op0=mybir.AluOpType.is_equal, op1=mybir.AluOpType.mult,
                            )
                            for h in range(2):
                                nc.tensor.matmul(
                                    aggT_ps[h][:],
                                    lhsT=xg[:, off * H + h * P : off * H + (h + 1) * P],
                                    rhs=S[:], start=(ci == 0), stop=(ci == n - 1),
                                )
                        sbs = []
                        for h in range(2):
                            aggT_sb = wp.tile(
                                [P, 2 * P], f32r, name=f"aggsb{h}", tag=f"aggsb{h}", bufs=2
                            )
                            if (2 * ri + h) % 3 == 0:
                                nc.vector.tensor_copy(aggT_sb[:], aggT_ps[h][:])
                            else:
                                nc.scalar.copy(aggT_sb[:], aggT_ps[h][:])
                            sbs.append(aggT_sb)
                        if pend is not None:
                            emit_w(l, *pend)
                        pend = (sbs, r, ri == len(rels) - 1, msgT)
                    if pend is not None:
                        emit_w(l, *pend)

                    # bias -> hT tiles (f32r); transposes/stores are deferred
                    # past the next t2's self-loop so PE doesn't wait on DVE
                    hTs = {}
                    for h in range(2):
                        hT = hT_dst(t2, h)
                        bcol = biasT_sb[:, l * 2 + h : l * 2 + h + 1]
                        if h == 0:
                            nc.vector.tensor_scalar(
                                out=hT, in0=msgT[h][:], scalar1=bcol,
                                scalar2=None, op0=mybir.AluOpType.add,
                            )
                        else:
                            nc.scalar.add(hT, msgT[h][:], bcol)
                        hTs[h] = hT

                    def make_out(t2=t2, hTs=hTs):
                        def go():
                            for si_ in range(2):
                                st = 2 * t2 + si_
                                out_sb = wp.tile(
                                    [P, H], tdt[li], name="outsb", tag="outsb", bufs=3
                                )
                                for h in range(2):
                                    tp = pp.tile(
                                        [P, P], f32r, name="tp", tag="agg0", bufs=2
                                    )
                                    nc.tensor.transpose(
                                        tp[:], hTs[h][:, si_ * P : (si_ + 1) * P],
                                        ident[:],
                                    )
                                    eng = nc.vector if h == 0 else nc.scalar
                                    if h == 0:
                                        nc.vector.tensor_copy(
                                            out_sb[:, h * P : (h + 1) * P], tp[:]
                                        )
                                    else:
                                        nc.scalar.copy(
                                            out_sb[:, h * P : (h + 1) * P], tp[:]
                                        )
                                nc.sync.dma_start(
                                    agin[li][st * P : (st + 1) * P, :], out_sb[:]
                                )
                                if (st + 1) % TPP == 0:
                                    emit_ag_piece(li, (st + 1) // TPP - 1)

                        return go

                    pend_out = make_out()
                pend_out()

            # xT providers return [128 (feat half h), 256 nodes] f32r slices
            x0ownT_cache = {}

            def xT_l1(t2, h):
                if (t2, h) not in x0ownT_cache:
                    xsl = wp.tile([P, 2 * P], f32r, name="xsl", tag="xsl", bufs=4)
                    nc.sync.dma_start(
                        xsl[:],
                        x0ownT_t.ap()[:, h * PADN + t2 * 2 * P : h * PADN + (t2 + 1) * 2 * P],
                    )
                    x0ownT_cache[(t2, h)] = xsl
                return x0ownT_cache[(t2, h)][:]

            def xT_l2(t2, h):
                return h1T_sb[:, h * PADN + t2 * 2 * P : h * PADN + (t2 + 1) * 2 * P]

            def hT_dst_l1(t2, h):
                return h1T_sb[:, h * PADN + t2 * 2 * P : h * PADN + (t2 + 1) * 2 * P]

            h2T_tiles = {}

            def hT_dst_l2(t2, h):
                hv = wp.tile([P, 2 * P], f32r, name="h2T", tag="h2T", bufs=4)
                h2T_tiles[(t2, h)] = hv
                return hv[:]

            layer(0, h0full_t.ap()[:], xT_l1, 1, hT_dst_l1)
            nc.sync.dma_start(dtW_sb[:], dtW_t.ap()[:])
            nc.sync.dma_start(fc1_sb[:], fc1_t.ap()[:])
            nc.sync.dma_start(fc1b_sb[:], fc1b_t.ap()[:])
            nc.sync.dma_start(fc2_sb[:], fc2_t.ap()[:])
            layer(1, agout[1][:], xT_l2, 2, hT_dst_l2)

            # ---- MLP head over this core's Q*P pairs (f32r, 256-pair sweeps)
            xcat_big = wp.tile([P, 2 * Q * H], f32r, name="xcat_big", tag="xcat", bufs=1)
            nc.gpsimd.dma_gather(
                xcat_big[:].rearrange("p (c w) -> p c w", w=H),
                agout[2][:], dtW_sb[:], 2 * Q * P, 2 * Q * P, H,
                single_packet=False,
            )
            qblocks = []
            q0 = 0
            while q0 < Q:
                nq = min(2, Q - q0)
                qblocks.append((q0, nq))
                q0 += nq
            for q0, nq in qblocks:
                PW = nq * P  # pairs in this sweep
                # pair j: q = q0 + j//128, p = j%128
                # xcatT[k][f, j]: f-chunk k of [drug(0:KC/2) | target] halves
                xcatT = []
                for k in range(KC):
                    xcT = wp.tile([P, 2 * P], f32r, name=f"xcT{k}", tag=f"xcT{k}", bufs=1)
                    for jh in range(nq):
                        q = q0 + jh
                        c = 2 * q + (1 if k >= KC // 2 else 0)
                        off = c * H + (k % (KC // 2)) * P
                        tp = pp.tile([P, P], f32r, name="tpm", tag="agg0", bufs=2)
                        nc.tensor.transpose(
                            tp[:], xcat_big[:, off : off + P], ident[:]
                        )
                        if jh == 0:
                            nc.vector.tensor_copy(xcT[:, jh * P : (jh + 1) * P], tp[:])
                        else:
                            nc.scalar.copy(xcT[:, jh * P : (jh + 1) * P], tp[:])
                    xcatT.append(xcT)
                z_ps = pp.tile([1, PW], f32, name="z", tag="msgT1", bufs=2)
                ypend = None  # deferred fc2 matmul, same PE-stall dodge as emit_w
                for m in range(MC):
                    yT_ps = pp.tile(
                        [P, PW], f32, name="yT",
                        tag=("msgT0" if m % 2 == 0 else "agg1"), bufs=2,
                    )
                    for k in range(KC):
                        nc.tensor.matmul(
                            yT_ps[:], lhsT=fc1_sb[:, (k * MC + m) * P : (k * MC + m + 1) * P],
                            rhs=xcatT[k][:, :PW], start=(k == 0), stop=(k == KC - 1),
                        )
                    yTr = wp.tile([P, 2 * P], f32r, name="yTr", tag="yTr", bufs=2)
                    nc.scalar.activation(
                        yTr[:, :PW], yT_ps[:], mybir.ActivationFunctionType.Relu,
                        bias=fc1b_sb[:, m : m + 1], scale=1.0,
                    )
                    if ypend is not None:
                        nc.tensor.matmul(
                            z_ps[:], lhsT=fc2_sb[:, ypend[1] : ypend[1] + 1],
                            rhs=ypend[0][:, :PW], start=(ypend[1] == 0), stop=False,
                        )
                    ypend = (yTr, m)
                nc.tensor.matmul(
                    z_ps[:], lhsT=fc2_sb[:, ypend[1] : ypend[1] + 1],
                    rhs=ypend[0][:, :PW], start=False, stop=True,
                )
                zs = wp.tile([1, 2 * P], f32, name="zs", tag="zs", bufs=2)
                nc.scalar.activation(
                    zs[:, :PW], z_ps[:], mybir.ActivationFunctionType.Sigmoid,
                    bias=meta["fc2b"], scale=1.0,
                )
                nc.sync.dma_start(
                    out_t.ap()[q0 * P : q0 * P + PW, :], zs[:, :PW]
                )
    return nc


_NC_CACHE = []


def kernel(**inputs):
    from concourse import bass_utils

    meta, in_maps = _preprocess(inputs)
    key = (meta["N"], meta["H"], meta["R"], meta["TC"], meta["Q"],
           tuple(int(x) for x in meta["nch"]))
    if _NC_CACHE and _NC_CACHE[0][0] == key:
        nc = _NC_CACHE[0][1]
    else:
        nc = _build(meta)
        nc.compile()
        _NC_CACHE[:] = [(key, nc)]
    res = bass_utils.run_bass_kernel_spmd(nc, in_maps, core_ids=list(range(NCORES)))
    out = np.concatenate([res.results[c]["out"] for c in range(NCORES)], axis=0)
    return out.astype(np.float32)


# revision 30
# speedup vs baseline: 3.2123x; 1.0359x over previous
"""Trainium2 Bass kernel for the DTI R-GCN (bdd) model, 8 NeuronCores.

Strategy (SPMD, one program, per-core data):
  - dst-shard the graph: core c owns nodes [c*2500, (c+1)*2500); host routes
    each edge to its dst owner and buckets it into (dst-tile, relation) cells,
    tiles of 256 dst nodes.
  - node features are bf16 in a piece-contiguous shared table (row =
    (piece*8 + core)*640 + pos) so each AllGather piece's output is a
    contiguous slice written by the collective directly, and both layers'
    gathers (plus the head's) read it with one host-remapped index space.
  - gathers use InstDMAGatherAnt (gpsimd.dma_gather, mlp Q7 library): one
    instruction fetches up to 48 chunks x 128 rows via an int16 index list,
    amortizing the ~1us SWDGE fixed cost; multi-packet mode for >1024 rows.
  - per 128-edge chunk: a scatter matrix S[e, d] = norm_e * (iota == dstloc_e)
    (one bf16 tensor_scalar) and two bf16 matmuls xg^T @ S accumulate the
    transposed per-(tile, rel) aggregate aggT[feat half, dst] in fp32 PSUM.
    Pad slots use src row 0 with norm 0 (contribution exactly zero).
  - everything downstream of the edge aggregation runs in float32r (tf32-ish,
    1 cycle/row at 256-wide moving dim -- same PE cost as bf16, ~16x less
    rounding error): per (cell, rel, half) one matmul msgT[of, 256 nodes] +=
    wblk^T aggT accumulates transposed messages; the self-loop joins the same
    PSUM accumulation with lhsT=loop_w slices and rhs=x^T (host-pretransposed
    f32 for layer 1, the resident SBUF h1T written by layer 1 for layer 2).
    The W-apply of relation r is emitted after the scatter of r+1 so PE never
    waits on the aggT PSUM->SBUF copy.
  - bias lands via per-partition tensor_scalar into h1T (resident, f32r) or a
    transient h2T tile; PE transposes produce the row-layout table tiles
    (bf16 for h1, f32 for h2 feeding the head) which are DMA'd to agin and
    AllGather'd in 4 pieces fired as soon as their tiles are stored.
  - MLP head (f32r): ONE dma_gather fetches drug+target rows for all pairs
    from the f32 h2 table; fc1/fc2 run 256 pairs per matmul sweep.

The program is built fresh per invocation from the actual inputs (host does
all index preprocessing; trip counts are data-dependent but identical across
cores by padding cells to the max chunk count over cores).
"""
import sys

sys.path.insert(0, "/opt/trn_rl_repo")
import numpy as np
import ml_dtypes

BF16 = ml_dtypes.bfloat16
P = 128
NCORES = 8
NPIECE = 4  # AllGather pieces per layer; full table is piece-contiguous


def _groups(TC):
    """dma_gather group boundaries over chunk columns: small leading groups
    so PE starts quickly, then big groups to amortize instruction overhead."""
    out = []
    g0 = 0
    for gw in (16, 16, 32):
        if g0 >= TC:
            return out
        gw = min(gw, TC - g0)
        out.append((g0, gw))
        g0 += gw
    while g0 < TC:
        gw = min(48, TC - g0)
        out.append((g0, gw))
        g0 += gw
    return out


def _wrap_idx(cols):
    """[P, K] int chunk-column indices -> [128, K*8] int16 dma_gather layout:
    linear i = c*128 + p; wrapped[p', s] = linear[s*16 + p' % 16]."""
    K = cols.shape[1]
    lin = cols.T.reshape(-1)  # [K*128]
    w16 = lin.reshape(K * 8, 16).T  # [16, K*8]
    return np.tile(w16, (8, 1)).astype(np.int16)


def _preprocess(inputs, ncores=NCORES):
    node_ids = np.asarray(inputs["node_ids"])
    src = np.asarray(inputs["src"])
    dst = np.asarray(inputs["dst"])
    etype = np.asarray(inputs["etype"])
    norm = np.asarray(inputs["norm"]).reshape(-1)
    emb = np.asarray(inputs["emb"], dtype=np.float32)
    drugs = np.asarray(inputs["drugs_index"])
    targets = np.asarray(inputs["targets_index"])

    N = node_ids.shape[0]
    H = emb.shape[1]
    R = int(inputs["w1"].shape[0])
    PAIRS = drugs.shape[0]
    assert N % ncores == 0 and PAIRS % ncores == 0
    NOWN = N // ncores
    TILES = -(-NOWN // P)
    PADN = TILES * P
    PPC = PAIRS // ncores
    assert PPC % P == 0
    Q = PPC // P

    TILE2 = 2 * P  # dst nodes per aggregation cell (segments on matmul free dim)
    T2 = -(-NOWN // TILE2)
    NCELL = T2 * R
    owner = dst // NOWN
    d_local = dst - owner * NOWN
    t_of_e = d_local // TILE2
    dstloc_of_e = (d_local % TILE2).astype(np.float32)
    cell_of_e = t_of_e * R + etype

    # global row in the piece-contiguous shared table: each AllGather piece's
    # output [(p*ncores + c)*PROWS + pos] is a contiguous slice, so the
    # collective writes it directly (no re-fold) and gathers read it directly.
    PROWS = PADN // NPIECE

    def grow(i):
        c = i // NOWN
        n = i - c * NOWN
        p = n // PROWS
        pos = n - p * PROWS
        return ((p * ncores + c) * PROWS + pos).astype(np.int32)

    gsrc = grow(src)

    counts = np.zeros((ncores, NCELL), np.int64)
    for c in range(ncores):
        counts[c] = np.bincount(cell_of_e[owner == c], minlength=NCELL)
    nch = -(-counts.max(axis=0) // P)  # chunks per cell (0 if empty everywhere)
    chunk_start = np.zeros(NCELL, np.int64)
    chunk_start[1:] = np.cumsum(nch)[:-1]
    TC = int(nch.sum())

    # pad slots: src row 0 (valid gather), norm 0 -> zero contribution
    srcT = np.zeros((ncores, P, TC), np.int32)
    dstlocT = np.zeros((ncores, P, TC), np.float32)
    normT = np.zeros((ncores, P, TC), np.float32)
    for c in range(ncores):
        m = owner == c
        eidx = np.where(m)[0]
        cell = cell_of_e[eidx]
        order = np.argsort(cell, kind="stable")
        eidx = eidx[order]
        cell = cell[order]
        cstart = np.zeros(NCELL, np.int64)
        cstart[1:] = np.cumsum(counts[c])[:-1]
        rank = np.arange(len(eidx)) - cstart[cell]
        col = chunk_start[cell] + rank // P
        part = rank % P
        srcT[c, part, col] = gsrc[eidx]
        dstlocT[c, part, col] = dstloc_of_e[eidx]
        normT[c, part, col] = norm[eidx]

    # host-side embedding lookup into the piece-contiguous bf16 table
    h0f = emb[node_ids].astype(np.float32)  # [N, H]
    h0 = h0f.astype(BF16)
    NFULL = ncores * PADN
    h0full = np.zeros((NFULL, H), BF16)
    h0full[grow(np.arange(N))] = h0

    # per-core pretransposed own features (local node order, f32) for the
    # layer-1 self-loop: x0ownT[p, h*PADN + n] = x0own[n, h*P + p]
    x0ownT = np.zeros((ncores, P, 2 * PADN), np.float32)
    for c in range(ncores):
        xo = np.zeros((PADN, H), np.float32)
        xo[:NOWN] = h0f[c * NOWN : (c + 1) * NOWN]
        t = np.ascontiguousarray(xo.T).reshape(2, P, PADN)
        x0ownT[c] = t.transpose(1, 0, 2).reshape(P, 2 * PADN)

    # gather index tables in dma_gather wrapped-int16 layout, per group
    srcW = np.zeros((ncores, P, TC * 8), np.int16)
    for c in range(ncores):
        for g0, gw in _groups(TC):
            srcW[c, :, g0 * 8 : (g0 + gw) * 8] = _wrap_idx(srcT[c][:, g0 : g0 + gw])

    # head pair indices, remapped to the shared-table layout, drug/target
    # interleaved: chunk col 2q = drugs of pair-chunk q, col 2q+1 = targets
    dtW = np.zeros((ncores, P, 2 * Q * 8), np.int16)
    for c in range(ncores):
        dtT = np.zeros((P, 2 * Q), np.int32)
        d = grow(drugs[c * PPC : (c + 1) * PPC]).reshape(Q, P)
        t = grow(targets[c * PPC : (c + 1) * PPC]).reshape(Q, P)
        dtT[:, 0::2] = d.T
        dtT[:, 1::2] = t.T
        dtW[c] = _wrap_idx(dtT)

    # relation block weights as lhsT [if_local, of_local] per (layer, rel, half)
    B = int(inputs["w1"].shape[1])
    si = H // B
    hb = P // si  # blocks per half
    wblk = np.zeros((2, R, 2, P, P), np.float32)
    for l, W in enumerate([inputs["w1"], inputs["w2"]]):
        W = np.asarray(W, np.float32)
        for r in range(R):
            for h in range(2):
                for bb in range(hb):
                    b = hb * h + bb
                    wblk[l, r, h, bb * si : (bb + 1) * si, bb * si : (bb + 1) * si] = W[r, b]
    wblk_in = wblk.transpose(3, 0, 1, 2, 4).reshape(P, 2 * R * 2 * P).copy()

    loopw = np.stack(
        [np.asarray(inputs["loop_w1"], np.float32), np.asarray(inputs["loop_w2"], np.float32)]
    )  # [2, H, H]
    loopw_in = loopw.reshape(2, 2, P, H).transpose(2, 0, 1, 3).reshape(P, 2 * 2 * H).copy()

    # bias as per-partition columns for the transposed pipeline:
    # biasT[p, l*2 + h] = b_l[h*128 + p]
    biasT_in = np.zeros((P, 4), np.float32)
    for l, b in enumerate([inputs["b1"], inputs["b2"]]):
        b = np.asarray(b, np.float32)
        for h in range(2):
            biasT_in[:, l * 2 + h] = b[h * P : (h + 1) * P]

    d2 = 2 * H
    KC = d2 // P  # fc1 contraction chunks
    MC = d2 // P  # fc1 output chunks
    fc1_in = (
        np.asarray(inputs["fc1_W"], np.float32)
        .reshape(KC, P, MC, P)
        .transpose(1, 0, 2, 3)
        .reshape(P, KC * MC * P)
        .copy()
    )
    fc1b_in = np.asarray(inputs["fc1_b"], np.float32).reshape(MC, P).T.copy()
    fc2_in = np.asarray(inputs["fc2_W"], np.float32).reshape(MC, P).T.copy()
    fc2b = float(np.asarray(inputs["fc2_b"]).reshape(-1)[0])

    iota = np.tile(np.arange(2 * P, dtype=np.float32), (P, 1)).astype(BF16)

    meta = dict(
        N=N, H=H, R=R, NOWN=NOWN, TILES=TILES, PADN=PADN, T2=T2, NCELL=NCELL,
        TC=TC, Q=Q, KC=KC, MC=MC, nch=nch, chunk_start=chunk_start, fc2b=fc2b,
    )
    shared = dict(
        h0full=h0full, iota=iota, wblk=wblk_in, loopw=loopw_in, biasT=biasT_in,
        fc1=fc1_in, fc1b=fc1b_in, fc2=fc2_in,
    )
    in_maps = []
    for c in range(ncores):
        m = dict(shared)
        m.update(
            srcW=srcW[c], dstlocT=dstlocT[c], normT=normT[c],
            x0ownT=x0ownT[c], dtW=dtW[c],
        )
        in_maps.append(m)
    return meta, in_maps


def _build(meta, ncores=NCORES, single=False):
    from concourse import bass, mybir, tile, bacc, library_config
    from concourse.masks import make_identity

    N, H, R = meta["N"], meta["H"], meta["R"]
    TILES, PADN, TC, Q = meta["TILES"], meta["PADN"], meta["TC"], meta["Q"]
    T2 = meta["T2"]
    KC, MC = meta["KC"], meta["MC"]
    nch, chunk_start = meta["nch"], meta["chunk_start"]
    NFULL = NCORES * PADN  # same table shape in both builds
    f32 = mybir.dt.float32
    f32r = mybir.dt.float32r
    bf = mybir.dt.bfloat16
    i16 = mybir.dt.int16

    assert PADN % NPIECE == 0 and TILES % NPIECE == 0
    PROWS = PADN // NPIECE
    TPP = TILES // NPIECE

    nc = bacc.Bacc(
        "TRN2", target_bir_lowering=False, debug=False,
        num_devices=(1 if single else ncores),
    )

    h0full_t = nc.dram_tensor("h0full", [NFULL, H], bf, kind="ExternalInput")
    srcW_t = nc.dram_tensor("srcW", [P, TC * 8], i16, kind="ExternalInput")
    dstlocT_t = nc.dram_tensor("dstlocT", [P, TC], f32, kind="ExternalInput")
    normT_t = nc.dram_tensor("normT", [P, TC], f32, kind="ExternalInput")
    x0ownT_t = nc.dram_tensor("x0ownT", [P, 2 * PADN], f32r, kind="ExternalInput")
    dtW_t = nc.dram_tensor("dtW", [P, 2 * Q * 8], i16, kind="ExternalInput")
    iota_t = nc.dram_tensor("iota", [P, 2 * P], bf, kind="ExternalInput")
    wblk_t = nc.dram_tensor("wblk", [P, 2 * R * 2 * P], f32r, kind="ExternalInput")
    loopw_t = nc.dram_tensor("loopw", [P, 2 * 2 * H], f32r, kind="ExternalInput")
    biasT_t = nc.dram_tensor("biasT", [P, 4], f32, kind="ExternalInput")
    fc1_t = nc.dram_tensor("fc1", [P, KC * MC * P], f32r, kind="ExternalInput")
    fc1b_t = nc.dram_tensor("fc1b", [P, MC], f32, kind="ExternalInput")
    fc2_t = nc.dram_tensor("fc2", [P, MC], f32r, kind="ExternalInput")
    out_t = nc.dram_tensor("out", [Q * P, 1], f32, kind="ExternalOutput")

    with tile.TileContext(nc) as tc:
        with (
            tc.tile_pool(name="const", bufs=1) as cp,
            tc.tile_pool(name="work", bufs=1) as wp,
            tc.tile_pool(name="ps", bufs=1, space="PSUM") as pp,
        ):
            # layer tables: h1 bf16 (gathered by layer-2 edges), h2 f32
            # (gathered only by the head; precision matters there)
            tdt = {1: bf, 2: f32r}
            agin = {}
            agout = {}
            for li in (1, 2):
                agin[li] = nc.dram_tensor(
                    f"h{li}_agin", [PADN, H], tdt[li], kind="Internal"
                ).ap()
                agout[li] = nc.dram_tensor(
                    f"h{li}_agout", [NFULL, H], tdt[li], kind="Internal",
                    addr_space=("Local" if single else "Shared"),
                ).ap()

            # ---- resident constants ----
            srcW_sb = cp.tile([P, TC * 8], i16, name="srcW_sb")
            nc.sync.dma_start(srcW_sb[:], srcW_t.ap()[:])
            dstlocT = cp.tile([P, TC], f32, name="dstlocT")
            nc.sync.dma_start(dstlocT[:], dstlocT_t.ap()[:])
            normT = cp.tile([P, TC], f32, name="normT")
            nc.sync.dma_start(normT[:], normT_t.ap()[:])
            iota_sb = cp.tile([P, 2 * P], bf, name="iota_sb")
            nc.sync.dma_start(iota_sb[:], iota_t.ap()[:])
            loopw_sb = cp.tile([P, 2 * 2 * H], f32r, name="loopw_sb")
            nc.sync.dma_start(loopw_sb[:], loopw_t.ap()[:])
            biasT_sb = cp.tile([P, 4], f32, name="biasT_sb")
            nc.sync.dma_start(biasT_sb[:], biasT_t.ap()[:])
            # first self-loop slices must not queue behind the 4MB wblk DMA
            xsl_warm = {}
            for t2w in (0, 1):
                for hw_ in (0, 1):
                    xslw = wp.tile([P, 2 * P], f32r, name="xsl", tag="xsl", bufs=6)
                    nc.sync.dma_start(
                        xslw[:],
                        x0ownT_t.ap()[:, hw_ * PADN + t2w * 2 * P : hw_ * PADN + (t2w + 1) * 2 * P],
                    )
                    xsl_warm[(t2w, hw_)] = xslw
            # wblk split by layer so layer-0's half lands before its first
            # W-apply; layer-1's half streams in behind it
            wblk_sb = cp.tile([P, 2 * R * 2 * P], f32r, name="wblk_sb")
            WHALF = R * 2 * P
            nc.sync.dma_start(wblk_sb[:, :WHALF], wblk_t.ap()[:, :WHALF])
            nc.sync.dma_start(wblk_sb[:, WHALF:], wblk_t.ap()[:, WHALF:])
            # head constants: emitted late (only needed after both layers)
            dtW_sb = cp.tile([P, 2 * Q * 8], i16, name="dtW_sb")
            fc1_sb = cp.tile([P, KC * MC * P], f32r, name="fc1_sb")
            fc1b_sb = cp.tile([P, MC], f32, name="fc1b_sb")
            fc2_sb = cp.tile([P, MC], f32r, name="fc2_sb")
            ident_f = cp.tile([P, P], f32, name="ident_f")
            make_identity(nc, ident_f[:])
            ident = cp.tile([P, P], f32r, name="ident")
            nc.vector.tensor_copy(ident[:], ident_f[:])
            nc.gpsimd.load_library(library_config.mlp)  # Q7 dma_gather ucode
            # resident transposed layer-1 activations (written by layer 1,
            # self-loop rhs for layer 2): h1T[p, h*PADN + n] = h1[n, h*P + p]
            h1T_sb = cp.tile([P, 2 * PADN], f32r, name="h1T_sb")

            def wblk_ap(l, r, h):
                o = ((l * R + r) * 2 + h) * P
                return wblk_sb[:, o : o + P]

            def loopw_ap(l, h):
                o = (l * 2 + h) * H
                return loopw_sb[:, o : o + H]

            def emit_ag_piece(li, pi):
                row0 = pi * PROWS
                orow0 = pi * ncores * PROWS
                if single:
                    nc.sync.dma_start(
                        agout[li][orow0 : orow0 + PROWS, :],
                        agin[li][row0 : row0 + PROWS, :],
                    )
                    return
                nc.gpsimd.collective_compute(
                    "AllGather", mybir.AluOpType.bypass,
                    replica_groups=[list(range(ncores))],
                    ins=[agin[li][row0 : row0 + PROWS, :]],
                    outs=[agout[li][orow0 : orow0 + ncores * PROWS, :]],
                )

            x0ownT_cache = {}
            groups = _groups(TC)
            GBMAX = max(gw for _, gw in groups)
            col2group = {}
            for gi, (g0, gw) in enumerate(groups):
                for c in range(g0, g0 + gw):
                    col2group[c] = gi

            def emit_w(l, sbs, r, last_rel, msgT):
                for h in range(2):
                    nc.tensor.matmul(
                        msgT[h][:], lhsT=wblk_ap(l, r, h), rhs=sbs[h][:],
                        start=False, stop=last_rel,
                    )

            def layer(l, xsrc_ap, xT_fn, li, hT_dst):
                group_tiles = {}

                def get_xg(col):
                    gi = col2group[col]
                    if gi not in group_tiles:
                        g0, gw = groups[gi]
                        xg = wp.tile([P, GBMAX * H], bf, name="xg", tag="xg", bufs=3)
                        dst3 = xg[:, : gw * H].rearrange("p (c w) -> p c w", w=H)
                        nc.gpsimd.dma_gather(
                            dst3, xsrc_ap, srcW_sb[:, g0 * 8 : (g0 + gw) * 8],
                            gw * P, gw * P, H, single_packet=False,
                        )
                        group_tiles[gi] = xg
                    return group_tiles[gi], col - groups[col2group[col]][0]

                pend_out = None  # deferred transpose/store closure of prev t2
                for t2 in range(T2):
                    rels = [r for r in range(R) if nch[t2 * R + r] > 0]
                    msgT = {}
                    for h in range(2):
                        msgT[h] = pp.tile(
                            [P, 2 * P], f32, name=f"msgT{h}", tag=f"msgT{h}", bufs=2
                        )
                    # self-loop starts the msgT accumulation:
                    # msgT[ho][of, n] += sum_f loopw[f, of] * xT[f, n]
                    for h in range(2):
                        xT = xT_fn(t2, h)
                        for ho in range(2):
                            nc.tensor.matmul(
                                msgT[ho][:],
                                lhsT=loopw_ap(l, h)[:, ho * P : (ho + 1) * P],
                                rhs=xT, start=(h == 0), stop=(h == 1 and not rels),
                            )
                    if pend_out is not None:
                        pend_out()
                        pend_out = None
                    pend = None  # deferred W-apply (aggT sb tiles, rel, last?)
                    Spend = None  # (r, cs, n, S tiles) built one cell ahead

                    def build_S(r):
                        cell = t2 * R + r
                        cs = int(chunk_start[cell])
                        n = int(nch[cell])
                        Ss = []
                        for ci in range(n):
                            col = cs + ci
                            S = wp.tile([P, 2 * P], bf, name="S", tag="S", bufs=8)
                            nc.vector.tensor_scalar(
                                out=S[:], in0=iota_sb[:],
                                scalar1=dstlocT[:, col : col + 1],
                                scalar2=normT[:, col : col + 1],
                                op0=mybir.AluOpType.is_equal, op1=mybir.AluOpType.mult,
                            )
                            Ss.append(S)
                        return (r, cs, n, Ss)

                    def run_cell(ri, spec):
                        nonlocal pend
                        r, cs, n, Ss = spec
                        aggT_ps = [
                            pp.tile([P, 2 * P], f32, name=f"agg{h}", tag=f"agg{h}", bufs=2)
                            for h in range(2)
                        ]
                        for ci in range(n):
                            col = cs + ci
                            xg, off = get_xg(col)
                            for h in range(2):
                                nc.tensor.matmul(
                                    aggT_ps[h][:],
                                    lhsT=xg[:, off * H + h * P : off * H + (h + 1) * P],
                                    rhs=Ss[ci][:], start=(ci == 0), stop=(ci == n - 1),
                                )
                        sbs = []
                        for h in range(2):
                            aggT_sb = wp.tile(
                                [P, 2 * P], f32r, name=f"aggsb{h}", tag=f"aggsb{h}", bufs=2
                            )
                            if (2 * ri + h) % 3 == 0:
                                nc.vector.tensor_copy(aggT_sb[:], aggT_ps[h][:])
                            else:
                                nc.scalar.copy(aggT_sb[:], aggT_ps[h][:])
                            sbs.append(aggT_sb)
                        if pend is not None:
                            emit_w(l, *pend)
                        pend = (sbs, r, ri == len(rels) - 1, msgT)

                    for ri, r in enumerate(rels):
                        spec = build_S(r)
                        if Spend is not None:
                            run_cell(ri - 1, Spend)
                        Spend = spec
                    if Spend is not None:
                        run_cell(len(rels) - 1, Spend)
                    if pend is not None:
                        emit_w(l, *pend)

                    # bias -> hT tiles (f32r); transposes/stores are deferred
                    # past the next t2's self-loop so PE doesn't wait on DVE
                    hTs = {}
                    for h in range(2):
                        hT = hT_dst(t2, h)
                        bcol = biasT_sb[:, l * 2 + h : l * 2 + h + 1]
                        if h == 0:
                            nc.vector.tensor_scalar(
                                out=hT, in0=msgT[h][:], scalar1=bcol,
                                scalar2=None, op0=mybir.AluOpType.add,
                            )
                        else:
                            nc.scalar.add(hT, msgT[h][:], bcol)
                        hTs[h] = hT

                    def make_out(t2=t2, hTs=hTs):
                        def go():
                            for si_ in range(2):
                                st = 2 * t2 + si_
                                out_sb = wp.tile(
                                    [P, H], tdt[li], name="outsb", tag="outsb", bufs=3
                                )
                                for h in range(2):
                                    tp = pp.tile(
                                        [P, P], f32r, name="tp", tag="agg0", bufs=2
                                    )
                                    nc.tensor.transpose(
                                        tp[:], hTs[h][:, si_ * P : (si_ + 1) * P],
                                        ident[:],
                                    )
                                    eng = nc.vector if h == 0 else nc.scalar
                                    if h == 0:
                                        nc.vector.tensor_copy(
                                            out_sb[:, h * P : (h + 1) * P], tp[:]
                                        )
                                    else:
                                        nc.scalar.copy(
                                            out_sb[:, h * P : (h + 1) * P], tp[:]
                                        )
                                nc.sync.dma_start(
                                    agin[li][st * P : (st + 1) * P, :], out_sb[:]
                                )
                                if (st + 1) % TPP == 0:
                                    emit_ag_piece(li, (st + 1) // TPP - 1)

                        return go

                    pend_out = make_out()
                pend_out()

            # xT providers return [128 (feat half h), 256 nodes] f32r slices

            x0ownT_cache.update(xsl_warm)

            def xT_l1(t2, h):
                if (t2, h) not in x0ownT_cache:
                    xsl = wp.tile([P, 2 * P], f32r, name="xsl", tag="xsl", bufs=6)
                    nc.sync.dma_start(
                        xsl[:],
                        x0ownT_t.ap()[:, h * PADN + t2 * 2 * P : h * PADN + (t2 + 1) * 2 * P],
                    )
                    x0ownT_cache[(t2, h)] = xsl
                return x0ownT_cache[(t2, h)][:]

            def xT_l2(t2, h):
                return h1T_sb[:, h * PADN + t2 * 2 * P : h * PADN + (t2 + 1) * 2 * P]

            def hT_dst_l1(t2, h):
                return h1T_sb[:, h * PADN + t2 * 2 * P : h * PADN + (t2 + 1) * 2 * P]

            h2T_tiles = {}

            def hT_dst_l2(t2, h):
                hv = wp.tile([P, 2 * P], f32r, name="h2T", tag="h2T", bufs=4)
                h2T_tiles[(t2, h)] = hv
                return hv[:]

            layer(0, h0full_t.ap()[:], xT_l1, 1, hT_dst_l1)
            nc.sync.dma_start(dtW_sb[:], dtW_t.ap()[:])
            nc.sync.dma_start(fc1_sb[:], fc1_t.ap()[:])
            nc.sync.dma_start(fc1b_sb[:], fc1b_t.ap()[:])
            nc.sync.dma_start(fc2_sb[:], fc2_t.ap()[:])
            layer(1, agout[1][:], xT_l2, 2, hT_dst_l2)

            # ---- MLP head over this core's Q*P pairs (f32r, 256-pair sweeps)
            xcat_big = wp.tile([P, 2 * Q * H], f32r, name="xcat_big", tag="xcat", bufs=1)
            nc.gpsimd.dma_gather(
                xcat_big[:].rearrange("p (c w) -> p c w", w=H),
                agout[2][:], dtW_sb[:], 2 * Q * P, 2 * Q * P, H,
                single_packet=False,
            )
            qblocks = []
            q0 = 0
            while q0 < Q:
                nq = min(2, Q - q0)
                qblocks.append((q0, nq))
                q0 += nq
            for q0, nq in qblocks:
                PW = nq * P  # pairs in this sweep
                # pair j: q = q0 + j//128, p = j%128
                # xcatT[k][f, j]: f-chunk k of [drug(0:KC/2) | target] halves
                xcatT = []
                for k in range(KC):
                    xcT = wp.tile([P, 2 * P], f32r, name=f"xcT{k}", tag=f"xcT{k}", bufs=1)
                    for jh in range(nq):
                        q = q0 + jh
                        c = 2 * q + (1 if k >= KC // 2 else 0)
                        off = c * H + (k % (KC // 2)) * P
                        tp = pp.tile([P, P], f32r, name="tpm", tag="agg0", bufs=2)
                        nc.tensor.transpose(
                            tp[:], xcat_big[:, off : off + P], ident[:]
                        )
                        if jh == 0:
                            nc.vector.tensor_copy(xcT[:, jh * P : (jh + 1) * P], tp[:])
                        else:
                            nc.scalar.copy(xcT[:, jh * P : (jh + 1) * P], tp[:])
                    xcatT.append(xcT)
                z_ps = pp.tile([1, PW], f32, name="z", tag="msgT1", bufs=2)
                ypend = None  # deferred fc2 matmul, same PE-stall dodge as emit_w
                for m in range(MC):
                    yT_ps = pp.tile(
                        [P, PW], f32, name="yT",
                        tag=("msgT0" if m % 2 == 0 else "agg1"), bufs=2,
                    )
                    for k in range(KC):
                        nc.tensor.matmul(
                            yT_ps[:], lhsT=fc1_sb[:, (k * MC + m) * P : (k * MC + m + 1) * P],
                            rhs=xcatT[k][:, :PW], start=(k == 0), stop=(k == KC - 1),
                        )
                    yTr = wp.tile([P, 2 * P], f32r, name="yTr", tag="yTr", bufs=2)
                    nc.scalar.activation(
                        yTr[:, :PW], yT_ps[:], mybir.ActivationFunctionType.Relu,
                        bias=fc1b_sb[:, m : m + 1], scale=1.0,
                    )
                    if ypend is not None:
                        nc.tensor.matmul(
                            z_ps[:], lhsT=fc2_sb[:, ypend[1] : ypend[1] + 1],
                            rhs=ypend[0][:, :PW], start=(ypend[1] == 0), stop=False,
                        )
                    ypend = (yTr, m)
                nc.tensor.matmul(
                    z_ps[:], lhsT=fc2_sb[:, ypend[1] : ypend[1] + 1],
                    rhs=ypend[0][:, :PW], start=False, stop=True,
                )
                zs = wp.tile([1, 2 * P], f32, name="zs", tag="zs", bufs=2)
                nc.scalar.activation(
                    zs[:, :PW], z_ps[:], mybir.ActivationFunctionType.Sigmoid,
                    bias=meta["fc2b"], scale=1.0,
                )
                nc.sync.dma_start(
                    out_t.ap()[q0 * P : q0 * P + PW, :], zs[:, :PW]
                )
    return nc


_NC_CACHE = []


def kernel(**inputs):
    from concourse import bass_utils

    meta, in_maps = _preprocess(inputs)
    key = (meta["N"], meta["H"], meta["R"], meta["TC"], meta["Q"],
           tuple(int(x) for x in meta["nch"]))
    if _NC_CACHE and _NC_CACHE[0][0] == key:
        nc = _NC_CACHE[0][1]
    else:
        nc = _build(meta)
        nc.compile()
        _NC_CACHE[:] = [(key, nc)]
    res = bass_utils.run_bass_kernel_spmd(nc, in_maps, core_ids=list(range(NCORES)))
    out = np.concatenate([res.results[c]["out"] for c in range(NCORES)], axis=0)
    return out.astype(np.float32)


# revision 32
# speedup vs baseline: 3.2333x; 1.0065x over previous
"""Trainium2 Bass kernel for the DTI R-GCN (bdd) model, 8 NeuronCores.

Strategy (SPMD, one program, per-core data):
  - dst-shard the graph: core c owns nodes [c*2500, (c+1)*2500); host routes
    each edge to its dst owner and buckets it into (dst-tile, relation) cells,
    tiles of 256 dst nodes.
  - node features are bf16 in a piece-contiguous shared table (row =
    (piece*8 + core)*640 + pos) so each AllGather piece's output is a
    contiguous slice written by the collective directly, and both layers'
    gathers (plus the head's) read it with one host-remapped index space.
  - gathers use InstDMAGatherAnt (gpsimd.dma_gather, mlp Q7 library): one
    instruction fetches up to 48 chunks x 128 rows via an int16 index list,
    amortizing the ~1us SWDGE fixed cost; multi-packet mode for >1024 rows.
  - per 128-edge chunk: a scatter matrix S[e, d] = norm_e * (iota == dstloc_e)
    (one bf16 tensor_scalar) and two bf16 matmuls xg^T @ S accumulate the
    transposed per-(tile, rel) aggregate aggT[feat half, dst] in fp32 PSUM.
    Pad slots use src row 0 with norm 0 (contribution exactly zero).
  - everything downstream of the edge aggregation runs in float32r (tf32-ish,
    1 cycle/row at 256-wide moving dim -- same PE cost as bf16, ~16x less
    rounding error): per (cell, rel, half) one matmul msgT[of, 256 nodes] +=
    wblk^T aggT accumulates transposed messages; the self-loop joins the same
    PSUM accumulation with lhsT=loop_w slices and rhs=x^T (host-pretransposed
    f32 for layer 1, the resident SBUF h1T written by layer 1 for layer 2).
    The W-apply of relation r is emitted after the scatter of r+1 so PE never
    waits on the aggT PSUM->SBUF copy.
  - bias lands via per-partition tensor_scalar into h1T (resident, f32r) or a
    transient h2T tile; PE transposes produce the row-layout table tiles
    (bf16 for h1, f32 for h2 feeding the head) which are DMA'd to agin and
    AllGather'd in 4 pieces fired as soon as their tiles are stored.
  - MLP head (f32r): ONE dma_gather fetches drug+target rows for all pairs
    from the f32 h2 table; fc1/fc2 run 256 pairs per matmul sweep.

The program is built fresh per invocation from the actual inputs (host does
all index preprocessing; trip counts are data-dependent but identical across
cores by padding cells to the max chunk count over cores).
"""
import sys

sys.path.insert(0, "/opt/trn_rl_repo")
import numpy as np
import ml_dtypes

BF16 = ml_dtypes.bfloat16
P = 128
NCORES = 8
NPIECE = 4  # AllGather pieces per layer; full table is piece-contiguous


def _groups(TC):
    """dma_gather group boundaries over chunk columns: small leading groups
    so PE starts quickly, then big groups to amortize instruction overhead."""
    out = []
    g0 = 0
    for gw in (16, 16, 32):
        if g0 >= TC:
            return out
        gw = min(gw, TC - g0)
        out.append((g0, gw))
        g0 += gw
    while g0 < TC:
        gw = min(48, TC - g0)
        out.append((g0, gw))
        g0 += gw
    return out


def _wrap_idx(cols):
    """[P, K] int chunk-column indices -> [128, K*8] int16 dma_gather layout:
    linear i = c*128 + p; wrapped[p', s] = linear[s*16 + p' % 16]."""
    K = cols.shape[1]
    lin = cols.T.reshape(-1)  # [K*128]
    w16 = lin.reshape(K * 8, 16).T  # [16, K*8]
    return np.tile(w16, (8, 1)).astype(np.int16)


def _preprocess(inputs, ncores=NCORES):
    node_ids = np.asarray(inputs["node_ids"])
    src = np.asarray(inputs["src"])
    dst = np.asarray(inputs["dst"])
    etype = np.asarray(inputs["etype"])
    norm = np.asarray(inputs["norm"]).reshape(-1)
    emb = np.asarray(inputs["emb"], dtype=np.float32)
    drugs = np.asarray(inputs["drugs_index"])
    targets = np.asarray(inputs["targets_index"])

    N = node_ids.shape[0]
    H = emb.shape[1]
    R = int(inputs["w1"].shape[0])
    PAIRS = drugs.shape[0]
    assert N % ncores == 0 and PAIRS % ncores == 0
    NOWN = N // ncores
    TILES = -(-NOWN // P)
    PADN = TILES * P
    PPC = PAIRS // ncores
    assert PPC % P == 0
    Q = PPC // P

    TILE2 = 2 * P  # dst nodes per aggregation cell (segments on matmul free dim)
    T2 = -(-NOWN // TILE2)
    NCELL = T2 * R
    owner = dst // NOWN
    d_local = dst - owner * NOWN
    t_of_e = d_local // TILE2
    dstloc_of_e = (d_local % TILE2).astype(np.float32)
    cell_of_e = t_of_e * R + etype

    # global row in the piece-contiguous shared table: each AllGather piece's
    # output [(p*ncores + c)*PROWS + pos] is a contiguous slice, so the
    # collective writes it directly (no re-fold) and gathers read it directly.
    PROWS = PADN // NPIECE

    def grow(i):
        c = i // NOWN
        n = i - c * NOWN
        p = n // PROWS
        pos = n - p * PROWS
        return ((p * ncores + c) * PROWS + pos).astype(np.int32)

    gsrc = grow(src)

    counts = np.zeros((ncores, NCELL), np.int64)
    for c in range(ncores):
        counts[c] = np.bincount(cell_of_e[owner == c], minlength=NCELL)
    nch = -(-counts.max(axis=0) // P)  # chunks per cell (0 if empty everywhere)
    chunk_start = np.zeros(NCELL, np.int64)
    chunk_start[1:] = np.cumsum(nch)[:-1]
    TC = int(nch.sum())

    # pad slots: src row 0 (valid gather), norm 0 -> zero contribution
    srcT = np.zeros((ncores, P, TC), np.int32)
    dstlocT = np.zeros((ncores, P, TC), np.float32)
    normT = np.zeros((ncores, P, TC), np.float32)
    for c in range(ncores):
        m = owner == c
        eidx = np.where(m)[0]
        cell = cell_of_e[eidx]
        order = np.argsort(cell, kind="stable")
        eidx = eidx[order]
        cell = cell[order]
        cstart = np.zeros(NCELL, np.int64)
        cstart[1:] = np.cumsum(counts[c])[:-1]
        rank = np.arange(len(eidx)) - cstart[cell]
        col = chunk_start[cell] + rank // P
        part = rank % P
        srcT[c, part, col] = gsrc[eidx]
        dstlocT[c, part, col] = dstloc_of_e[eidx]
        normT[c, part, col] = norm[eidx]

    # host-side embedding lookup into the piece-contiguous bf16 table
    h0f = emb[node_ids].astype(np.float32)  # [N, H]
    h0 = h0f.astype(BF16)
    NFULL = ncores * PADN
    h0full = np.zeros((NFULL, H), BF16)
    h0full[grow(np.arange(N))] = h0

    # per-core pretransposed own features (local node order, f32) for the
    # layer-1 self-loop: x0ownT[p, h*PADN + n] = x0own[n, h*P + p]
    x0ownT = np.zeros((ncores, P, 2 * PADN), np.float32)
    for c in range(ncores):
        xo = np.zeros((PADN, H), np.float32)
        xo[:NOWN] = h0f[c * NOWN : (c + 1) * NOWN]
        t = np.ascontiguousarray(xo.T).reshape(2, P, PADN)
        x0ownT[c] = t.transpose(1, 0, 2).reshape(P, 2 * PADN)

    # gather index tables in dma_gather wrapped-int16 layout, per group
    srcW = np.zeros((ncores, P, TC * 8), np.int16)
    for c in range(ncores):
        for g0, gw in _groups(TC):
            srcW[c, :, g0 * 8 : (g0 + gw) * 8] = _wrap_idx(srcT[c][:, g0 : g0 + gw])

    # head pair indices, remapped to the shared-table layout, drug/target
    # interleaved: chunk col 2q = drugs of pair-chunk q, col 2q+1 = targets
    dtW = np.zeros((ncores, P, 2 * Q * 8), np.int16)
    for c in range(ncores):
        dtT = np.zeros((P, 2 * Q), np.int32)
        d = grow(drugs[c * PPC : (c + 1) * PPC]).reshape(Q, P)
        t = grow(targets[c * PPC : (c + 1) * PPC]).reshape(Q, P)
        dtT[:, 0::2] = d.T
        dtT[:, 1::2] = t.T
        dtW[c] = _wrap_idx(dtT)

    # relation block weights as lhsT [if_local, of_local] per (layer, rel, half)
    B = int(inputs["w1"].shape[1])
    si = H // B
    hb = P // si  # blocks per half
    wblk = np.zeros((2, R, 2, P, P), np.float32)
    for l, W in enumerate([inputs["w1"], inputs["w2"]]):
        W = np.asarray(W, np.float32)
        for r in range(R):
            for h in range(2):
                for bb in range(hb):
                    b = hb * h + bb
                    wblk[l, r, h, bb * si : (bb + 1) * si, bb * si : (bb + 1) * si] = W[r, b]
    wblk_in = wblk.transpose(3, 0, 1, 2, 4).reshape(P, 2 * R * 2 * P).copy()

    loopw = np.stack(
        [np.asarray(inputs["loop_w1"], np.float32), np.asarray(inputs["loop_w2"], np.float32)]
    )  # [2, H, H]
    loopw_in = loopw.reshape(2, 2, P, H).transpose(2, 0, 1, 3).reshape(P, 2 * 2 * H).copy()

    # bias as per-partition columns for the transposed pipeline:
    # biasT[p, l*2 + h] = b_l[h*128 + p]
    biasT_in = np.zeros((P, 4), np.float32)
    for l, b in enumerate([inputs["b1"], inputs["b2"]]):
        b = np.asarray(b, np.float32)
        for h in range(2):
            biasT_in[:, l * 2 + h] = b[h * P : (h + 1) * P]

    d2 = 2 * H
    KC = d2 // P  # fc1 contraction chunks
    MC = d2 // P  # fc1 output chunks
    fc1_in = (
        np.asarray(inputs["fc1_W"], np.float32)
        .reshape(KC, P, MC, P)
        .transpose(1, 0, 2, 3)
        .reshape(P, KC * MC * P)
        .copy()
    )
    fc1b_in = np.asarray(inputs["fc1_b"], np.float32).reshape(MC, P).T.copy()
    fc2_in = np.asarray(inputs["fc2_W"], np.float32).reshape(MC, P).T.copy()
    fc2b = float(np.asarray(inputs["fc2_b"]).reshape(-1)[0])

    iota = np.tile(np.arange(2 * P, dtype=np.float32), (P, 1)).astype(BF16)

    meta = dict(
        N=N, H=H, R=R, NOWN=NOWN, TILES=TILES, PADN=PADN, T2=T2, NCELL=NCELL,
        TC=TC, Q=Q, KC=KC, MC=MC, nch=nch, chunk_start=chunk_start, fc2b=fc2b,
    )
    shared = dict(
        h0full=h0full, iota=iota, wblk=wblk_in, loopw=loopw_in, biasT=biasT_in,
        fc1=fc1_in, fc1b=fc1b_in, fc2=fc2_in,
    )
    in_maps = []
    for c in range(ncores):
        m = dict(shared)
        m.update(
            srcW=srcW[c], dstlocT=dstlocT[c], normT=normT[c],
            x0ownT=x0ownT[c], dtW=dtW[c],
        )
        in_maps.append(m)
    return meta, in_maps


def _build(meta, ncores=NCORES, single=False):
    from concourse import bass, mybir, tile, bacc, library_config
    from concourse.masks import make_identity

    N, H, R = meta["N"], meta["H"], meta["R"]
    TILES, PADN, TC, Q = meta["TILES"], meta["PADN"], meta["TC"], meta["Q"]
    T2 = meta["T2"]
    KC, MC = meta["KC"], meta["MC"]
    nch, chunk_start = meta["nch"], meta["chunk_start"]
    NFULL = NCORES * PADN  # same table shape in both builds
    f32 = mybir.dt.float32
    f32r = mybir.dt.float32r
    bf = mybir.dt.bfloat16
    i16 = mybir.dt.int16

    assert PADN % NPIECE == 0 and TILES % NPIECE == 0
    PROWS = PADN // NPIECE
    TPP = TILES // NPIECE

    nc = bacc.Bacc(
        "TRN2", target_bir_lowering=False, debug=False,
        num_devices=(1 if single else ncores),
    )

    h0full_t = nc.dram_tensor("h0full", [NFULL, H], bf, kind="ExternalInput")
    srcW_t = nc.dram_tensor("srcW", [P, TC * 8], i16, kind="ExternalInput")
    dstlocT_t = nc.dram_tensor("dstlocT", [P, TC], f32, kind="ExternalInput")
    normT_t = nc.dram_tensor("normT", [P, TC], f32, kind="ExternalInput")
    x0ownT_t = nc.dram_tensor("x0ownT", [P, 2 * PADN], f32r, kind="ExternalInput")
    dtW_t = nc.dram_tensor("dtW", [P, 2 * Q * 8], i16, kind="ExternalInput")
    iota_t = nc.dram_tensor("iota", [P, 2 * P], bf, kind="ExternalInput")
    wblk_t = nc.dram_tensor("wblk", [P, 2 * R * 2 * P], f32r, kind="ExternalInput")
    loopw_t = nc.dram_tensor("loopw", [P, 2 * 2 * H], f32r, kind="ExternalInput")
    biasT_t = nc.dram_tensor("biasT", [P, 4], f32, kind="ExternalInput")
    fc1_t = nc.dram_tensor("fc1", [P, KC * MC * P], f32r, kind="ExternalInput")
    fc1b_t = nc.dram_tensor("fc1b", [P, MC], f32, kind="ExternalInput")
    fc2_t = nc.dram_tensor("fc2", [P, MC], f32r, kind="ExternalInput")
    out_t = nc.dram_tensor("out", [Q * P, 1], f32, kind="ExternalOutput")

    with tile.TileContext(nc) as tc:
        with (
            tc.tile_pool(name="const", bufs=1) as cp,
            tc.tile_pool(name="work", bufs=1) as wp,
            tc.tile_pool(name="ps", bufs=1, space="PSUM") as pp,
        ):
            # layer tables: h1 bf16 (gathered by layer-2 edges), h2 f32
            # (gathered only by the head; precision matters there)
            tdt = {1: bf, 2: f32r}
            agin = {}
            agout = {}
            for li in (1, 2):
                agin[li] = nc.dram_tensor(
                    f"h{li}_agin", [PADN, H], tdt[li], kind="Internal"
                ).ap()
                agout[li] = nc.dram_tensor(
                    f"h{li}_agout", [NFULL, H], tdt[li], kind="Internal",
                    addr_space=("Local" if single else "Shared"),
                ).ap()

            # ---- resident constants (ordered so the first gather + first
            # self-loop are not queued behind multi-MB weight loads) ----
            srcW_sb = cp.tile([P, TC * 8], i16, name="srcW_sb")
            SHEAD = min(TC, 64) * 8
            nc.sync.dma_start(srcW_sb[:, :SHEAD], srcW_t.ap()[:, :SHEAD])
            dstlocT = cp.tile([P, TC], f32, name="dstlocT")
            nc.sync.dma_start(dstlocT[:], dstlocT_t.ap()[:])
            normT = cp.tile([P, TC], f32, name="normT")
            nc.sync.dma_start(normT[:], normT_t.ap()[:])
            iota_sb = cp.tile([P, 2 * P], bf, name="iota_sb")
            nc.sync.dma_start(iota_sb[:], iota_t.ap()[:])
            loopw_sb = cp.tile([P, 2 * 2 * H], f32r, name="loopw_sb")
            nc.sync.dma_start(loopw_sb[:], loopw_t.ap()[:])
            biasT_sb = cp.tile([P, 4], f32, name="biasT_sb")
            nc.sync.dma_start(biasT_sb[:], biasT_t.ap()[:])
            # first self-loop slices must not queue behind the 4MB wblk DMA
            xsl_warm = {}
            for t2w in (0, 1):
                for hw_ in (0, 1):
                    xslw = wp.tile([P, 2 * P], f32r, name="xsl", tag="xsl", bufs=6)
                    nc.sync.dma_start(
                        xslw[:],
                        x0ownT_t.ap()[:, hw_ * PADN + t2w * 2 * P : hw_ * PADN + (t2w + 1) * 2 * P],
                    )
                    xsl_warm[(t2w, hw_)] = xslw
            # wblk in quarters (layer-0 rels 0-7 first, layer-1 tail last)
            # so W-applies get their slices without one 4MB DMA monopolizing
            # the engines ahead of the early edge gathers
            wblk_sb = cp.tile([P, 2 * R * 2 * P], f32r, name="wblk_sb")
            WQ = R * 2 * P // 2
            nc.sync.dma_start(wblk_sb[:, :WQ], wblk_t.ap()[:, :WQ])
            nc.sync.dma_start(srcW_sb[:, SHEAD:], srcW_t.ap()[:, SHEAD:])
            for wq in range(1, 4):
                nc.sync.dma_start(
                    wblk_sb[:, wq * WQ : (wq + 1) * WQ],
                    wblk_t.ap()[:, wq * WQ : (wq + 1) * WQ],
                )
            # head constants: emitted late (only needed after both layers)
            dtW_sb = cp.tile([P, 2 * Q * 8], i16, name="dtW_sb")
            fc1_sb = cp.tile([P, KC * MC * P], f32r, name="fc1_sb")
            fc1b_sb = cp.tile([P, MC], f32, name="fc1b_sb")
            fc2_sb = cp.tile([P, MC], f32r, name="fc2_sb")
            ident_f = cp.tile([P, P], f32, name="ident_f")
            make_identity(nc, ident_f[:])
            ident = cp.tile([P, P], f32r, name="ident")
            nc.vector.tensor_copy(ident[:], ident_f[:])
            nc.gpsimd.load_library(library_config.mlp)  # Q7 dma_gather ucode
            # resident transposed layer-1 activations (written by layer 1,
            # self-loop rhs for layer 2): h1T[p, h*PADN + n] = h1[n, h*P + p]
            h1T_sb = cp.tile([P, 2 * PADN], f32r, name="h1T_sb")

            def wblk_ap(l, r, h):
                o = ((l * R + r) * 2 + h) * P
                return wblk_sb[:, o : o + P]

            def loopw_ap(l, h):
                o = (l * 2 + h) * H
                return loopw_sb[:, o : o + H]

            def emit_ag_piece(li, pi):
                row0 = pi * PROWS
                orow0 = pi * ncores * PROWS
                if single:
                    nc.sync.dma_start(
                        agout[li][orow0 : orow0 + PROWS, :],
                        agin[li][row0 : row0 + PROWS, :],
                    )
                    return
                nc.gpsimd.collective_compute(
                    "AllGather", mybir.AluOpType.bypass,
                    replica_groups=[list(range(ncores))],
                    ins=[agin[li][row0 : row0 + PROWS, :]],
                    outs=[agout[li][orow0 : orow0 + ncores * PROWS, :]],
                )

            x0ownT_cache = {}
            groups = _groups(TC)
            GBMAX = max(gw for _, gw in groups)
            col2group = {}
            for gi, (g0, gw) in enumerate(groups):
                for c in range(g0, g0 + gw):
                    col2group[c] = gi

            def emit_w(l, sbs, r, last_rel, msgT):
                for h in range(2):
                    nc.tensor.matmul(
                        msgT[h][:], lhsT=wblk_ap(l, r, h), rhs=sbs[h][:],
                        start=False, stop=last_rel,
                    )

            def layer(l, xsrc_ap, xT_fn, li, hT_dst):
                group_tiles = {}

                def get_xg(col):
                    gi = col2group[col]
                    if gi not in group_tiles:
                        g0, gw = groups[gi]
                        xg = wp.tile([P, GBMAX * H], bf, name="xg", tag="xg", bufs=3)
                        dst3 = xg[:, : gw * H].rearrange("p (c w) -> p c w", w=H)
                        nc.gpsimd.dma_gather(
                            dst3, xsrc_ap, srcW_sb[:, g0 * 8 : (g0 + gw) * 8],
                            gw * P, gw * P, H, single_packet=False,
                        )
                        group_tiles[gi] = xg
                    return group_tiles[gi], col - groups[col2group[col]][0]

                pend_out = None  # deferred transpose/store closure of prev t2
                for t2 in range(T2):
                    rels = [r for r in range(R) if nch[t2 * R + r] > 0]
                    msgT = {}
                    for h in range(2):
                        msgT[h] = pp.tile(
                            [P, 2 * P], f32, name=f"msgT{h}", tag=f"msgT{h}", bufs=2
                        )
                    # self-loop starts the msgT accumulation:
                    # msgT[ho][of, n] += sum_f loopw[f, of] * xT[f, n]
                    for h in range(2):
                        xT = xT_fn(t2, h)
                        for ho in range(2):
                            nc.tensor.matmul(
                                msgT[ho][:],
                                lhsT=loopw_ap(l, h)[:, ho * P : (ho + 1) * P],
                                rhs=xT, start=(h == 0), stop=(h == 1 and not rels),
                            )
                    if pend_out is not None:
                        pend_out()
                        pend_out = None
                    pend = None  # deferred W-apply (aggT sb tiles, rel, last?)
                    Spend = None  # (r, cs, n, S tiles) built one cell ahead

                    def build_S(r):
                        cell = t2 * R + r
                        cs = int(chunk_start[cell])
                        n = int(nch[cell])
                        Ss = []
                        for ci in range(n):
                            col = cs + ci
                            S = wp.tile([P, 2 * P], bf, name="S", tag="S", bufs=8)
                            nc.vector.tensor_scalar(
                                out=S[:], in0=iota_sb[:],
                                scalar1=dstlocT[:, col : col + 1],
                                scalar2=normT[:, col : col + 1],
                                op0=mybir.AluOpType.is_equal, op1=mybir.AluOpType.mult,
                            )
                            Ss.append(S)
                        return (r, cs, n, Ss)

                    def run_cell(ri, spec):
                        nonlocal pend
                        r, cs, n, Ss = spec
                        aggT_ps = [
                            pp.tile([P, 2 * P], f32, name=f"agg{h}", tag=f"agg{h}", bufs=2)
                            for h in range(2)
                        ]
                        for ci in range(n):
                            col = cs + ci
                            xg, off = get_xg(col)
                            for h in range(2):
                                nc.tensor.matmul(
                                    aggT_ps[h][:],
                                    lhsT=xg[:, off * H + h * P : off * H + (h + 1) * P],
                                    rhs=Ss[ci][:], start=(ci == 0), stop=(ci == n - 1),
                                )
                        sbs = []
                        for h in range(2):
                            aggT_sb = wp.tile(
                                [P, 2 * P], f32r, name=f"aggsb{h}", tag=f"aggsb{h}", bufs=2
                            )
                            if (2 * ri + h) % 3 == 0:
                                nc.vector.tensor_copy(aggT_sb[:], aggT_ps[h][:])
                            else:
                                nc.scalar.copy(aggT_sb[:], aggT_ps[h][:])
                            sbs.append(aggT_sb)
                        if pend is not None:
                            emit_w(l, *pend)
                        pend = (sbs, r, ri == len(rels) - 1, msgT)

                    for ri, r in enumerate(rels):
                        spec = build_S(r)
                        if Spend is not None:
                            run_cell(ri - 1, Spend)
                        Spend = spec
                    if Spend is not None:
                        run_cell(len(rels) - 1, Spend)
                    if pend is not None:
                        emit_w(l, *pend)

                    # bias -> hT tiles (f32r); transposes/stores are deferred
                    # past the next t2's self-loop so PE doesn't wait on DVE
                    hTs = {}
                    for h in range(2):
                        hT = hT_dst(t2, h)
                        bcol = biasT_sb[:, l * 2 + h : l * 2 + h + 1]
                        if h == 0:
                            nc.vector.tensor_scalar(
                                out=hT, in0=msgT[h][:], scalar1=bcol,
                                scalar2=None, op0=mybir.AluOpType.add,
                            )
                        else:
                            nc.scalar.add(hT, msgT[h][:], bcol)
                        hTs[h] = hT

                    def make_out(t2=t2, hTs=hTs):
                        def go():
                            for si_ in range(2):
                                st = 2 * t2 + si_
                                out_sb = wp.tile(
                                    [P, H], tdt[li], name="outsb", tag="outsb", bufs=3
                                )
                                for h in range(2):
                                    tp = pp.tile(
                                        [P, P], f32r, name="tp", tag="agg0", bufs=2
                                    )
                                    nc.tensor.transpose(
                                        tp[:], hTs[h][:, si_ * P : (si_ + 1) * P],
                                        ident[:],
                                    )
                                    eng = nc.vector if h == 0 else nc.scalar
                                    if h == 0:
                                        nc.vector.tensor_copy(
                                            out_sb[:, h * P : (h + 1) * P], tp[:]
                                        )
                                    else:
                                        nc.scalar.copy(
                                            out_sb[:, h * P : (h + 1) * P], tp[:]
                                        )
                                nc.sync.dma_start(
                                    agin[li][st * P : (st + 1) * P, :], out_sb[:]
                                )
                                if (st + 1) % TPP == 0:
                                    emit_ag_piece(li, (st + 1) // TPP - 1)

                        return go

                    pend_out = make_out()
                pend_out()

            # xT providers return [128 (feat half h), 256 nodes] f32r slices

            x0ownT_cache.update(xsl_warm)

            def xT_l1(t2, h):
                if (t2, h) not in x0ownT_cache:
                    xsl = wp.tile([P, 2 * P], f32r, name="xsl", tag="xsl", bufs=6)
                    nc.sync.dma_start(
                        xsl[:],
                        x0ownT_t.ap()[:, h * PADN + t2 * 2 * P : h * PADN + (t2 + 1) * 2 * P],
                    )
                    x0ownT_cache[(t2, h)] = xsl
                return x0ownT_cache[(t2, h)][:]

            def xT_l2(t2, h):
                return h1T_sb[:, h * PADN + t2 * 2 * P : h * PADN + (t2 + 1) * 2 * P]

            def hT_dst_l1(t2, h):
                return h1T_sb[:, h * PADN + t2 * 2 * P : h * PADN + (t2 + 1) * 2 * P]

            h2T_tiles = {}

            def hT_dst_l2(t2, h):
                hv = wp.tile([P, 2 * P], f32r, name="h2T", tag="h2T", bufs=4)
                h2T_tiles[(t2, h)] = hv
                return hv[:]

            layer(0, h0full_t.ap()[:], xT_l1, 1, hT_dst_l1)
            nc.sync.dma_start(dtW_sb[:], dtW_t.ap()[:])
            nc.sync.dma_start(fc1_sb[:], fc1_t.ap()[:])
            nc.sync.dma_start(fc1b_sb[:], fc1b_t.ap()[:])
            nc.sync.dma_start(fc2_sb[:], fc2_t.ap()[:])
            layer(1, agout[1][:], xT_l2, 2, hT_dst_l2)

            # ---- MLP head over this core's Q*P pairs (f32r, 256-pair sweeps)
            xcat_big = wp.tile([P, 2 * Q * H], f32r, name="xcat_big", tag="xcat", bufs=1)
            nc.gpsimd.dma_gather(
                xcat_big[:].rearrange("p (c w) -> p c w", w=H),
                agout[2][:], dtW_sb[:], 2 * Q * P, 2 * Q * P, H,
                single_packet=False,
            )
            qblocks = []
            q0 = 0
            while q0 < Q:
                nq = min(2, Q - q0)
                qblocks.append((q0, nq))
                q0 += nq
            for q0, nq in qblocks:
                PW = nq * P  # pairs in this sweep
                # pair j: q = q0 + j//128, p = j%128
                # xcatT[k][f, j]: f-chunk k of [drug(0:KC/2) | target] halves
                xcatT = []
                for k in range(KC):
                    xcT = wp.tile([P, 2 * P], f32r, name=f"xcT{k}", tag=f"xcT{k}", bufs=1)
                    for jh in range(nq):
                        q = q0 + jh
                        c = 2 * q + (1 if k >= KC // 2 else 0)
                        off = c * H + (k % (KC // 2)) * P
                        tp = pp.tile([P, P], f32r, name="tpm", tag="agg0", bufs=2)
                        nc.tensor.transpose(
                            tp[:], xcat_big[:, off : off + P], ident[:]
                        )
                        if jh == 0:
                            nc.vector.tensor_copy(xcT[:, jh * P : (jh + 1) * P], tp[:])
                        else:
                            nc.scalar.copy(xcT[:, jh * P : (jh + 1) * P], tp[:])
                    xcatT.append(xcT)
                z_ps = pp.tile([1, PW], f32, name="z", tag="msgT1", bufs=2)
                ypend = None  # deferred fc2 matmul, same PE-stall dodge as emit_w
                for m in range(MC):
                    yT_ps = pp.tile(
                        [P, PW], f32, name="yT",
                        tag=("msgT0" if m % 2 == 0 else "agg1"), bufs=2,
                    )
                    for k in range(KC):
                        nc.tensor.matmul(
                            yT_ps[:], lhsT=fc1_sb[:, (k * MC + m) * P : (k * MC + m + 1) * P],
                            rhs=xcatT[k][:, :PW], start=(k == 0), stop=(k == KC - 1),
                        )
                    yTr = wp.tile([P, 2 * P], f32r, name="yTr", tag="yTr", bufs=2)
                    nc.scalar.activation(
                        yTr[:, :PW], yT_ps[:], mybir.ActivationFunctionType.Relu,
                        bias=fc1b_sb[:, m : m + 1], scale=1.0,
                    )
                    if ypend is not None:
                        nc.tensor.matmul(
                            z_ps[:], lhsT=fc2_sb[:, ypend[1] : ypend[1] + 1],
                            rhs=ypend[0][:, :PW], start=(ypend[1] == 0), stop=False,
                        )
                    ypend = (yTr, m)
                nc.tensor.matmul(
                    z_ps[:], lhsT=fc2_sb[:, ypend[1] : ypend[1] + 1],
                    rhs=ypend[0][:, :PW], start=False, stop=True,
                )
                zs = wp.tile([1, 2 * P], f32, name="zs", tag="zs", bufs=2)
                nc.scalar.activation(
                    zs[:, :PW], z_ps[:], mybir.ActivationFunctionType.Sigmoid,
                    bias=meta["fc2b"], scale=1.0,
                )
                nc.sync.dma_start(
                    out_t.ap()[q0 * P : q0 * P + PW, :], zs[:, :PW]
                )
    return nc


_NC_CACHE = []


def kernel(**inputs):
    from concourse import bass_utils

    meta, in_maps = _preprocess(inputs)
    key = (meta["N"], meta["H"], meta["R"], meta["TC"], meta["Q"],
           tuple(int(x) for x in meta["nch"]))
    if _NC_CACHE and _NC_CACHE[0][0] == key:
        nc = _NC_CACHE[0][1]
    else:
        nc = _build(meta)
        nc.compile()
        _NC_CACHE[:] = [(key, nc)]
    res = bass_utils.run_bass_kernel_spmd(nc, in_maps, core_ids=list(range(NCORES)))
    out = np.concatenate([res.results[c]["out"] for c in range(NCORES)], axis=0)
    return out.astype(np.float32)


# revision 33
# speedup vs baseline: 3.2399x; 1.0020x over previous
"""Trainium2 Bass kernel for the DTI R-GCN (bdd) model, 8 NeuronCores.

Strategy (SPMD, one program, per-core data):
  - dst-shard the graph: core c owns nodes [c*2500, (c+1)*2500); host routes
    each edge to its dst owner and buckets it into (dst-tile, relation) cells,
    tiles of 256 dst nodes.
  - node features are bf16 in a piece-contiguous shared table (row =
    (piece*8 + core)*640 + pos) so each AllGather piece's output is a
    contiguous slice written by the collective directly, and both layers'
    gathers (plus the head's) read it with one host-remapped index space.
  - gathers use InstDMAGatherAnt (gpsimd.dma_gather, mlp Q7 library): one
    instruction fetches up to 48 chunks x 128 rows via an int16 index list,
    amortizing the ~1us SWDGE fixed cost; multi-packet mode for >1024 rows.
  - per 128-edge chunk: a scatter matrix S[e, d] = norm_e * (iota == dstloc_e)
    (one bf16 tensor_scalar) and two bf16 matmuls xg^T @ S accumulate the
    transposed per-(tile, rel) aggregate aggT[feat half, dst] in fp32 PSUM.
    Pad slots use src row 0 with norm 0 (contribution exactly zero).
  - everything downstream of the edge aggregation runs in float32r (tf32-ish,
    1 cycle/row at 256-wide moving dim -- same PE cost as bf16, ~16x less
    rounding error): per (cell, rel, half) one matmul msgT[of, 256 nodes] +=
    wblk^T aggT accumulates transposed messages; the self-loop joins the same
    PSUM accumulation with lhsT=loop_w slices and rhs=x^T (host-pretransposed
    f32 for layer 1, the resident SBUF h1T written by layer 1 for layer 2).
    The W-apply of relation r is emitted after the scatter of r+1 so PE never
    waits on the aggT PSUM->SBUF copy.
  - bias lands via per-partition tensor_scalar into h1T (resident, f32r) or a
    transient h2T tile; PE transposes produce the row-layout table tiles
    (bf16 for h1, f32 for h2 feeding the head) which are DMA'd to agin and
    AllGather'd in 4 pieces fired as soon as their tiles are stored.
  - MLP head (f32r): ONE dma_gather fetches drug+target rows for all pairs
    from the f32 h2 table; fc1/fc2 run 256 pairs per matmul sweep.

The program is built fresh per invocation from the actual inputs (host does
all index preprocessing; trip counts are data-dependent but identical across
cores by padding cells to the max chunk count over cores).
"""
import sys

sys.path.insert(0, "/opt/trn_rl_repo")
import numpy as np
import ml_dtypes

BF16 = ml_dtypes.bfloat16
P = 128
NCORES = 8
NPIECE = 4  # AllGather pieces per layer; full table is piece-contiguous


def _groups(TC):
    """dma_gather group boundaries over chunk columns: small leading groups
    so PE starts quickly, then big groups to amortize instruction overhead."""
    out = []
    g0 = 0
    for gw in (4, 8, 16, 32):
        if g0 >= TC:
            return out
        gw = min(gw, TC - g0)
        out.append((g0, gw))
        g0 += gw
    while g0 < TC:
        gw = min(48, TC - g0)
        out.append((g0, gw))
        g0 += gw
    return out


def _wrap_idx(cols):
    """[P, K] int chunk-column indices -> [128, K*8] int16 dma_gather layout:
    linear i = c*128 + p; wrapped[p', s] = linear[s*16 + p' % 16]."""
    K = cols.shape[1]
    lin = cols.T.reshape(-1)  # [K*128]
    w16 = lin.reshape(K * 8, 16).T  # [16, K*8]
    return np.tile(w16, (8, 1)).astype(np.int16)


def _preprocess(inputs, ncores=NCORES):
    node_ids = np.asarray(inputs["node_ids"])
    src = np.asarray(inputs["src"])
    dst = np.asarray(inputs["dst"])
    etype = np.asarray(inputs["etype"])
    norm = np.asarray(inputs["norm"]).reshape(-1)
    emb = np.asarray(inputs["emb"], dtype=np.float32)
    drugs = np.asarray(inputs["drugs_index"])
    targets = np.asarray(inputs["targets_index"])

    N = node_ids.shape[0]
    H = emb.shape[1]
    R = int(inputs["w1"].shape[0])
    PAIRS = drugs.shape[0]
    assert N % ncores == 0 and PAIRS % ncores == 0
    NOWN = N // ncores
    TILES = -(-NOWN // P)
    PADN = TILES * P
    PPC = PAIRS // ncores
    assert PPC % P == 0
    Q = PPC // P

    TILE2 = 2 * P  # dst nodes per aggregation cell (segments on matmul free dim)
    T2 = -(-NOWN // TILE2)
    NCELL = T2 * R
    owner = dst // NOWN
    d_local = dst - owner * NOWN
    t_of_e = d_local // TILE2
    dstloc_of_e = (d_local % TILE2).astype(np.float32)
    cell_of_e = t_of_e * R + etype

    # global row in the piece-contiguous shared table: each AllGather piece's
    # output [(p*ncores + c)*PROWS + pos] is a contiguous slice, so the
    # collective writes it directly (no re-fold) and gathers read it directly.
    PROWS = PADN // NPIECE

    def grow(i):
        c = i // NOWN
        n = i - c * NOWN
        p = n // PROWS
        pos = n - p * PROWS
        return ((p * ncores + c) * PROWS + pos).astype(np.int32)

    gsrc = grow(src)

    counts = np.zeros((ncores, NCELL), np.int64)
    for c in range(ncores):
        counts[c] = np.bincount(cell_of_e[owner == c], minlength=NCELL)
    nch = -(-counts.max(axis=0) // P)  # chunks per cell (0 if empty everywhere)
    chunk_start = np.zeros(NCELL, np.int64)
    chunk_start[1:] = np.cumsum(nch)[:-1]
    TC = int(nch.sum())

    # pad slots: src row 0 (valid gather), norm 0 -> zero contribution
    srcT = np.zeros((ncores, P, TC), np.int32)
    dstlocT = np.zeros((ncores, P, TC), np.float32)
    normT = np.zeros((ncores, P, TC), np.float32)
    for c in range(ncores):
        m = owner == c
        eidx = np.where(m)[0]
        cell = cell_of_e[eidx]
        order = np.argsort(cell, kind="stable")
        eidx = eidx[order]
        cell = cell[order]
        cstart = np.zeros(NCELL, np.int64)
        cstart[1:] = np.cumsum(counts[c])[:-1]
        rank = np.arange(len(eidx)) - cstart[cell]
        col = chunk_start[cell] + rank // P
        part = rank % P
        srcT[c, part, col] = gsrc[eidx]
        dstlocT[c, part, col] = dstloc_of_e[eidx]
        normT[c, part, col] = norm[eidx]

    # host-side embedding lookup into the piece-contiguous bf16 table
    h0f = emb[node_ids].astype(np.float32)  # [N, H]
    h0 = h0f.astype(BF16)
    NFULL = ncores * PADN
    h0full = np.zeros((NFULL, H), BF16)
    h0full[grow(np.arange(N))] = h0

    # per-core pretransposed own features (local node order, f32) for the
    # layer-1 self-loop: x0ownT[p, h*PADN + n] = x0own[n, h*P + p]
    x0ownT = np.zeros((ncores, P, 2 * PADN), np.float32)
    for c in range(ncores):
        xo = np.zeros((PADN, H), np.float32)
        xo[:NOWN] = h0f[c * NOWN : (c + 1) * NOWN]
        t = np.ascontiguousarray(xo.T).reshape(2, P, PADN)
        x0ownT[c] = t.transpose(1, 0, 2).reshape(P, 2 * PADN)

    # gather index tables in dma_gather wrapped-int16 layout, per group
    srcW = np.zeros((ncores, P, TC * 8), np.int16)
    for c in range(ncores):
        for g0, gw in _groups(TC):
            srcW[c, :, g0 * 8 : (g0 + gw) * 8] = _wrap_idx(srcT[c][:, g0 : g0 + gw])

    # head pair indices, remapped to the shared-table layout, drug/target
    # interleaved: chunk col 2q = drugs of pair-chunk q, col 2q+1 = targets
    dtW = np.zeros((ncores, P, 2 * Q * 8), np.int16)
    for c in range(ncores):
        dtT = np.zeros((P, 2 * Q), np.int32)
        d = grow(drugs[c * PPC : (c + 1) * PPC]).reshape(Q, P)
        t = grow(targets[c * PPC : (c + 1) * PPC]).reshape(Q, P)
        dtT[:, 0::2] = d.T
        dtT[:, 1::2] = t.T
        dtW[c] = _wrap_idx(dtT)

    # relation block weights as lhsT [if_local, of_local] per (layer, rel, half)
    B = int(inputs["w1"].shape[1])
    si = H // B
    hb = P // si  # blocks per half
    wblk = np.zeros((2, R, 2, P, P), np.float32)
    for l, W in enumerate([inputs["w1"], inputs["w2"]]):
        W = np.asarray(W, np.float32)
        for r in range(R):
            for h in range(2):
                for bb in range(hb):
                    b = hb * h + bb
                    wblk[l, r, h, bb * si : (bb + 1) * si, bb * si : (bb + 1) * si] = W[r, b]
    wblk_in = wblk.transpose(3, 0, 1, 2, 4).reshape(P, 2 * R * 2 * P).copy()

    loopw = np.stack(
        [np.asarray(inputs["loop_w1"], np.float32), np.asarray(inputs["loop_w2"], np.float32)]
    )  # [2, H, H]
    loopw_in = loopw.reshape(2, 2, P, H).transpose(2, 0, 1, 3).reshape(P, 2 * 2 * H).copy()

    # bias as per-partition columns for the transposed pipeline:
    # biasT[p, l*2 + h] = b_l[h*128 + p]
    biasT_in = np.zeros((P, 4), np.float32)
    for l, b in enumerate([inputs["b1"], inputs["b2"]]):
        b = np.asarray(b, np.float32)
        for h in range(2):
            biasT_in[:, l * 2 + h] = b[h * P : (h + 1) * P]

    d2 = 2 * H
    KC = d2 // P  # fc1 contraction chunks
    MC = d2 // P  # fc1 output chunks
    fc1_in = (
        np.asarray(inputs["fc1_W"], np.float32)
        .reshape(KC, P, MC, P)
        .transpose(1, 0, 2, 3)
        .reshape(P, KC * MC * P)
        .copy()
    )
    fc1b_in = np.asarray(inputs["fc1_b"], np.float32).reshape(MC, P).T.copy()
    fc2_in = np.asarray(inputs["fc2_W"], np.float32).reshape(MC, P).T.copy()
    fc2b = float(np.asarray(inputs["fc2_b"]).reshape(-1)[0])

    iota = np.tile(np.arange(2 * P, dtype=np.float32), (P, 1)).astype(BF16)

    meta = dict(
        N=N, H=H, R=R, NOWN=NOWN, TILES=TILES, PADN=PADN, T2=T2, NCELL=NCELL,
        TC=TC, Q=Q, KC=KC, MC=MC, nch=nch, chunk_start=chunk_start, fc2b=fc2b,
    )
    shared = dict(
        h0full=h0full, iota=iota, wblk=wblk_in, loopw=loopw_in, biasT=biasT_in,
        fc1=fc1_in, fc1b=fc1b_in, fc2=fc2_in,
    )
    in_maps = []
    for c in range(ncores):
        m = dict(shared)
        m.update(
            srcW=srcW[c], dstlocT=dstlocT[c], normT=normT[c],
            x0ownT=x0ownT[c], dtW=dtW[c],
        )
        in_maps.append(m)
    return meta, in_maps


def _build(meta, ncores=NCORES, single=False):
    from concourse import bass, mybir, tile, bacc, library_config
    from concourse.masks import make_identity

    N, H, R = meta["N"], meta["H"], meta["R"]
    TILES, PADN, TC, Q = meta["TILES"], meta["PADN"], meta["TC"], meta["Q"]
    T2 = meta["T2"]
    KC, MC = meta["KC"], meta["MC"]
    nch, chunk_start = meta["nch"], meta["chunk_start"]
    NFULL = NCORES * PADN  # same table shape in both builds
    f32 = mybir.dt.float32
    f32r = mybir.dt.float32r
    bf = mybir.dt.bfloat16
    i16 = mybir.dt.int16

    assert PADN % NPIECE == 0 and TILES % NPIECE == 0
    PROWS = PADN // NPIECE
    TPP = TILES // NPIECE

    nc = bacc.Bacc(
        "TRN2", target_bir_lowering=False, debug=False,
        num_devices=(1 if single else ncores),
    )

    h0full_t = nc.dram_tensor("h0full", [NFULL, H], bf, kind="ExternalInput")
    srcW_t = nc.dram_tensor("srcW", [P, TC * 8], i16, kind="ExternalInput")
    dstlocT_t = nc.dram_tensor("dstlocT", [P, TC], f32, kind="ExternalInput")
    normT_t = nc.dram_tensor("normT", [P, TC], f32, kind="ExternalInput")
    x0ownT_t = nc.dram_tensor("x0ownT", [P, 2 * PADN], f32r, kind="ExternalInput")
    dtW_t = nc.dram_tensor("dtW", [P, 2 * Q * 8], i16, kind="ExternalInput")
    iota_t = nc.dram_tensor("iota", [P, 2 * P], bf, kind="ExternalInput")
    wblk_t = nc.dram_tensor("wblk", [P, 2 * R * 2 * P], f32r, kind="ExternalInput")
    loopw_t = nc.dram_tensor("loopw", [P, 2 * 2 * H], f32r, kind="ExternalInput")
    biasT_t = nc.dram_tensor("biasT", [P, 4], f32, kind="ExternalInput")
    fc1_t = nc.dram_tensor("fc1", [P, KC * MC * P], f32r, kind="ExternalInput")
    fc1b_t = nc.dram_tensor("fc1b", [P, MC], f32, kind="ExternalInput")
    fc2_t = nc.dram_tensor("fc2", [P, MC], f32r, kind="ExternalInput")
    out_t = nc.dram_tensor("out", [Q * P, 1], f32, kind="ExternalOutput")

    with tile.TileContext(nc) as tc:
        with (
            tc.tile_pool(name="const", bufs=1) as cp,
            tc.tile_pool(name="work", bufs=1) as wp,
            tc.tile_pool(name="ps", bufs=1, space="PSUM") as pp,
        ):
            # layer tables: h1 bf16 (gathered by layer-2 edges), h2 f32
            # (gathered only by the head; precision matters there)
            tdt = {1: bf, 2: f32r}
            agin = {}
            agout = {}
            for li in (1, 2):
                agin[li] = nc.dram_tensor(
                    f"h{li}_agin", [PADN, H], tdt[li], kind="Internal"
                ).ap()
                agout[li] = nc.dram_tensor(
                    f"h{li}_agout", [NFULL, H], tdt[li], kind="Internal",
                    addr_space=("Local" if single else "Shared"),
                ).ap()

            # ---- resident constants (ordered so the first gather + first
            # self-loop are not queued behind multi-MB weight loads) ----
            srcW_sb = cp.tile([P, TC * 8], i16, name="srcW_sb")
            SHEAD = min(TC, 64) * 8
            nc.sync.dma_start(srcW_sb[:, :SHEAD], srcW_t.ap()[:, :SHEAD])
            dstlocT = cp.tile([P, TC], f32, name="dstlocT")
            nc.sync.dma_start(dstlocT[:], dstlocT_t.ap()[:])
            normT = cp.tile([P, TC], f32, name="normT")
            nc.sync.dma_start(normT[:], normT_t.ap()[:])
            iota_sb = cp.tile([P, 2 * P], bf, name="iota_sb")
            nc.sync.dma_start(iota_sb[:], iota_t.ap()[:])
            loopw_sb = cp.tile([P, 2 * 2 * H], f32r, name="loopw_sb")
            nc.sync.dma_start(loopw_sb[:], loopw_t.ap()[:])
            biasT_sb = cp.tile([P, 4], f32, name="biasT_sb")
            nc.sync.dma_start(biasT_sb[:], biasT_t.ap()[:])
            # first self-loop slices must not queue behind the 4MB wblk DMA
            xsl_warm = {}
            for t2w in (0, 1):
                for hw_ in (0, 1):
                    xslw = wp.tile([P, 2 * P], f32r, name="xsl", tag="xsl", bufs=6)
                    nc.sync.dma_start(
                        xslw[:],
                        x0ownT_t.ap()[:, hw_ * PADN + t2w * 2 * P : hw_ * PADN + (t2w + 1) * 2 * P],
                    )
                    xsl_warm[(t2w, hw_)] = xslw
            # wblk in quarters (layer-0 rels 0-7 first, layer-1 tail last)
            # so W-applies get their slices without one 4MB DMA monopolizing
            # the engines ahead of the early edge gathers
            wblk_sb = cp.tile([P, 2 * R * 2 * P], f32r, name="wblk_sb")
            WQ = R * 2 * P // 2
            nc.sync.dma_start(wblk_sb[:, :WQ], wblk_t.ap()[:, :WQ])
            nc.sync.dma_start(srcW_sb[:, SHEAD:], srcW_t.ap()[:, SHEAD:])
            for wq in range(1, 4):
                nc.sync.dma_start(
                    wblk_sb[:, wq * WQ : (wq + 1) * WQ],
                    wblk_t.ap()[:, wq * WQ : (wq + 1) * WQ],
                )
            # head constants: emitted late (only needed after both layers)
            dtW_sb = cp.tile([P, 2 * Q * 8], i16, name="dtW_sb")
            fc1_sb = cp.tile([P, KC * MC * P], f32r, name="fc1_sb")
            fc1b_sb = cp.tile([P, MC], f32, name="fc1b_sb")
            fc2_sb = cp.tile([P, MC], f32r, name="fc2_sb")
            ident_f = cp.tile([P, P], f32, name="ident_f")
            make_identity(nc, ident_f[:])
            ident = cp.tile([P, P], f32r, name="ident")
            nc.vector.tensor_copy(ident[:], ident_f[:])
            nc.gpsimd.load_library(library_config.mlp)  # Q7 dma_gather ucode
            # resident transposed layer-1 activations (written by layer 1,
            # self-loop rhs for layer 2): h1T[p, h*PADN + n] = h1[n, h*P + p]
            h1T_sb = cp.tile([P, 2 * PADN], f32r, name="h1T_sb")

            def wblk_ap(l, r, h):
                o = ((l * R + r) * 2 + h) * P
                return wblk_sb[:, o : o + P]

            def loopw_ap(l, h):
                o = (l * 2 + h) * H
                return loopw_sb[:, o : o + H]

            def emit_ag_piece(li, pi):
                row0 = pi * PROWS
                orow0 = pi * ncores * PROWS
                if single:
                    nc.sync.dma_start(
                        agout[li][orow0 : orow0 + PROWS, :],
                        agin[li][row0 : row0 + PROWS, :],
                    )
                    return
                nc.gpsimd.collective_compute(
                    "AllGather", mybir.AluOpType.bypass,
                    replica_groups=[list(range(ncores))],
                    ins=[agin[li][row0 : row0 + PROWS, :]],
                    outs=[agout[li][orow0 : orow0 + ncores * PROWS, :]],
                )

            x0ownT_cache = {}
            groups = _groups(TC)
            GBMAX = max(gw for _, gw in groups)
            col2group = {}
            for gi, (g0, gw) in enumerate(groups):
                for c in range(g0, g0 + gw):
                    col2group[c] = gi

            def emit_w(l, sbs, r, last_rel, msgT):
                for h in range(2):
                    nc.tensor.matmul(
                        msgT[h][:], lhsT=wblk_ap(l, r, h), rhs=sbs[h][:],
                        start=False, stop=last_rel,
                    )

            def layer(l, xsrc_ap, xT_fn, li, hT_dst):
                group_tiles = {}

                def get_xg(col):
                    gi = col2group[col]
                    if gi not in group_tiles:
                        g0, gw = groups[gi]
                        xg = wp.tile([P, GBMAX * H], bf, name="xg", tag="xg", bufs=3)
                        dst3 = xg[:, : gw * H].rearrange("p (c w) -> p c w", w=H)
                        nc.gpsimd.dma_gather(
                            dst3, xsrc_ap, srcW_sb[:, g0 * 8 : (g0 + gw) * 8],
                            gw * P, gw * P, H, single_packet=False,
                        )
                        group_tiles[gi] = xg
                    return group_tiles[gi], col - groups[col2group[col]][0]

                pend_out = None  # deferred transpose/store closure of prev t2
                for t2 in range(T2):
                    rels = [r for r in range(R) if nch[t2 * R + r] > 0]
                    msgT = {}
                    for h in range(2):
                        msgT[h] = pp.tile(
                            [P, 2 * P], f32, name=f"msgT{h}", tag=f"msgT{h}", bufs=2
                        )
                    # self-loop starts the msgT accumulation:
                    # msgT[ho][of, n] += sum_f loopw[f, of] * xT[f, n]
                    for h in range(2):
                        xT = xT_fn(t2, h)
                        for ho in range(2):
                            nc.tensor.matmul(
                                msgT[ho][:],
                                lhsT=loopw_ap(l, h)[:, ho * P : (ho + 1) * P],
                                rhs=xT, start=(h == 0), stop=(h == 1 and not rels),
                            )
                    if pend_out is not None:
                        pend_out()
                        pend_out = None
                    pend = None  # deferred W-apply (aggT sb tiles, rel, last?)
                    Spend = None  # (r, cs, n, S tiles) built one cell ahead

                    def build_S(r):
                        cell = t2 * R + r
                        cs = int(chunk_start[cell])
                        n = int(nch[cell])
                        Ss = []
                        for ci in range(n):
                            col = cs + ci
                            S = wp.tile([P, 2 * P], bf, name="S", tag="S", bufs=8)
                            nc.vector.tensor_scalar(
                                out=S[:], in0=iota_sb[:],
                                scalar1=dstlocT[:, col : col + 1],
                                scalar2=normT[:, col : col + 1],
                                op0=mybir.AluOpType.is_equal, op1=mybir.AluOpType.mult,
                            )
                            Ss.append(S)
                        return (r, cs, n, Ss)

                    def run_cell(ri, spec):
                        nonlocal pend
                        r, cs, n, Ss = spec
                        aggT_ps = [
                            pp.tile([P, 2 * P], f32, name=f"agg{h}", tag=f"agg{h}", bufs=2)
                            for h in range(2)
                        ]
                        for ci in range(n):
                            col = cs + ci
                            xg, off = get_xg(col)
                            for h in range(2):
                                nc.tensor.matmul(
                                    aggT_ps[h][:],
                                    lhsT=xg[:, off * H + h * P : off * H + (h + 1) * P],
                                    rhs=Ss[ci][:], start=(ci == 0), stop=(ci == n - 1),
                                )
                        sbs = []
                        for h in range(2):
                            aggT_sb = wp.tile(
                                [P, 2 * P], f32r, name=f"aggsb{h}", tag=f"aggsb{h}", bufs=2
                            )
                            if (2 * ri + h) % 3 == 0:
                                nc.vector.tensor_copy(aggT_sb[:], aggT_ps[h][:])
                            else:
                                nc.scalar.copy(aggT_sb[:], aggT_ps[h][:])
                            sbs.append(aggT_sb)
                        if pend is not None:
                            emit_w(l, *pend)
                        pend = (sbs, r, ri == len(rels) - 1, msgT)

                    for ri, r in enumerate(rels):
                        spec = build_S(r)
                        if Spend is not None:
                            run_cell(ri - 1, Spend)
                        Spend = spec
                    if Spend is not None:
                        run_cell(len(rels) - 1, Spend)
                    if pend is not None:
                        emit_w(l, *pend)

                    # bias -> hT tiles (f32r); transposes/stores are deferred
                    # past the next t2's self-loop so PE doesn't wait on DVE
                    hTs = {}
                    for h in range(2):
                        hT = hT_dst(t2, h)
                        bcol = biasT_sb[:, l * 2 + h : l * 2 + h + 1]
                        if h == 0:
                            nc.vector.tensor_scalar(
                                out=hT, in0=msgT[h][:], scalar1=bcol,
                                scalar2=None, op0=mybir.AluOpType.add,
                            )
                        else:
                            nc.scalar.add(hT, msgT[h][:], bcol)
                        hTs[h] = hT

                    def make_out(t2=t2, hTs=hTs):
                        def go():
                            for si_ in range(2):
                                st = 2 * t2 + si_
                                out_sb = wp.tile(
                                    [P, H], tdt[li], name="outsb", tag="outsb", bufs=3
                                )
                                for h in range(2):
                                    tp = pp.tile(
                                        [P, P], f32r, name="tp", tag="agg0", bufs=2
                                    )
                                    nc.tensor.transpose(
                                        tp[:], hTs[h][:, si_ * P : (si_ + 1) * P],
                                        ident[:],
                                    )
                                    eng = nc.vector if h == 0 else nc.scalar
                                    if h == 0:
                                        nc.vector.tensor_copy(
                                            out_sb[:, h * P : (h + 1) * P], tp[:]
                                        )
                                    else:
                                        nc.scalar.copy(
                                            out_sb[:, h * P : (h + 1) * P], tp[:]
                                        )
                                nc.sync.dma_start(
                                    agin[li][st * P : (st + 1) * P, :], out_sb[:]
                                )
                                if (st + 1) % TPP == 0:
                                    emit_ag_piece(li, (st + 1) // TPP - 1)

                        return go

                    pend_out = make_out()
                pend_out()

            # xT providers return [128 (feat half h), 256 nodes] f32r slices

            x0ownT_cache.update(xsl_warm)

            def xT_l1(t2, h):
                if (t2, h) not in x0ownT_cache:
                    xsl = wp.tile([P, 2 * P], f32r, name="xsl", tag="xsl", bufs=6)
                    nc.sync.dma_start(
                        xsl[:],
                        x0ownT_t.ap()[:, h * PADN + t2 * 2 * P : h * PADN + (t2 + 1) * 2 * P],
                    )
                    x0ownT_cache[(t2, h)] = xsl
                return x0ownT_cache[(t2, h)][:]

            def xT_l2(t2, h):
                return h1T_sb[:, h * PADN + t2 * 2 * P : h * PADN + (t2 + 1) * 2 * P]

            def hT_dst_l1(t2, h):
                return h1T_sb[:, h * PADN + t2 * 2 * P : h * PADN + (t2 + 1) * 2 * P]

            h2T_tiles = {}

            def hT_dst_l2(t2, h):
                hv = wp.tile([P, 2 * P], f32r, name="h2T", tag="h2T", bufs=4)
                h2T_tiles[(t2, h)] = hv
                return hv[:]

            layer(0, h0full_t.ap()[:], xT_l1, 1, hT_dst_l1)
            nc.sync.dma_start(dtW_sb[:], dtW_t.ap()[:])
            nc.sync.dma_start(fc1_sb[:], fc1_t.ap()[:])
            nc.sync.dma_start(fc1b_sb[:], fc1b_t.ap()[:])
            nc.sync.dma_start(fc2_sb[:], fc2_t.ap()[:])
            layer(1, agout[1][:], xT_l2, 2, hT_dst_l2)

            # ---- MLP head over this core's Q*P pairs (f32r, 256-pair sweeps)
            xcat_big = wp.tile([P, 2 * Q * H], f32r, name="xcat_big", tag="xcat", bufs=1)
            HG = 2 * Q // 2 if Q > 1 else 2 * Q  # chunks per head gather
            for hg0 in range(0, 2 * Q, HG):
                nc.gpsimd.dma_gather(
                    xcat_big[:, hg0 * H : (hg0 + HG) * H].rearrange(
                        "p (c w) -> p c w", w=H
                    ),
                    agout[2][:], dtW_sb[:, hg0 * 8 : (hg0 + HG) * 8],
                    HG * P, HG * P, H, single_packet=False,
                )
            qblocks = []
            q0 = 0
            while q0 < Q:
                nq = min(2, Q - q0)
                qblocks.append((q0, nq))
                q0 += nq
            for q0, nq in qblocks:
                PW = nq * P  # pairs in this sweep
                # pair j: q = q0 + j//128, p = j%128
                # xcatT[k][f, j]: f-chunk k of [drug(0:KC/2) | target] halves
                xcatT = []
                for k in range(KC):
                    xcT = wp.tile([P, 2 * P], f32r, name=f"xcT{k}", tag=f"xcT{k}", bufs=1)
                    for jh in range(nq):
                        q = q0 + jh
                        c = 2 * q + (1 if k >= KC // 2 else 0)
                        off = c * H + (k % (KC // 2)) * P
                        tp = pp.tile([P, P], f32r, name="tpm", tag="agg0", bufs=2)
                        nc.tensor.transpose(
                            tp[:], xcat_big[:, off : off + P], ident[:]
                        )
                        if jh == 0:
                            nc.vector.tensor_copy(xcT[:, jh * P : (jh + 1) * P], tp[:])
                        else:
                            nc.scalar.copy(xcT[:, jh * P : (jh + 1) * P], tp[:])
                    xcatT.append(xcT)
                z_ps = pp.tile([1, PW], f32, name="z", tag="msgT1", bufs=2)
                ypend = None  # deferred fc2 matmul, same PE-stall dodge as emit_w
                for m in range(MC):
                    yT_ps = pp.tile(
                        [P, PW], f32, name="yT",
                        tag=("msgT0" if m % 2 == 0 else "agg1"), bufs=2,
                    )
                    for k in range(KC):
                        nc.tensor.matmul(
                            yT_ps[:], lhsT=fc1_sb[:, (k * MC + m) * P : (k * MC + m + 1) * P],
                            rhs=xcatT[k][:, :PW], start=(k == 0), stop=(k == KC - 1),
                        )
                    yTr = wp.tile([P, 2 * P], f32r, name="yTr", tag="yTr", bufs=2)
                    nc.scalar.activation(
                        yTr[:, :PW], yT_ps[:], mybir.ActivationFunctionType.Relu,
                        bias=fc1b_sb[:, m : m + 1], scale=1.0,
                    )
                    if ypend is not None:
                        nc.tensor.matmul(
                            z_ps[:], lhsT=fc2_sb[:, ypend[1] : ypend[1] + 1],
                            rhs=ypend[0][:, :PW], start=(ypend[1] == 0), stop=False,
                        )
                    ypend = (yTr, m)
                nc.tensor.matmul(
                    z_ps[:], lhsT=fc2_sb[:, ypend[1] : ypend[1] + 1],
                    rhs=ypend[0][:, :PW], start=False, stop=True,
                )
                zs = wp.tile([1, 2 * P], f32, name="zs", tag="zs", bufs=2)
                nc.scalar.activation(
                    zs[:, :PW], z_ps[:], mybir.ActivationFunctionType.Sigmoid,
                    bias=meta["fc2b"], scale=1.0,
                )
                nc.sync.dma_start(
                    out_t.ap()[q0 * P : q0 * P + PW, :], zs[:, :PW]
                )
    return nc


_NC_CACHE = []


def kernel(**inputs):
    from concourse import bass_utils

    meta, in_maps = _preprocess(inputs)
    key = (meta["N"], meta["H"], meta["R"], meta["TC"], meta["Q"],
           tuple(int(x) for x in meta["nch"]))
    if _NC_CACHE and _NC_CACHE[0][0] == key:
        nc = _NC_CACHE[0][1]
    else:
        nc = _build(meta)
        nc.compile()
        _NC_CACHE[:] = [(key, nc)]
    res = bass_utils.run_bass_kernel_spmd(nc, in_maps, core_ids=list(range(NCORES)))
    out = np.concatenate([res.results[c]["out"] for c in range(NCORES)], axis=0)
    return out.astype(np.float32)


# revision 34
# speedup vs baseline: 3.2840x; 1.0136x over previous
"""Trainium2 Bass kernel for the DTI R-GCN (bdd) model, 8 NeuronCores.

Strategy (SPMD, one program, per-core data):
  - dst-shard the graph: core c owns nodes [c*2500, (c+1)*2500); host routes
    each edge to its dst owner and buckets it into (dst-tile, relation) cells,
    tiles of 256 dst nodes.
  - node features are bf16 in a piece-contiguous shared table (row =
    (piece*8 + core)*640 + pos) so each AllGather piece's output is a
    contiguous slice written by the collective directly, and both layers'
    gathers (plus the head's) read it with one host-remapped index space.
  - gathers use InstDMAGatherAnt (gpsimd.dma_gather, mlp Q7 library): one
    instruction fetches up to 48 chunks x 128 rows via an int16 index list,
    amortizing the ~1us SWDGE fixed cost; multi-packet mode for >1024 rows.
  - per 128-edge chunk: a scatter matrix S[e, d] = norm_e * (iota == dstloc_e)
    (one bf16 tensor_scalar) and two bf16 matmuls xg^T @ S accumulate the
    transposed per-(tile, rel) aggregate aggT[feat half, dst] in fp32 PSUM.
    Pad slots use src row 0 with norm 0 (contribution exactly zero).
  - everything downstream of the edge aggregation runs in float32r (tf32-ish,
    1 cycle/row at 256-wide moving dim -- same PE cost as bf16, ~16x less
    rounding error): per (cell, rel, half) one matmul msgT[of, 256 nodes] +=
    wblk^T aggT accumulates transposed messages; the self-loop joins the same
    PSUM accumulation with lhsT=loop_w slices and rhs=x^T (host-pretransposed
    f32 for layer 1, the resident SBUF h1T written by layer 1 for layer 2).
    The W-apply of relation r is emitted after the scatter of r+1 so PE never
    waits on the aggT PSUM->SBUF copy.
  - bias lands via per-partition tensor_scalar into h1T (resident, f32r) or a
    transient h2T tile; PE transposes produce the row-layout table tiles
    (bf16 for h1, f32 for h2 feeding the head) which are DMA'd to agin and
    AllGather'd in 4 pieces fired as soon as their tiles are stored.
  - MLP head (f32r): ONE dma_gather fetches drug+target rows for all pairs
    from the f32 h2 table; fc1/fc2 run 256 pairs per matmul sweep.

The program is built fresh per invocation from the actual inputs (host does
all index preprocessing; trip counts are data-dependent but identical across
cores by padding cells to the max chunk count over cores).
"""
import sys

sys.path.insert(0, "/opt/trn_rl_repo")
import numpy as np
import ml_dtypes

BF16 = ml_dtypes.bfloat16
P = 128
NCORES = 8
NPIECE = 4  # AllGather pieces per layer; full table is piece-contiguous


def _groups(TC):
    """dma_gather group boundaries over chunk columns: small leading groups
    so PE starts quickly, then big groups to amortize instruction overhead."""
    out = []
    g0 = 0
    for gw in (4, 8, 16, 32):
        if g0 >= TC:
            return out
        gw = min(gw, TC - g0)
        out.append((g0, gw))
        g0 += gw
    while g0 < TC:
        gw = min(48, TC - g0)
        out.append((g0, gw))
        g0 += gw
    return out


def _wrap_idx(cols):
    """[P, K] int chunk-column indices -> [128, K*8] int16 dma_gather layout:
    linear i = c*128 + p; wrapped[p', s] = linear[s*16 + p' % 16]."""
    K = cols.shape[1]
    lin = cols.T.reshape(-1)  # [K*128]
    w16 = lin.reshape(K * 8, 16).T  # [16, K*8]
    return np.tile(w16, (8, 1)).astype(np.int16)


def _preprocess(inputs, ncores=NCORES):
    node_ids = np.asarray(inputs["node_ids"])
    src = np.asarray(inputs["src"])
    dst = np.asarray(inputs["dst"])
    etype = np.asarray(inputs["etype"])
    norm = np.asarray(inputs["norm"]).reshape(-1)
    emb = np.asarray(inputs["emb"], dtype=np.float32)
    drugs = np.asarray(inputs["drugs_index"])
    targets = np.asarray(inputs["targets_index"])

    N = node_ids.shape[0]
    H = emb.shape[1]
    R = int(inputs["w1"].shape[0])
    PAIRS = drugs.shape[0]
    assert N % ncores == 0 and PAIRS % ncores == 0
    NOWN = N // ncores
    TILES = -(-NOWN // P)
    PADN = TILES * P
    PPC = PAIRS // ncores
    assert PPC % P == 0
    Q = PPC // P

    TILE2 = 2 * P  # dst nodes per aggregation cell (segments on matmul free dim)
    T2 = -(-NOWN // TILE2)
    NCELL = T2 * R
    owner = dst // NOWN
    d_local = dst - owner * NOWN
    t_of_e = d_local // TILE2
    dstloc_of_e = (d_local % TILE2).astype(np.float32)
    cell_of_e = t_of_e * R + etype

    # global row in the piece-contiguous shared table: each AllGather piece's
    # output [(p*ncores + c)*PROWS + pos] is a contiguous slice, so the
    # collective writes it directly (no re-fold) and gathers read it directly.
    PROWS = PADN // NPIECE

    def grow(i):
        c = i // NOWN
        n = i - c * NOWN
        p = n // PROWS
        pos = n - p * PROWS
        return ((p * ncores + c) * PROWS + pos).astype(np.int32)

    gsrc = grow(src)

    counts = np.zeros((ncores, NCELL), np.int64)
    for c in range(ncores):
        counts[c] = np.bincount(cell_of_e[owner == c], minlength=NCELL)
    nch = -(-counts.max(axis=0) // P)  # chunks per cell (0 if empty everywhere)
    chunk_start = np.zeros(NCELL, np.int64)
    chunk_start[1:] = np.cumsum(nch)[:-1]
    TC = int(nch.sum())

    # pad slots: src row 0 (valid gather), norm 0 -> zero contribution
    srcT = np.zeros((ncores, P, TC), np.int32)
    dstlocT = np.zeros((ncores, P, TC), np.float32)
    normT = np.zeros((ncores, P, TC), np.float32)
    for c in range(ncores):
        m = owner == c
        eidx = np.where(m)[0]
        cell = cell_of_e[eidx]
        order = np.argsort(cell, kind="stable")
        eidx = eidx[order]
        cell = cell[order]
        cstart = np.zeros(NCELL, np.int64)
        cstart[1:] = np.cumsum(counts[c])[:-1]
        rank = np.arange(len(eidx)) - cstart[cell]
        col = chunk_start[cell] + rank // P
        part = rank % P
        srcT[c, part, col] = gsrc[eidx]
        dstlocT[c, part, col] = dstloc_of_e[eidx]
        normT[c, part, col] = norm[eidx]

    # host-side embedding lookup into the piece-contiguous bf16 table
    h0f = emb[node_ids].astype(np.float32)  # [N, H]
    h0 = h0f.astype(BF16)
    NFULL = ncores * PADN
    h0full = np.zeros((NFULL, H), BF16)
    h0full[grow(np.arange(N))] = h0

    # per-core pretransposed own features (local node order, f32) for the
    # layer-1 self-loop: x0ownT[p, h*PADN + n] = x0own[n, h*P + p]
    x0ownT = np.zeros((ncores, P, 2 * PADN), np.float32)
    for c in range(ncores):
        xo = np.zeros((PADN, H), np.float32)
        xo[:NOWN] = h0f[c * NOWN : (c + 1) * NOWN]
        t = np.ascontiguousarray(xo.T).reshape(2, P, PADN)
        x0ownT[c] = t.transpose(1, 0, 2).reshape(P, 2 * PADN)

    # gather index tables in dma_gather wrapped-int16 layout, per group
    srcW = np.zeros((ncores, P, TC * 8), np.int16)
    for c in range(ncores):
        for g0, gw in _groups(TC):
            srcW[c, :, g0 * 8 : (g0 + gw) * 8] = _wrap_idx(srcT[c][:, g0 : g0 + gw])

    # head pair indices, remapped to the shared-table layout, drug/target
    # interleaved: chunk col 2q = drugs of pair-chunk q, col 2q+1 = targets
    dtW = np.zeros((ncores, P, 2 * Q * 8), np.int16)
    for c in range(ncores):
        dtT = np.zeros((P, 2 * Q), np.int32)
        d = grow(drugs[c * PPC : (c + 1) * PPC]).reshape(Q, P)
        t = grow(targets[c * PPC : (c + 1) * PPC]).reshape(Q, P)
        dtT[:, 0::2] = d.T
        dtT[:, 1::2] = t.T
        dtW[c] = _wrap_idx(dtT)

    # relation block weights as lhsT [if_local, of_local] per (layer, rel, half)
    B = int(inputs["w1"].shape[1])
    si = H // B
    hb = P // si  # blocks per half
    wblk = np.zeros((2, R, 2, P, P), np.float32)
    for l, W in enumerate([inputs["w1"], inputs["w2"]]):
        W = np.asarray(W, np.float32)
        for r in range(R):
            for h in range(2):
                for bb in range(hb):
                    b = hb * h + bb
                    wblk[l, r, h, bb * si : (bb + 1) * si, bb * si : (bb + 1) * si] = W[r, b]
    wblk_in = wblk.transpose(3, 0, 1, 2, 4).reshape(P, 2 * R * 2 * P).copy()

    loopw = np.stack(
        [np.asarray(inputs["loop_w1"], np.float32), np.asarray(inputs["loop_w2"], np.float32)]
    )  # [2, H, H]
    loopw_in = loopw.reshape(2, 2, P, H).transpose(2, 0, 1, 3).reshape(P, 2 * 2 * H).copy()

    # bias as per-partition columns for the transposed pipeline:
    # biasT[p, l*2 + h] = b_l[h*128 + p]
    biasT_in = np.zeros((P, 4), np.float32)
    for l, b in enumerate([inputs["b1"], inputs["b2"]]):
        b = np.asarray(b, np.float32)
        for h in range(2):
            biasT_in[:, l * 2 + h] = b[h * P : (h + 1) * P]

    d2 = 2 * H
    KC = d2 // P  # fc1 contraction chunks
    MC = d2 // P  # fc1 output chunks
    fc1_in = (
        np.asarray(inputs["fc1_W"], np.float32)
        .reshape(KC, P, MC, P)
        .transpose(1, 0, 2, 3)
        .reshape(P, KC * MC * P)
        .copy()
    )
    fc1b_in = np.asarray(inputs["fc1_b"], np.float32).reshape(MC, P).T.copy()
    fc2_in = np.asarray(inputs["fc2_W"], np.float32).reshape(MC, P).T.copy()
    fc2b = float(np.asarray(inputs["fc2_b"]).reshape(-1)[0])

    iota = np.tile(np.arange(2 * P, dtype=np.float32), (P, 1)).astype(BF16)

    meta = dict(
        N=N, H=H, R=R, NOWN=NOWN, TILES=TILES, PADN=PADN, T2=T2, NCELL=NCELL,
        TC=TC, Q=Q, KC=KC, MC=MC, nch=nch, chunk_start=chunk_start, fc2b=fc2b,
    )
    shared = dict(
        h0full=h0full, iota=iota, wblk=wblk_in, loopw=loopw_in, biasT=biasT_in,
        fc1=fc1_in, fc1b=fc1b_in, fc2=fc2_in,
    )
    in_maps = []
    for c in range(ncores):
        m = dict(shared)
        m.update(
            srcW=srcW[c], dstlocT=dstlocT[c], normT=normT[c],
            x0ownT=x0ownT[c], dtW=dtW[c],
        )
        in_maps.append(m)
    return meta, in_maps


def _build(meta, ncores=NCORES, single=False):
    from concourse import bass, mybir, tile, bacc, library_config
    from concourse.masks import make_identity

    N, H, R = meta["N"], meta["H"], meta["R"]
    TILES, PADN, TC, Q = meta["TILES"], meta["PADN"], meta["TC"], meta["Q"]
    T2 = meta["T2"]
    KC, MC = meta["KC"], meta["MC"]
    nch, chunk_start = meta["nch"], meta["chunk_start"]
    NFULL = NCORES * PADN  # same table shape in both builds
    f32 = mybir.dt.float32
    f32r = mybir.dt.float32r
    bf = mybir.dt.bfloat16
    i16 = mybir.dt.int16

    assert PADN % NPIECE == 0 and TILES % NPIECE == 0
    PROWS = PADN // NPIECE
    TPP = TILES // NPIECE

    nc = bacc.Bacc(
        "TRN2", target_bir_lowering=False, debug=False,
        num_devices=(1 if single else ncores),
    )

    h0full_t = nc.dram_tensor("h0full", [NFULL, H], bf, kind="ExternalInput")
    srcW_t = nc.dram_tensor("srcW", [P, TC * 8], i16, kind="ExternalInput")
    dstlocT_t = nc.dram_tensor("dstlocT", [P, TC], f32, kind="ExternalInput")
    normT_t = nc.dram_tensor("normT", [P, TC], f32, kind="ExternalInput")
    x0ownT_t = nc.dram_tensor("x0ownT", [P, 2 * PADN], f32r, kind="ExternalInput")
    dtW_t = nc.dram_tensor("dtW", [P, 2 * Q * 8], i16, kind="ExternalInput")
    iota_t = nc.dram_tensor("iota", [P, 2 * P], bf, kind="ExternalInput")
    wblk_t = nc.dram_tensor("wblk", [P, 2 * R * 2 * P], f32r, kind="ExternalInput")
    loopw_t = nc.dram_tensor("loopw", [P, 2 * 2 * H], f32r, kind="ExternalInput")
    biasT_t = nc.dram_tensor("biasT", [P, 4], f32, kind="ExternalInput")
    fc1_t = nc.dram_tensor("fc1", [P, KC * MC * P], f32r, kind="ExternalInput")
    fc1b_t = nc.dram_tensor("fc1b", [P, MC], f32, kind="ExternalInput")
    fc2_t = nc.dram_tensor("fc2", [P, MC], f32r, kind="ExternalInput")
    out_t = nc.dram_tensor("out", [Q * P, 1], f32, kind="ExternalOutput")

    with tile.TileContext(nc) as tc:
        with (
            tc.tile_pool(name="const", bufs=1) as cp,
            tc.tile_pool(name="work", bufs=1) as wp,
            tc.tile_pool(name="ps", bufs=1, space="PSUM") as pp,
        ):
            # layer tables: h1 bf16 (gathered by layer-2 edges), h2 f32
            # (gathered only by the head; precision matters there)
            tdt = {1: bf, 2: f32r}
            agin = {}
            agout = {}
            for li in (1, 2):
                agin[li] = nc.dram_tensor(
                    f"h{li}_agin", [PADN, H], tdt[li], kind="Internal"
                ).ap()
                agout[li] = nc.dram_tensor(
                    f"h{li}_agout", [NFULL, H], tdt[li], kind="Internal",
                    addr_space=("Local" if single else "Shared"),
                ).ap()

            # ---- resident constants (ordered so the first gather + first
            # self-loop are not queued behind multi-MB weight loads) ----
            srcW_sb = cp.tile([P, TC * 8], i16, name="srcW_sb")
            SHEAD = min(TC, 64) * 8
            nc.sync.dma_start(srcW_sb[:, :SHEAD], srcW_t.ap()[:, :SHEAD])
            dstlocT = cp.tile([P, TC], f32, name="dstlocT")
            nc.sync.dma_start(dstlocT[:], dstlocT_t.ap()[:])
            normT = cp.tile([P, TC], f32, name="normT")
            nc.sync.dma_start(normT[:], normT_t.ap()[:])
            iota_sb = cp.tile([P, 2 * P], bf, name="iota_sb")
            nc.sync.dma_start(iota_sb[:], iota_t.ap()[:])
            loopw_sb = cp.tile([P, 2 * 2 * H], f32r, name="loopw_sb")
            nc.sync.dma_start(loopw_sb[:], loopw_t.ap()[:])
            biasT_sb = cp.tile([P, 4], f32, name="biasT_sb")
            nc.sync.dma_start(biasT_sb[:], biasT_t.ap()[:])
            # first self-loop slices must not queue behind the 4MB wblk DMA
            xsl_warm = {}
            for t2w in (0, 1):
                for hw_ in (0, 1):
                    xslw = wp.tile([P, 2 * P], f32r, name="xsl", tag="xsl", bufs=6)
                    nc.sync.dma_start(
                        xslw[:],
                        x0ownT_t.ap()[:, hw_ * PADN + t2w * 2 * P : hw_ * PADN + (t2w + 1) * 2 * P],
                    )
                    xsl_warm[(t2w, hw_)] = xslw
            # wblk in quarters (layer-0 rels 0-7 first, layer-1 tail last)
            # so W-applies get their slices without one 4MB DMA monopolizing
            # the engines ahead of the early edge gathers
            wblk_sb = cp.tile([P, 2 * R * 2 * P], f32r, name="wblk_sb")
            WQ = R * 2 * P // 2
            nc.sync.dma_start(wblk_sb[:, :WQ], wblk_t.ap()[:, :WQ])
            nc.sync.dma_start(srcW_sb[:, SHEAD:], srcW_t.ap()[:, SHEAD:])
            for wq in range(1, 4):
                nc.sync.dma_start(
                    wblk_sb[:, wq * WQ : (wq + 1) * WQ],
                    wblk_t.ap()[:, wq * WQ : (wq + 1) * WQ],
                )
            # head constants: emitted late (only needed after both layers)
            dtW_sb = cp.tile([P, 2 * Q * 8], i16, name="dtW_sb")
            fc1_sb = cp.tile([P, KC * MC * P], f32r, name="fc1_sb")
            fc1b_sb = cp.tile([P, MC], f32, name="fc1b_sb")
            fc2_sb = cp.tile([P, MC], f32r, name="fc2_sb")
            ident_f = cp.tile([P, P], f32, name="ident_f")
            make_identity(nc, ident_f[:])
            ident = cp.tile([P, P], f32r, name="ident")
            nc.vector.tensor_copy(ident[:], ident_f[:])
            nc.gpsimd.load_library(library_config.mlp)  # Q7 dma_gather ucode
            # resident transposed layer-1 activations (written by layer 1,
            # self-loop rhs for layer 2): h1T[p, h*PADN + n] = h1[n, h*P + p]
            h1T_sb = cp.tile([P, 2 * PADN], f32r, name="h1T_sb")

            def wblk_ap(l, r, h):
                o = ((l * R + r) * 2 + h) * P
                return wblk_sb[:, o : o + P]

            def loopw_ap(l, h):
                o = (l * 2 + h) * H
                return loopw_sb[:, o : o + H]

            def emit_ag_piece(li, pi):
                row0 = pi * PROWS
                orow0 = pi * ncores * PROWS
                if single:
                    nc.sync.dma_start(
                        agout[li][orow0 : orow0 + PROWS, :],
                        agin[li][row0 : row0 + PROWS, :],
                    )
                    return
                nc.gpsimd.collective_compute(
                    "AllGather", mybir.AluOpType.bypass,
                    replica_groups=[list(range(ncores))],
                    ins=[agin[li][row0 : row0 + PROWS, :]],
                    outs=[agout[li][orow0 : orow0 + ncores * PROWS, :]],
                )

            x0ownT_cache = {}
            groups = _groups(TC)
            GBMAX = max(gw for _, gw in groups)
            col2group = {}
            for gi, (g0, gw) in enumerate(groups):
                for c in range(g0, g0 + gw):
                    col2group[c] = gi

            def emit_w(l, sbs, r, last_rel, msgT):
                for h in range(2):
                    nc.tensor.matmul(
                        msgT[h][:], lhsT=wblk_ap(l, r, h), rhs=sbs[h][:],
                        start=False, stop=last_rel,
                    )

            def layer(l, xsrc_ap, xT_fn, li, hT_dst):
                group_tiles = {}

                def get_xg(col):
                    gi = col2group[col]
                    if gi not in group_tiles:
                        g0, gw = groups[gi]
                        xg = wp.tile([P, GBMAX * H], bf, name="xg", tag="xg", bufs=3)
                        dst3 = xg[:, : gw * H].rearrange("p (c w) -> p c w", w=H)
                        nc.gpsimd.dma_gather(
                            dst3, xsrc_ap, srcW_sb[:, g0 * 8 : (g0 + gw) * 8],
                            gw * P, gw * P, H, single_packet=False,
                        )
                        group_tiles[gi] = xg
                    return group_tiles[gi], col - groups[col2group[col]][0]

                pend_out = None  # deferred transpose/store closure of prev t2
                for t2 in range(T2):
                    rels = [r for r in range(R) if nch[t2 * R + r] > 0]
                    msgT = {}
                    for h in range(2):
                        msgT[h] = pp.tile(
                            [P, 2 * P], f32, name=f"msgT{h}", tag=f"msgT{h}", bufs=2
                        )
                    # self-loop starts the msgT accumulation:
                    # msgT[ho][of, n] += sum_f loopw[f, of] * xT[f, n]
                    for h in range(2):
                        xT = xT_fn(t2, h)
                        for ho in range(2):
                            nc.tensor.matmul(
                                msgT[ho][:],
                                lhsT=loopw_ap(l, h)[:, ho * P : (ho + 1) * P],
                                rhs=xT, start=(h == 0), stop=(h == 1 and not rels),
                            )
                    pend = None  # deferred W-apply (aggT sb tiles, rel, last?)
                    Spend = None  # (r, cs, n, S tiles) built one cell ahead

                    def build_S(r):
                        cell = t2 * R + r
                        cs = int(chunk_start[cell])
                        n = int(nch[cell])
                        Ss = []
                        for ci in range(n):
                            col = cs + ci
                            S = wp.tile([P, 2 * P], bf, name="S", tag="S", bufs=8)
                            nc.vector.tensor_scalar(
                                out=S[:], in0=iota_sb[:],
                                scalar1=dstlocT[:, col : col + 1],
                                scalar2=normT[:, col : col + 1],
                                op0=mybir.AluOpType.is_equal, op1=mybir.AluOpType.mult,
                            )
                            Ss.append(S)
                        return (r, cs, n, Ss)

                    def run_cell(ri, spec):
                        nonlocal pend
                        r, cs, n, Ss = spec
                        aggT_ps = [
                            pp.tile([P, 2 * P], f32, name=f"agg{h}", tag=f"agg{h}", bufs=2)
                            for h in range(2)
                        ]
                        for ci in range(n):
                            col = cs + ci
                            xg, off = get_xg(col)
                            for h in range(2):
                                nc.tensor.matmul(
                                    aggT_ps[h][:],
                                    lhsT=xg[:, off * H + h * P : off * H + (h + 1) * P],
                                    rhs=Ss[ci][:], start=(ci == 0), stop=(ci == n - 1),
                                )
                        sbs = []
                        for h in range(2):
                            aggT_sb = wp.tile(
                                [P, 2 * P], f32r, name=f"aggsb{h}", tag=f"aggsb{h}", bufs=2
                            )
                            if (2 * ri + h) % 3 == 0:
                                nc.vector.tensor_copy(aggT_sb[:], aggT_ps[h][:])
                            else:
                                nc.scalar.copy(aggT_sb[:], aggT_ps[h][:])
                            sbs.append(aggT_sb)
                        if pend is not None:
                            emit_w(l, *pend)
                        pend = (sbs, r, ri == len(rels) - 1, msgT)

                    for ri, r in enumerate(rels):
                        spec = build_S(r)
                        if Spend is not None:
                            run_cell(ri - 1, Spend)
                            if pend_out is not None:
                                pend_out()
                                pend_out = None
                        Spend = spec
                    if Spend is not None:
                        run_cell(len(rels) - 1, Spend)
                    if pend is not None:
                        emit_w(l, *pend)
                    if pend_out is not None:
                        pend_out()
                        pend_out = None

                    # bias -> hT tiles (f32r); transposes/stores are deferred
                    # past the next t2's self-loop so PE doesn't wait on DVE
                    hTs = {}
                    for h in range(2):
                        hT = hT_dst(t2, h)
                        bcol = biasT_sb[:, l * 2 + h : l * 2 + h + 1]
                        if h == 0:
                            nc.vector.tensor_scalar(
                                out=hT, in0=msgT[h][:], scalar1=bcol,
                                scalar2=None, op0=mybir.AluOpType.add,
                            )
                        else:
                            nc.scalar.add(hT, msgT[h][:], bcol)
                        hTs[h] = hT

                    def make_out(t2=t2, hTs=hTs):
                        def go():
                            for si_ in range(2):
                                st = 2 * t2 + si_
                                out_sb = wp.tile(
                                    [P, H], tdt[li], name="outsb", tag="outsb", bufs=3
                                )
                                for h in range(2):
                                    tp = pp.tile(
                                        [P, P], f32r, name="tp", tag="agg0", bufs=2
                                    )
                                    nc.tensor.transpose(
                                        tp[:], hTs[h][:, si_ * P : (si_ + 1) * P],
                                        ident[:],
                                    )
                                    eng = nc.vector if h == 0 else nc.scalar
                                    if h == 0:
                                        nc.vector.tensor_copy(
                                            out_sb[:, h * P : (h + 1) * P], tp[:]
                                        )
                                    else:
                                        nc.scalar.copy(
                                            out_sb[:, h * P : (h + 1) * P], tp[:]
                                        )
                                nc.sync.dma_start(
                                    agin[li][st * P : (st + 1) * P, :], out_sb[:]
                                )
                                if (st + 1) % TPP == 0:
                                    emit_ag_piece(li, (st + 1) // TPP - 1)

                        return go

                    pend_out = make_out()
                pend_out()

            # xT providers return [128 (feat half h), 256 nodes] f32r slices

            x0ownT_cache.update(xsl_warm)

            def xT_l1(t2, h):
                if (t2, h) not in x0ownT_cache:
                    xsl = wp.tile([P, 2 * P], f32r, name="xsl", tag="xsl", bufs=6)
                    nc.sync.dma_start(
                        xsl[:],
                        x0ownT_t.ap()[:, h * PADN + t2 * 2 * P : h * PADN + (t2 + 1) * 2 * P],
                    )
                    x0ownT_cache[(t2, h)] = xsl
                return x0ownT_cache[(t2, h)][:]

            def xT_l2(t2, h):
                return h1T_sb[:, h * PADN + t2 * 2 * P : h * PADN + (t2 + 1) * 2 * P]

            def hT_dst_l1(t2, h):
                return h1T_sb[:, h * PADN + t2 * 2 * P : h * PADN + (t2 + 1) * 2 * P]

            h2T_tiles = {}

            def hT_dst_l2(t2, h):
                hv = wp.tile([P, 2 * P], f32r, name="h2T", tag="h2T", bufs=4)
                h2T_tiles[(t2, h)] = hv
                return hv[:]

            layer(0, h0full_t.ap()[:], xT_l1, 1, hT_dst_l1)
            nc.sync.dma_start(dtW_sb[:], dtW_t.ap()[:])
            nc.sync.dma_start(fc1_sb[:], fc1_t.ap()[:])
            nc.sync.dma_start(fc1b_sb[:], fc1b_t.ap()[:])
            nc.sync.dma_start(fc2_sb[:], fc2_t.ap()[:])
            layer(1, agout[1][:], xT_l2, 2, hT_dst_l2)

            # ---- MLP head over this core's Q*P pairs (f32r, 256-pair sweeps)
            xcat_big = wp.tile([P, 2 * Q * H], f32r, name="xcat_big", tag="xcat", bufs=1)
            HG = 2 * Q // 2 if Q > 1 else 2 * Q  # chunks per head gather
            for hg0 in range(0, 2 * Q, HG):
                nc.gpsimd.dma_gather(
                    xcat_big[:, hg0 * H : (hg0 + HG) * H].rearrange(
                        "p (c w) -> p c w", w=H
                    ),
                    agout[2][:], dtW_sb[:, hg0 * 8 : (hg0 + HG) * 8],
                    HG * P, HG * P, H, single_packet=False,
                )
            qblocks = []
            q0 = 0
            while q0 < Q:
                nq = min(2, Q - q0)
                qblocks.append((q0, nq))
                q0 += nq
            for q0, nq in qblocks:
                PW = nq * P  # pairs in this sweep
                # pair j: q = q0 + j//128, p = j%128
                # xcatT[k][f, j]: f-chunk k of [drug(0:KC/2) | target] halves
                xcatT = []
                for k in range(KC):
                    xcT = wp.tile([P, 2 * P], f32r, name=f"xcT{k}", tag=f"xcT{k}", bufs=1)
                    for jh in range(nq):
                        q = q0 + jh
                        c = 2 * q + (1 if k >= KC // 2 else 0)
                        off = c * H + (k % (KC // 2)) * P
                        tp = pp.tile([P, P], f32r, name="tpm", tag="agg0", bufs=2)
                        nc.tensor.transpose(
                            tp[:], xcat_big[:, off : off + P], ident[:]
                        )
                        if jh == 0:
                            nc.vector.tensor_copy(xcT[:, jh * P : (jh + 1) * P], tp[:])
                        else:
                            nc.scalar.copy(xcT[:, jh * P : (jh + 1) * P], tp[:])
                    xcatT.append(xcT)
                z_ps = pp.tile([1, PW], f32, name="z", tag="msgT1", bufs=2)
                ypend = None  # deferred fc2 matmul, same PE-stall dodge as emit_w
                for m in range(MC):
                    yT_ps = pp.tile(
                        [P, PW], f32, name="yT",
                        tag=("msgT0" if m % 2 == 0 else "agg1"), bufs=2,
                    )
                    for k in range(KC):
                        nc.tensor.matmul(
                            yT_ps[:], lhsT=fc1_sb[:, (k * MC + m) * P : (k * MC + m + 1) * P],
                            rhs=xcatT[k][:, :PW], start=(k == 0), stop=(k == KC - 1),
                        )
                    yTr = wp.tile([P, 2 * P], f32r, name="yTr", tag="yTr", bufs=2)
                    nc.scalar.activation(
                        yTr[:, :PW], yT_ps[:], mybir.ActivationFunctionType.Relu,
                        bias=fc1b_sb[:, m : m + 1], scale=1.0,
                    )
                    if ypend is not None:
                        nc.tensor.matmul(
                            z_ps[:], lhsT=fc2_sb[:, ypend[1] : ypend[1] + 1],
                            rhs=ypend[0][:, :PW], start=(ypend[1] == 0), stop=False,
                        )
                    ypend = (yTr, m)
                nc.tensor.matmul(
                    z_ps[:], lhsT=fc2_sb[:, ypend[1] : ypend[1] + 1],
                    rhs=ypend[0][:, :PW], start=False, stop=True,
                )
                zs = wp.tile([1, 2 * P], f32, name="zs", tag="zs", bufs=2)
                nc.scalar.activation(
                    zs[:, :PW], z_ps[:], mybir.ActivationFunctionType.Sigmoid,
                    bias=meta["fc2b"], scale=1.0,
                )
                nc.sync.dma_start(
                    out_t.ap()[q0 * P : q0 * P + PW, :], zs[:, :PW]
                )
    return nc


_NC_CACHE = []


def kernel(**inputs):
    from concourse import bass_utils

    meta, in_maps = _preprocess(inputs)
    key = (meta["N"], meta["H"], meta["R"], meta["TC"], meta["Q"],
           tuple(int(x) for x in meta["nch"]))
    if _NC_CACHE and _NC_CACHE[0][0] == key:
        nc = _NC_CACHE[0][1]
    else:
        nc = _build(meta)
        nc.compile()
        _NC_CACHE[:] = [(key, nc)]
    res = bass_utils.run_bass_kernel_spmd(nc, in_maps, core_ids=list(range(NCORES)))
    out = np.concatenate([res.results[c]["out"] for c in range(NCORES)], axis=0)
    return out.astype(np.float32)
